# revision 1
# baseline (speedup 1.0000x reference)
"""2-layer GAT (PyG GATConv eval semantics) on 8 Trainium2 NeuronCores.

Sharding: nodes by contiguous id range (6272/core, 49 blocks of 128); edges
(with self loops) partitioned by destination core/block so segment softmax and
scatter-add stay local. Per layer, a replicated node table is gathered by
source id via the GPSIMD dma_gather ucode (per-index descriptors); attention
weights t = exp(leaky_relu(as[src]+ad[dst])) are applied in SBUF and the
per-destination-block aggregation (numerator + denominator in one pass) is a
one-hot matmul accumulated in PSUM. Layer-2's (small) node table is shared via
one AllGather. dma_gather indices are int16, so tables are split in two
halves (A: rows < 25088, B: rest) and each block's edges are grouped into
A-tiles then B-tiles, padded to uniform tile counts (TA, TB) so one SPMD
program serves all cores.
"""

import numpy as np

import concourse.bacc as bacc
import concourse.bass as bass
import concourse.mybir as mybir
import concourse.tile as tile
from concourse import library_config
from concourse.bass_utils import run_bass_kernel_spmd

N_NODES = 50000
N_CORES = 8
P_NODES = 6272                  # nodes per core (49 blocks of 128)
NPAD = P_NODES * N_CORES        # 50176
HALF = NPAD // 2                # 25088 (A/B table split, int16-safe)
NB = P_NODES // 128             # 49 destination blocks per core
TBLK = NPAD // 128              # 392 table-build blocks
D_IN = 256
H1 = 4
ROW1 = 320                      # [h(256) | as(4) | ad(4) | pad] f32, 1280B
ROW2 = 128                      # [h2(64) | as2(1) | ad2(1) | pad] f32, 512B
ROWA = 64                       # adtab row: [ad1(4) | ad2(1) | pad] f32, 256B
NEG = 0.2
EPS = 1e-16
G_BLK = 1                       # destination blocks per gather group

F32 = mybir.dt.float32
I16 = mybir.dt.int16


# ---------------------------------------------------------------- host prep

def _wrap16(vals, n_slots):
    """dma_gather index layout: index j at [j%16, j//16], replicated to all
    eight 16-partition groups."""
    a = np.zeros((16, n_slots // 16), np.int16)
    j = np.arange(len(vals))
    a[j % 16, j // 16] = vals
    return np.tile(a, (8, 1))


def preprocess_edges(edge_index):
    ei = np.asarray(edge_index).astype(np.int64)
    loops = np.arange(N_NODES, dtype=np.int64)
    src = np.concatenate([ei[0], loops])
    dst = np.concatenate([ei[1], loops])

    percore = []
    TA = TB = 1
    for c in range(N_CORES):
        lo = c * P_NODES
        m = (dst >= lo) & (dst < lo + P_NODES)
        s, d = src[m], dst[m] - lo
        blocks = []
        for b in range(NB):
            mb = (d // 128) == b
            sb, db = s[mb], d[mb]
            a_m = sb < HALF
            blocks.append((sb[a_m], db[a_m], sb[~a_m], db[~a_m]))
            TA = max(TA, int(np.ceil(len(sb[a_m]) / 128)))
            TB = max(TB, int(np.ceil(len(sb[~a_m]) / 128)))
        percore.append(blocks)

    T1 = TA + TB
    ngrp = (NB + G_BLK - 1) // G_BLK
    grp_sizes = [min(G_BLK, NB - g * G_BLK) for g in range(ngrp)]

    idxA = np.zeros((N_CORES, 128, NB * TA * 8), np.int16)
    idxB = np.zeros((N_CORES, 128, NB * TB * 8), np.int16)
    dstg = np.zeros((N_CORES, 128, NB * T1 * 8), np.int16)
    dstl = np.full((N_CORES, 128, NB * T1), 999.0, np.float32)

    for c in range(N_CORES):
        colA = colB = colG = 0
        for g, gs in enumerate(grp_sizes):
            b0 = g * G_BLK
            # group layout: all A tiles of its blocks, then all B tiles
            a_src, b_src = [], []
            a_dst, b_dst = [], []     # dst id local to core, -1 = pad slot
            for gb in range(gs):
                sa, da, sb_, db_ = percore[c][b0 + gb]
                va = np.zeros(TA * 128, np.int64)
                la = np.full(TA * 128, -1, np.int64)
                va[:len(sa)] = sa
                la[:len(sa)] = da
                a_src.append(va); a_dst.append(la)
                vb = np.zeros(TB * 128, np.int64)
                lb = np.full(TB * 128, -1, np.int64)
                vb[:len(sb_)] = sb_ - HALF
                lb[:len(sb_)] = db_
                b_src.append(vb); b_dst.append(lb)
            a_src = np.concatenate(a_src)
            b_src = np.concatenate(b_src)
            g_dst = np.concatenate(a_dst + b_dst)          # slot order A.. B..
            g_blkbase = np.concatenate(
                [np.repeat(np.arange(b0, b0 + gs), TA * 128),
                 np.repeat(np.arange(b0, b0 + gs), TB * 128)]) * 128

            nA, nB = gs * TA * 128, gs * TB * 128
            idxA[c, :, colA * 8:(colA + gs * TA) * 8] = _wrap16(a_src, nA)
            idxB[c, :, colB * 8:(colB + gs * TB) * 8] = _wrap16(b_src, nB)
            dstg[c, :, colG * 8:(colG + gs * T1) * 8] = _wrap16(
                np.where(g_dst >= 0, g_dst, 0), gs * T1 * 128)
            # one-hot block-local ids per slot (pad -> 999)
            lo_sl = np.where(g_dst >= 0, (g_dst - g_blkbase).astype(np.float32), 999.0)
            dstl[c, :, colG:colG + gs * T1] = lo_sl.reshape(gs * T1, 128).T
            colA += gs * TA
            colB += gs * TB
            colG += gs * T1
    return TA, TB, grp_sizes, idxA, idxB, dstg, dstl


def prep_weights(x, W1, att_src1, att_dst1, W2, att_src2, att_dst2):
    x = np.asarray(x, np.float32)
    W1 = np.asarray(W1, np.float32)
    W2 = np.asarray(W2, np.float32)
    As1 = np.einsum("khc,hc->kh", W1.reshape(D_IN, H1, 64), np.asarray(att_src1, np.float32))
    Ad1 = np.einsum("khc,hc->kh", W1.reshape(D_IN, H1, 64), np.asarray(att_dst1, np.float32))
    As2 = W2 @ np.asarray(att_src2, np.float32).reshape(64)
    Ad2 = W2 @ np.asarray(att_dst2, np.float32).reshape(64)
    W1ext = np.zeros((D_IN, ROW1), np.float32)
    W1ext[:, :256] = W1
    W1ext[:, 256:260] = As1
    W1ext[:, 260:264] = Ad1
    W2ext = np.zeros((D_IN, ROW2), np.float32)
    W2ext[:, :64] = W2
    W2ext[:, 64] = As2
    W2ext[:, 65] = Ad2
    xT = np.zeros((D_IN, NPAD), np.float32)
    xT[:, :N_NODES] = x.T
    return xT, W1ext, W2ext


# ------------------------------------------------------------- bass program

def build_program(TA, TB, grp_sizes, phases=6):
    import os
    ngrp_lim = int(os.environ.get("KERN_NGRP", "9999"))
    stage = int(os.environ.get("KERN_STAGE", "9"))
    T1 = TA + TB
    nc = bacc.Bacc("TRN2", target_bir_lowering=False, debug=False,
                   num_devices=N_CORES)

    xT = nc.dram_tensor("xT", [D_IN, NPAD], F32, kind="ExternalInput").ap()
    xTo = nc.dram_tensor("xT_own", [D_IN, P_NODES], F32, kind="ExternalInput").ap()
    w1d = nc.dram_tensor("W1ext", [D_IN, ROW1], F32, kind="ExternalInput").ap()
    w2d = nc.dram_tensor("W2ext", [D_IN, ROW2], F32, kind="ExternalInput").ap()
    b1d = nc.dram_tensor("b1v", [1, D_IN], F32, kind="ExternalInput").ap()
    b2d = nc.dram_tensor("b2v", [1, 64], F32, kind="ExternalInput").ap()
    iotad = nc.dram_tensor("iota128", [128, 128], F32, kind="ExternalInput").ap()
    identd = nc.dram_tensor("ident128", [128, 128], F32, kind="ExternalInput").ap()
    idxAd = nc.dram_tensor("idxA", [128, NB * TA * 8], I16, kind="ExternalInput").ap()
    idxBd = nc.dram_tensor("idxB", [128, NB * TB * 8], I16, kind="ExternalInput").ap()
    dstgd = nc.dram_tensor("dstg", [128, NB * T1 * 8], I16, kind="ExternalInput").ap()
    dstld = nc.dram_tensor("dstl", [128, NB * T1], F32, kind="ExternalInput").ap()
    y = nc.dram_tensor("y", [P_NODES, 64], F32, kind="ExternalOutput").ap()

    table1 = nc.dram_tensor("table1", [NPAD, ROW1], F32).ap()
    adtab = nc.dram_tensor("adtab", [P_NODES, ROWA], F32).ap()

    AL = mybir.AluOpType
    ACT = mybir.ActivationFunctionType

    with tile.TileContext(nc) as tc:
        with (
            tc.tile_pool(name="const", bufs=1) as cp,
            tc.tile_pool(name="sb", bufs=2) as sb,
            tc.tile_pool(name="psum", bufs=2, space="PSUM") as pp,
            tc.tile_pool(name="dram", bufs=1, space="DRAM") as dram,
        ):
            nc.gpsimd.load_library(library_config.mlp)

            # ---------------- constants
            w1e = cp.tile([128, 2, ROW1], F32)
            nc.sync.dma_start(out=w1e[:, 0], in_=w1d[0:128, :])
            nc.sync.dma_start(out=w1e[:, 1], in_=w1d[128:256, :])
            w2e = cp.tile([128, 2, ROW2], F32)
            nc.sync.dma_start(out=w2e[:, 0], in_=w2d[0:128, :])
            nc.sync.dma_start(out=w2e[:, 1], in_=w2d[128:256, :])
            iota_f = cp.tile([128, 128], F32)
            nc.sync.dma_start(out=iota_f[:], in_=iotad[:])
            ident = cp.tile([128, 128], F32)
            nc.sync.dma_start(out=ident[:], in_=identd[:])

            b1row = cp.tile([1, D_IN], F32)
            nc.sync.dma_start(out=b1row[:], in_=b1d[:])
            b2row = cp.tile([1, 64], F32)
            nc.sync.dma_start(out=b2row[:], in_=b2d[:])
            ones1 = cp.tile([1, 128], F32)
            nc.vector.memset(ones1[:], 1.0)
            bias_ps = pp.tile([128, D_IN], F32, tag="ps_tab")
            nc.tensor.matmul(bias_ps[:], lhsT=ones1[:], rhs=b1row[:], start=True, stop=True)
            bias1 = cp.tile([128, D_IN], F32)
            nc.vector.tensor_copy(out=bias1[:], in_=bias_ps[:])
            bias_ps2 = pp.tile([128, 64], F32, tag="ps_tab")
            nc.tensor.matmul(bias_ps2[:], lhsT=ones1[:], rhs=b2row[:], start=True, stop=True)
            bias2 = cp.tile([128, 64], F32)
            nc.vector.tensor_copy(out=bias2[:], in_=bias_ps2[:])

            idxA = cp.tile([128, NB * TA * 8], I16)
            nc.sync.dma_start(out=idxA[:], in_=idxAd[:])
            idxB = cp.tile([128, NB * TB * 8], I16)
            nc.sync.dma_start(out=idxB[:], in_=idxBd[:])
            dstg = cp.tile([128, NB * T1 * 8], I16)
            nc.sync.dma_start(out=dstg[:], in_=dstgd[:])
            dstl = cp.tile([128, NB * T1], F32)
            nc.sync.dma_start(out=dstl[:], in_=dstld[:])

            h1T = cp.tile([128, 2, P_NODES], F32)

            # ---------------- phase 1: layer-1 node table (replicated build)
            for tb in range(TBLK):
                xt = sb.tile([128, 2, 128], F32, tag="xt")
                nc.sync.dma_start(out=xt[:, 0], in_=xT[0:128, tb * 128:(tb + 1) * 128])
                nc.sync.dma_start(out=xt[:, 1], in_=xT[128:256, tb * 128:(tb + 1) * 128])
                ps = pp.tile([128, ROW1], F32, tag="ps_tab")
                nc.tensor.matmul(ps[:], lhsT=xt[:, 0], rhs=w1e[:, 0], start=True, stop=False)
                nc.tensor.matmul(ps[:], lhsT=xt[:, 1], rhs=w1e[:, 1], start=False, stop=True)
                ev = sb.tile([128, ROW1], F32, tag="ev")
                nc.scalar.copy(out=ev[:], in_=ps[:])
                nc.sync.dma_start(out=table1[tb * 128:(tb + 1) * 128, :], in_=ev[:])

            # ---------------- phase 1b: adtab[:, 0:4] = alpha_dst1 (own)
            for b in range(NB):
                xo = sb.tile([128, 2, 128], F32, tag="xt")
                nc.sync.dma_start(out=xo[:, 0], in_=xTo[0:128, b * 128:(b + 1) * 128])
                nc.sync.dma_start(out=xo[:, 1], in_=xTo[128:256, b * 128:(b + 1) * 128])
                psa = pp.tile([128, 4], F32, tag="ps_tr")
                nc.tensor.matmul(psa[:], lhsT=xo[:, 0], rhs=w1e[:, 0, 260:264], start=True, stop=False)
                nc.tensor.matmul(psa[:], lhsT=xo[:, 1], rhs=w1e[:, 1, 260:264], start=False, stop=True)
                eva = sb.tile([128, 4], F32, tag="eva")
                nc.vector.tensor_copy(out=eva[:], in_=psa[:])
                nc.sync.dma_start(out=adtab[b * 128:(b + 1) * 128, 0:4], in_=eva[:])

            def edge_group(g, gs, colA, colB, colG, gtab, row, nh, t_off,
                           ad_col, agg_cols, out_cb):
                """Gather + aggregate one group of gs destination blocks."""
                nTa, nTb, nT = gs * TA, gs * TB, gs * T1
                gb = sb.tile([128, G_BLK * T1 * row], F32, tag="gbuf")
                gbv = gb.rearrange("p (t f) -> p t f", f=row)

                def chunked(out_v, tab_ap, idx_t, col0, ntiles, r):
                    # descriptor ring limit: <= 1024 indices (8 tiles) per call
                    t0 = 0
                    while t0 < ntiles:
                        ct = min(4, ntiles - t0)
                        nc.gpsimd.dma_gather(
                            out_v[:, t0:t0 + ct, :], tab_ap,
                            idx_t[:, (col0 + t0) * 8:(col0 + t0 + ct) * 8],
                            ct * 128, ct * 128, r)
                        t0 += ct

                chunked(gbv[:, 0:nTa, :], gtab[0:HALF, :], idxA, colA, nTa, row)
                chunked(gbv[:, nTa:nT, :], gtab[HALF:NPAD, :], idxB, colB, nTb, row)
                adg = sb.tile([128, G_BLK * T1, ROWA], F32, tag="adg")
                chunked(adg[:, 0:nT, :], adtab[:, :], dstg, colG, nT, ROWA)
                if stage < 2:
                    return
                # t = exp(leaky_relu(as[src] + ad[dst]))
                e0 = sb.tile([128, G_BLK * T1 * 4], F32, tag="e0")
                e0v = e0.rearrange("p (t h) -> p t h", h=4)[:, 0:nT, 0:nh]
                nc.vector.tensor_tensor(out=e0v, in0=gbv[:, 0:nT, t_off:t_off + nh],
                                        in1=adg[:, 0:nT, ad_col:ad_col + nh], op=AL.add)
                e1 = sb.tile([128, G_BLK * T1 * 4], F32, tag="e1")
                e1v = e1.rearrange("p (t h) -> p t h", h=4)[:, 0:nT, 0:nh]
                nc.vector.tensor_scalar_mul(e1v, e0v, NEG)
                nc.vector.tensor_tensor(out=e0v, in0=e0v, in1=e1v, op=AL.max)
                tb_ = sb.tile([128, G_BLK * T1 * 4], F32, tag="tblk")
                tv = tb_.rearrange("p (t h) -> p t h", h=4)[:, 0:nT, 0:nh]
                nc.scalar.activation(tv, e0v, ACT.Exp)
                if stage < 3:
                    return
                # messages in place; t into the alpha_src slot
                nc.vector.tensor_tensor(
                    out=gbv[:, 0:nT, 0:nh * 64].rearrange("p t (h c) -> p t h c", c=64),
                    in0=gbv[:, 0:nT, 0:nh * 64].rearrange("p t (h c) -> p t h c", c=64),
                    in1=tv[:, :, :, None].to_broadcast([128, nT, nh, 64]),
                    op=AL.mult)
                nc.scalar.copy(out=gbv[:, 0:nT, t_off:t_off + nh], in_=tv)
                # one-hot for the whole group
                oh = sb.tile([128, G_BLK * T1 * 128], F32, tag="oh")
                ohv = oh.rearrange("p (t m) -> p t m", m=128)[:, 0:nT, :]
                nc.vector.tensor_tensor(
                    out=ohv,
                    in0=dstl[:, colG:colG + nT][:, :, None].to_broadcast([128, nT, 128]),
                    in1=iota_f[:, None, :].to_broadcast([128, nT, 128]),
                    op=AL.is_equal)
                if stage < 4:
                    return
                for gb_i in range(gs):
                    ps_agg = pp.tile([128, agg_cols], F32, tag="ps_agg")
                    tiles = ([gb_i * TA + t for t in range(TA)] +
                             [nTa + gb_i * TB + t for t in range(TB)])
                    for k, t in enumerate(tiles):
                        nc.tensor.matmul(ps_agg[:],
                                         lhsT=oh[:, t * 128:(t + 1) * 128],
                                         rhs=gb[:, t * row:t * row + agg_cols],
                                         start=(k == 0), stop=(k == len(tiles) - 1))
                    if stage >= 5:
                        out_cb(ps_agg, g * G_BLK + gb_i)

            # ---------------- phase 3: layer-1 edge aggregation
            def fin1(ps_agg, b):
                den = sb.tile([128, H1], F32, tag="den")
                nc.vector.tensor_scalar_add(den[:], ps_agg[:, 256:260], EPS)
                rec = sb.tile([128, H1], F32, tag="rec")
                nc.vector.reciprocal(rec[:], den[:])
                o1 = sb.tile([128, D_IN], F32, tag="o1")
                o1v = o1.rearrange("p (h c) -> p h c", c=64)
                nc.vector.tensor_tensor(
                    out=o1v,
                    in0=ps_agg[:, 0:256].rearrange("p (h c) -> p h c", c=64),
                    in1=rec[:, :, None].to_broadcast([128, H1, 64]),
                    op=AL.mult)
                nc.vector.tensor_tensor(out=o1[:], in0=o1[:], in1=bias1[:], op=AL.add)
                eu = sb.tile([128, D_IN], F32, tag="eu")
                nc.vector.tensor_scalar_min(eu[:], o1[:], 0.0)
                nc.scalar.activation(eu[:], eu[:], ACT.Exp)
                nc.scalar.activation(o1[:], o1[:], ACT.Relu)
                nc.vector.scalar_tensor_tensor(out=o1[:], in0=eu[:], scalar=-1.0,
                                               in1=o1[:], op0=AL.add, op1=AL.add)
                for hf in range(2):
                    tps = pp.tile([128, 128], F32, tag="ps_tr")
                    nc.tensor.transpose(out=tps[:], in_=o1[:, hf * 128:(hf + 1) * 128],
                                        identity=ident[:])
                    nc.vector.tensor_copy(out=h1T[:, hf, b * 128:(b + 1) * 128], in_=tps[:])

            if phases >= 3:
                colA = colB = colG = 0
                for g, gs in enumerate(grp_sizes):
                    if g < ngrp_lim:
                        edge_group(g, gs, colA, colB, colG, table1, ROW1, H1, 256, 0, 260, fin1)
                    colA += gs * TA; colB += gs * TB; colG += gs * T1

            # ---------------- phase 4: layer-2 table (own) + AllGather
            h2own = dram.tile([P_NODES, ROW2], F32)
            table2 = dram.tile([NPAD, ROW2], F32)
            for b in range(NB if phases >= 4 else 0):
                ps2 = pp.tile([128, ROW2], F32, tag="ps_tab")
                nc.tensor.matmul(ps2[:], lhsT=h1T[:, 0, b * 128:(b + 1) * 128],
                                 rhs=w2e[:, 0], start=True, stop=False)
                nc.tensor.matmul(ps2[:], lhsT=h1T[:, 1, b * 128:(b + 1) * 128],
                                 rhs=w2e[:, 1], start=False, stop=True)
                ev2 = sb.tile([128, ROW2], F32, tag="ev")
                nc.scalar.copy(out=ev2[:], in_=ps2[:])
                nc.sync.dma_start(out=h2own[b * 128:(b + 1) * 128, :], in_=ev2[:])
                nc.sync.dma_start(out=adtab[b * 128:(b + 1) * 128, 4:5], in_=ev2[:, 65:66])
            if phases >= 4:
                nc.gpsimd.collective_compute(
                    "AllGather", AL.bypass,
                    replica_groups=[list(range(N_CORES))],
                    ins=[h2own.opt()], outs=[table2.opt()])

            # ---------------- phase 6: layer-2 edge aggregation + output
            def fin2(ps_agg, b):
                den = sb.tile([128, 1], F32, tag="den")
                nc.vector.tensor_scalar_add(den[:], ps_agg[:, 64:65], EPS)
                rec = sb.tile([128, 1], F32, tag="rec")
                nc.vector.reciprocal(rec[:], den[:])
                o2 = sb.tile([128, 64], F32, tag="o2")
                nc.vector.tensor_scalar(out=o2[:], in0=ps_agg[:, 0:64],
                                        scalar1=rec[:], scalar2=None, op0=AL.mult)
                nc.vector.tensor_tensor(out=o2[:], in0=o2[:], in1=bias2[:], op=AL.add)
                nc.sync.dma_start(out=y[b * 128:(b + 1) * 128, :], in_=o2[:])

            if phases >= 6:
                colA = colB = colG = 0
                for g, gs in enumerate(grp_sizes):
                    edge_group(g, gs, colA, colB, colG, table2, ROW2, 1, 64, 4, 65, fin2)
                    colA += gs * TA; colB += gs * TB; colG += gs * T1

    nc.compile()
    return nc


_CACHE = {}


def _get_program(key):
    if key not in _CACHE:
        _CACHE[key] = build_program(*key[:2], list(key[2]), phases=key[3])
    return _CACHE[key]


def run(inputs, trace=False, trace_kwargs=None, phases=6):
    x = np.asarray(inputs["x"], np.float32)
    TA, TB, grp_sizes, idxA, idxB, dstg, dstl = preprocess_edges(inputs["edge_index"])
    xT, W1ext, W2ext = prep_weights(
        x, inputs["W1"], inputs["att_src1"], inputs["att_dst1"],
        inputs["W2"], inputs["att_src2"], inputs["att_dst2"])
    b1v = np.asarray(inputs["b1"], np.float32).reshape(1, D_IN)
    b2v = np.asarray(inputs["b2"], np.float32).reshape(1, 64)
    iota = np.tile(np.arange(128, dtype=np.float32), (128, 1))
    ident = np.eye(128, dtype=np.float32)

    nc = _get_program((TA, TB, tuple(grp_sizes), phases))
    in_maps = []
    for c in range(N_CORES):
        in_maps.append({
            "xT": xT, "xT_own": np.ascontiguousarray(xT[:, c * P_NODES:(c + 1) * P_NODES]),
            "W1ext": W1ext, "W2ext": W2ext, "b1v": b1v, "b2v": b2v,
            "iota128": iota, "ident128": ident,
            "idxA": idxA[c], "idxB": idxB[c], "dstg": dstg[c], "dstl": dstl[c],
        })
    res = run_bass_kernel_spmd(nc, in_maps, core_ids=list(range(N_CORES)),
                               trace=trace, **(trace_kwargs or {}))
    out = np.concatenate([res.results[c]["y"] for c in range(N_CORES)], axis=0)
    return np.ascontiguousarray(out[:N_NODES]), res


def kernel(**inputs):
    out, _ = run(inputs, trace=False)
    return out



# revision 6
# speedup vs baseline: 1.5944x; 1.5944x over previous
"""2-layer GAT (PyG GATConv eval semantics) on 8 Trainium2 NeuronCores.

Sharding: nodes by contiguous id range (6272/core, 49 blocks of 128); edges
(with self loops) partitioned by destination core/block so segment softmax and
scatter-add stay local. Per layer a replicated node table (bf16 rows) is
gathered by source id via the GPSIMD dma_gather ucode; ad[dst] is expanded
per edge on the tensor engine (K=1 outer-product of local dst ids + is_equal
against a per-partition iota builds a transposed one-hot ohT[m,e]; a small
matmul ohT^T @ adblk yields ad per edge) instead of a second dma_gather.
Attention weights t = exp(leaky_relu(as[src]+ad[dst])) scale the messages in
SBUF and the per-destination-block aggregation (numerator + denominator) is a
one-hot matmul accumulated in PSUM. Layer-2's table is built per block right
after the layer-1 finish and shared via one bf16 AllGather. dma_gather
indices are int16, so tables are split in two halves (A: rows < 25088,
B: rest); tile counts are per-block maxima over the 8 cores so one SPMD
program serves all cores with minimal padding.
"""

import numpy as np
import ml_dtypes

import concourse.bacc as bacc
import concourse.bass as bass
import concourse.mybir as mybir
import concourse.tile as tile
from concourse import library_config
from concourse.bass_utils import run_bass_kernel_spmd

N_NODES = 50000
N_CORES = 8
P_NODES = 6272                  # nodes per core (49 blocks of 128)
NPAD = P_NODES * N_CORES        # 50176
HALF = NPAD // 2                # 25088 (A/B table split, int16-safe)
NB = P_NODES // 128             # 49 destination blocks per core
TBLK = NPAD // 128              # 392 table-build blocks
D_IN = 256
H1 = 4
ROW1 = 384                      # bf16 row: [h(256) | as1(4) | ad1(4) | pad], 768B
C1 = 264                        # computed cols of a layer-1 table row
ROW2 = 128                      # bf16 row: [h2(64) | as2(1) | ad2(1) | pad], 256B
C2 = 66                         # computed cols of a layer-2 table row
NEG = 0.2
EPS = 1e-16
PADV = 960.0                    # pad dst-id sentinel (bf16-exact, != 0..127)
GCHUNK = 8                      # gather tiles per dma_gather call (<=1024 idx)

F32 = mybir.dt.float32
BF16 = mybir.dt.bfloat16
I16 = mybir.dt.int16
BF = ml_dtypes.bfloat16


# ---------------------------------------------------------------- host prep

def _wrap16(vals, n_slots):
    """dma_gather index layout: index j at [j%16, j//16], replicated to all
    eight 16-partition groups."""
    a = np.zeros((16, n_slots // 16), np.int16)
    j = np.arange(len(vals))
    a[j % 16, j // 16] = vals
    return np.tile(a, (8, 1))


def preprocess_edges(edge_index):
    ei = np.asarray(edge_index).astype(np.int64)
    loops = np.arange(N_NODES, dtype=np.int64)
    src = np.concatenate([ei[0], loops])
    dst = np.concatenate([ei[1], loops])

    percore = []                # [c][b] = (srcA, dlocA, srcB, dlocB)
    for c in range(N_CORES):
        lo = c * P_NODES
        m = (dst >= lo) & (dst < lo + P_NODES)
        s, d = src[m], dst[m] - lo
        blocks = []
        for b in range(NB):
            mb = (d // 128) == b
            sb_, db_ = s[mb], d[mb] - b * 128
            am = sb_ < HALF
            blocks.append((sb_[am], db_[am], sb_[~am] - HALF, db_[~am]))
        percore.append(blocks)

    TA = [max(max(1, -(-len(percore[c][b][0]) // 128)) for c in range(N_CORES))
          for b in range(NB)]
    TB = [max(max(1, -(-len(percore[c][b][2]) // 128)) for c in range(N_CORES))
          for b in range(NB)]
    T1 = [a + b for a, b in zip(TA, TB)]
    sumA, sumB, sumT = sum(TA), sum(TB), sum(T1)

    idxA = np.zeros((N_CORES, 128, sumA * 8), np.int16)
    idxB = np.zeros((N_CORES, 128, sumB * 8), np.int16)
    dstl = np.full((N_CORES, 128, sumT), PADV, np.float32)
    dstf = np.full((N_CORES, 1, sumT * 128), PADV, np.float32)

    for c in range(N_CORES):
        colA = colB = colG = 0
        for b in range(NB):
            sa, da, sb_, db_ = percore[c][b]
            va = np.zeros(TA[b] * 128, np.int64)
            va[:len(sa)] = sa
            vb = np.zeros(TB[b] * 128, np.int64)
            vb[:len(sb_)] = sb_
            idxA[c, :, colA * 8:(colA + TA[b]) * 8] = _wrap16(va, TA[b] * 128)
            idxB[c, :, colB * 8:(colB + TB[b]) * 8] = _wrap16(vb, TB[b] * 128)
            lo_sl = np.full(T1[b] * 128, PADV, np.float32)
            lo_sl[:len(da)] = da
            lo_sl[TA[b] * 128:TA[b] * 128 + len(db_)] = db_
            dstl[c, :, colG:colG + T1[b]] = lo_sl.reshape(T1[b], 128).T
            dstf[c, 0, colG * 128:(colG + T1[b]) * 128] = lo_sl
            colA += TA[b]
            colB += TB[b]
            colG += T1[b]
    return tuple(TA), tuple(TB), idxA, idxB, dstl, dstf.astype(BF)


def prep_weights(x, W1, att_src1, att_dst1, W2, att_src2, att_dst2):
    x = np.asarray(x, np.float32)
    W1 = np.asarray(W1, np.float32)
    W2 = np.asarray(W2, np.float32)
    As1 = np.einsum("khc,hc->kh", W1.reshape(D_IN, H1, 64), np.asarray(att_src1, np.float32))
    Ad1 = np.einsum("khc,hc->kh", W1.reshape(D_IN, H1, 64), np.asarray(att_dst1, np.float32))
    As2 = W2 @ np.asarray(att_src2, np.float32).reshape(64)
    Ad2 = W2 @ np.asarray(att_dst2, np.float32).reshape(64)
    W1ext = np.zeros((D_IN, ROW1), np.float32)
    W1ext[:, :256] = W1
    W1ext[:, 256:260] = As1
    W1ext[:, 260:264] = Ad1
    W2ext = np.zeros((D_IN, ROW2), np.float32)
    W2ext[:, :64] = W2
    W2ext[:, 64] = As2
    W2ext[:, 65] = Ad2
    xT = np.zeros((D_IN, NPAD), np.float32)
    xT[:, :N_NODES] = x.T
    return xT.astype(BF), W1ext.astype(BF), W2ext.astype(BF)


# ------------------------------------------------------------- bass program

def build_program(TA, TB):
    T1 = [a + b for a, b in zip(TA, TB)]
    sumA, sumB, sumT = sum(TA), sum(TB), sum(T1)
    T1MAX = max(T1)
    nc = bacc.Bacc("TRN2", target_bir_lowering=False, debug=False,
                   num_devices=N_CORES)

    xT = nc.dram_tensor("xT", [D_IN, NPAD], BF16, kind="ExternalInput").ap()
    xTo = nc.dram_tensor("xT_own", [D_IN, P_NODES], BF16, kind="ExternalInput").ap()
    w1d = nc.dram_tensor("W1ext", [D_IN, ROW1], BF16, kind="ExternalInput").ap()
    w2d = nc.dram_tensor("W2ext", [D_IN, ROW2], BF16, kind="ExternalInput").ap()
    b1d = nc.dram_tensor("b1v", [1, D_IN], F32, kind="ExternalInput").ap()
    b2d = nc.dram_tensor("b2v", [1, 64], F32, kind="ExternalInput").ap()
    iotad = nc.dram_tensor("iota128", [128, 128], F32, kind="ExternalInput").ap()
    iotacd = nc.dram_tensor("iotacol", [128, 1], F32, kind="ExternalInput").ap()
    identd = nc.dram_tensor("ident128", [128, 128], F32, kind="ExternalInput").ap()
    idxAd = nc.dram_tensor("idxA", [128, sumA * 8], I16, kind="ExternalInput").ap()
    idxBd = nc.dram_tensor("idxB", [128, sumB * 8], I16, kind="ExternalInput").ap()
    dstld = nc.dram_tensor("dstl", [128, sumT], F32, kind="ExternalInput").ap()
    dstfd = nc.dram_tensor("dstf", [1, sumT * 128], BF16, kind="ExternalInput").ap()
    y = nc.dram_tensor("y", [P_NODES, 64], F32, kind="ExternalOutput").ap()

    AL = mybir.AluOpType
    ACT = mybir.ActivationFunctionType

    with tile.TileContext(nc) as tc:
        with (
            tc.tile_pool(name="const", bufs=1) as cp,
            tc.tile_pool(name="sb", bufs=2) as sb,
            tc.tile_pool(name="psum", bufs=2, space="PSUM") as pp,
            tc.tile_pool(name="dram", bufs=1, space="DRAM") as dram,
        ):
            nc.gpsimd.load_library(library_config.mlp)

            tableA = dram.tile([HALF, ROW1], BF16)
            tableB = dram.tile([NPAD - HALF, ROW1], BF16)
            h2own = dram.tile([P_NODES, ROW2], BF16)
            table2 = dram.tile([NPAD, ROW2], BF16, addr_space="Shared")

            # ---------------- constants
            w1e = cp.tile([128, 2, ROW1], BF16)
            nc.sync.dma_start(out=w1e[:, 0], in_=w1d[0:128, :])
            nc.sync.dma_start(out=w1e[:, 1], in_=w1d[128:256, :])
            w2e = cp.tile([128, 2, ROW2], BF16)
            nc.sync.dma_start(out=w2e[:, 0], in_=w2d[0:128, :])
            nc.sync.dma_start(out=w2e[:, 1], in_=w2d[128:256, :])
            iota_f = cp.tile([128, 128], F32)
            nc.sync.dma_start(out=iota_f[:], in_=iotad[:])
            iota_c = cp.tile([128, 1], F32)
            nc.sync.dma_start(out=iota_c[:], in_=iotacd[:])
            ident = cp.tile([128, 128], F32)
            nc.sync.dma_start(out=ident[:], in_=identd[:])

            b1row = cp.tile([1, D_IN], F32)
            nc.sync.dma_start(out=b1row[:], in_=b1d[:])
            b2row = cp.tile([1, 64], F32)
            nc.sync.dma_start(out=b2row[:], in_=b2d[:])
            ones1 = cp.tile([1, 128], F32)
            nc.vector.memset(ones1[:], 1.0)
            ones1b = cp.tile([1, 128], BF16)
            nc.vector.memset(ones1b[:], 1.0)
            bias_ps = pp.tile([128, D_IN], F32, tag="ps_tab")
            nc.tensor.matmul(bias_ps[:], lhsT=ones1[:], rhs=b1row[:], start=True, stop=True)
            bias1 = cp.tile([128, D_IN], F32)
            nc.vector.tensor_copy(out=bias1[:], in_=bias_ps[:])
            bias_ps2 = pp.tile([128, 64], F32, tag="ps_tab")
            nc.tensor.matmul(bias_ps2[:], lhsT=ones1[:], rhs=b2row[:], start=True, stop=True)
            bias2 = cp.tile([128, 64], F32)
            nc.vector.tensor_copy(out=bias2[:], in_=bias_ps2[:])

            idxA = cp.tile([128, sumA * 8], I16)
            nc.sync.dma_start(out=idxA[:], in_=idxAd[:])
            idxB = cp.tile([128, sumB * 8], I16)
            nc.sync.dma_start(out=idxB[:], in_=idxBd[:])
            dstl = cp.tile([128, sumT], F32)
            nc.sync.dma_start(out=dstl[:], in_=dstld[:])

            adtab1 = cp.tile([128, NB * H1], BF16)
            adtab2 = cp.tile([128, NB], BF16)

            # ---------------- phase 1: layer-1 node table (replicated build)
            for tb in range(TBLK):
                xt = sb.tile([128, 2, 128], BF16, tag="xt")
                nc.sync.dma_start(out=xt[:, 0], in_=xT[0:128, tb * 128:(tb + 1) * 128])
                nc.sync.dma_start(out=xt[:, 1], in_=xT[128:256, tb * 128:(tb + 1) * 128])
                ps = pp.tile([128, C1], F32, tag="ps_tab")
                nc.tensor.matmul(ps[:], lhsT=xt[:, 0], rhs=w1e[:, 0, 0:C1], start=True, stop=False)
                nc.tensor.matmul(ps[:], lhsT=xt[:, 1], rhs=w1e[:, 1, 0:C1], start=False, stop=True)
                ev = sb.tile([128, C1], BF16, tag="ev")
                nc.scalar.copy(out=ev[:], in_=ps[:])
                if tb < HALF // 128:
                    nc.sync.dma_start(out=tableA[tb * 128:(tb + 1) * 128, 0:C1], in_=ev[:])
                else:
                    tb2 = tb - HALF // 128
                    nc.sync.dma_start(out=tableB[tb2 * 128:(tb2 + 1) * 128, 0:C1], in_=ev[:])

            # ---------------- phase 1b: adtab1 = alpha_dst1 of own nodes
            for b in range(NB):
                xo = sb.tile([128, 2, 128], BF16, tag="xt")
                nc.sync.dma_start(out=xo[:, 0], in_=xTo[0:128, b * 128:(b + 1) * 128])
                nc.sync.dma_start(out=xo[:, 1], in_=xTo[128:256, b * 128:(b + 1) * 128])
                psa = pp.tile([128, max(T1) * 4], F32, tag="ps_ad", bufs=1)
                nc.tensor.matmul(psa[:, 0:H1], lhsT=xo[:, 0], rhs=w1e[:, 0, 260:264], start=True, stop=False)
                nc.tensor.matmul(psa[:, 0:H1], lhsT=xo[:, 1], rhs=w1e[:, 1, 260:264], start=False, stop=True)
                nc.vector.tensor_copy(out=adtab1[:, b * H1:(b + 1) * H1], in_=psa[:, 0:H1])

            def edge_block(b, colA, colB, colG, tabA, tabB, row, cols, nh,
                           t_off, adtab_s, out_cb):
                """Gather + attention + aggregation for one destination block."""
                nt = T1[b]
                gb = sb.tile([128, T1MAX, row], BF16, tag=f"gbuf{row}")
                gbv = gb  # [p, t, f]

                def chunked(out_v, tab_ap, idx_t, col0, ntiles):
                    t0 = 0
                    while t0 < ntiles:
                        ct = min(GCHUNK, ntiles - t0)
                        nc.gpsimd.dma_gather(
                            out_v[:, t0:t0 + ct, :], tab_ap,
                            idx_t[:, (col0 + t0) * 8:(col0 + t0 + ct) * 8],
                            ct * 128, ct * 128, row)
                        t0 += ct

                chunked(gbv[:, 0:TA[b]], tabA, idxA, colA, TA[b])
                chunked(gbv[:, TA[b]:nt], tabB, idxB, colB, TB[b])

                # dst ids of this block's edge slots, replicated to all
                # partitions via a K=1 outer product; is_equal vs iota gives
                # the m-major one-hot ohT.
                dstf_t = sb.tile([1, T1MAX * 128], BF16, tag="dstf")
                nc.sync.dma_start(out=dstf_t[:, 0:nt * 128],
                                  in_=dstfd[:, colG * 128:(colG + nt) * 128])
                ohT = sb.tile([128, T1MAX * 128], BF16, tag="ohT")
                c0 = 0
                while c0 < nt:
                    cc = min(4, nt - c0)
                    ps_rep = pp.tile([128, 512], F32, tag="ps_rep")
                    nc.tensor.matmul(ps_rep[:, 0:cc * 128], lhsT=ones1b[:],
                                     rhs=dstf_t[:, c0 * 128:(c0 + cc) * 128],
                                     start=True, stop=True)
                    nc.vector.tensor_scalar(
                        out=ohT[:, c0 * 128:(c0 + cc) * 128],
                        in0=ps_rep[:, 0:cc * 128], scalar1=iota_c[:],
                        scalar2=None, op0=AL.is_equal)
                    c0 += cc

                # ad[dst] per edge: ohT^T @ adblk, one matmul per tile
                # (slots at stride 4 so PSUM column offsets stay 16B-aligned)
                ps_ad = pp.tile([128, T1MAX * 4], F32, tag="ps_ad", bufs=1)
                for t in range(nt):
                    nc.tensor.matmul(ps_ad[:, t * 4:t * 4 + nh],
                                     lhsT=ohT[:, t * 128:(t + 1) * 128],
                                     rhs=adtab_s, start=True, stop=True)
                ps_adv = ps_ad[:, 0:nt * 4].rearrange("p (t q) -> p t q", q=4)

                # t = exp(leaky_relu(as[src] + ad[dst]))
                as_f = sb.tile([128, T1MAX * 4], F32, tag="asf")
                as_fv = as_f[:, 0:nt * nh].rearrange("p (t h) -> p t h", h=nh)
                nc.scalar.copy(out=as_fv, in_=gbv[:, 0:nt, t_off:t_off + nh])
                e0 = sb.tile([128, T1MAX * 4], F32, tag="e0")
                nc.vector.tensor_tensor(
                    out=e0[:, 0:nt * nh].rearrange("p (t h) -> p t h", h=nh),
                    in0=as_fv, in1=ps_adv[:, :, 0:nh], op=AL.add)
                e1 = sb.tile([128, T1MAX * 4], F32, tag="e1")
                nc.vector.scalar_tensor_tensor(out=e1[:, 0:nt * nh], in0=e0[:, 0:nt * nh],
                                               scalar=NEG, in1=e0[:, 0:nt * nh],
                                               op0=AL.mult, op1=AL.max)
                tbf = sb.tile([128, T1MAX * 4], BF16, tag="tbf")
                nc.scalar.activation(tbf[:, 0:nt * nh], e1[:, 0:nt * nh], ACT.Exp)
                tv = tbf[:, 0:nt * nh].rearrange("p (t h) -> p t h", h=nh)
                # messages in place; t into the as slot (denominator column)
                nc.vector.tensor_tensor(
                    out=gbv[:, 0:nt, 0:nh * 64].rearrange("p t (h c) -> p t h c", c=64),
                    in0=gbv[:, 0:nt, 0:nh * 64].rearrange("p t (h c) -> p t h c", c=64),
                    in1=tv[:, :, :, None].to_broadcast([128, nt, nh, 64]),
                    op=AL.mult)
                nc.scalar.copy(out=gbv[:, 0:nt, t_off:t_off + nh], in_=tv)

                # edge-major one-hot for the aggregation matmul
                oh = sb.tile([128, T1MAX * 128], BF16, tag="oh")
                nc.vector.tensor_tensor(
                    out=oh[:, 0:nt * 128].rearrange("p (t m) -> p t m", m=128),
                    in0=dstl[:, colG:colG + nt][:, :, None].to_broadcast([128, nt, 128]),
                    in1=iota_f[:, None, :].to_broadcast([128, nt, 128]),
                    op=AL.is_equal)

                ps_agg = pp.tile([128, cols], F32, tag="ps_agg")
                for t in range(nt):
                    nc.tensor.matmul(ps_agg[:],
                                     lhsT=oh[:, t * 128:(t + 1) * 128],
                                     rhs=gb[:, t, 0:cols],
                                     start=(t == 0), stop=(t == nt - 1))
                out_cb(ps_agg, b)

            # ---------------- phase 3: layer-1 edges; fused layer-2 table
            def fin1(ps_agg, b):
                den = sb.tile([128, H1], F32, tag="den")
                nc.vector.tensor_scalar_add(den[:], ps_agg[:, 256:260], EPS)
                rec = sb.tile([128, H1], F32, tag="rec")
                nc.vector.reciprocal(rec[:], den[:])
                o1 = sb.tile([128, D_IN], F32, tag="o1")
                o1v = o1.rearrange("p (h c) -> p h c", c=64)
                nc.vector.tensor_tensor(
                    out=o1v,
                    in0=ps_agg[:, 0:256].rearrange("p (h c) -> p h c", c=64),
                    in1=rec[:, :, None].to_broadcast([128, H1, 64]),
                    op=AL.mult)
                nc.vector.tensor_tensor(out=o1[:], in0=o1[:], in1=bias1[:], op=AL.add)
                eu = sb.tile([128, D_IN], F32, tag="eu")
                nc.vector.tensor_scalar_min(eu[:], o1[:], 0.0)
                nc.scalar.activation(eu[:], eu[:], ACT.Exp)
                nc.scalar.activation(o1[:], o1[:], ACT.Relu)
                nc.vector.scalar_tensor_tensor(out=o1[:], in0=eu[:], scalar=-1.0,
                                               in1=o1[:], op0=AL.add, op1=AL.add)
                # layer-2 table row for this block (h1 must be transposed)
                h1tb = sb.tile([128, 2, 128], BF16, tag="h1tb")
                for hf in range(2):
                    tps = pp.tile([128, 128], F32, tag="ps_tr", bufs=1)
                    nc.tensor.transpose(out=tps[:], in_=o1[:, hf * 128:(hf + 1) * 128],
                                        identity=ident[:])
                    nc.vector.tensor_copy(out=h1tb[:, hf], in_=tps[:])
                ps2 = pp.tile([128, C2], F32, tag="ps_tab")
                nc.tensor.matmul(ps2[:], lhsT=h1tb[:, 0], rhs=w2e[:, 0, 0:C2],
                                 start=True, stop=False)
                nc.tensor.matmul(ps2[:], lhsT=h1tb[:, 1], rhs=w2e[:, 1, 0:C2],
                                 start=False, stop=True)
                ev2 = sb.tile([128, C2], BF16, tag="ev")
                nc.scalar.copy(out=ev2[:], in_=ps2[:])
                nc.sync.dma_start(out=h2own[b * 128:(b + 1) * 128, 0:C2], in_=ev2[:])
                nc.vector.tensor_copy(out=adtab2[:, b:b + 1], in_=ev2[:, 65:66])

            colA = colB = colG = 0
            for b in range(NB):
                edge_block(b, colA, colB, colG, tableA[:, :], tableB[:, :],
                           ROW1, 260, H1, 256, adtab1[:, b * H1:(b + 1) * H1], fin1)
                colA += TA[b]; colB += TB[b]; colG += T1[b]

            # ---------------- phase 4: share layer-2 table
            nc.gpsimd.collective_compute(
                "AllGather", AL.bypass,
                replica_groups=[list(range(N_CORES))],
                ins=[h2own.opt()], outs=[table2.opt()])

            # ---------------- phase 6: layer-2 edges + output
            def fin2(ps_agg, b):
                den = sb.tile([128, 1], F32, tag="den")
                nc.vector.tensor_scalar_add(den[:], ps_agg[:, 64:65], EPS)
                rec = sb.tile([128, 1], F32, tag="rec")
                nc.vector.reciprocal(rec[:], den[:])
                o2 = sb.tile([128, 64], F32, tag="o2s")
                nc.vector.tensor_scalar(out=o2[:], in0=ps_agg[:, 0:64],
                                        scalar1=rec[:], scalar2=None, op0=AL.mult)
                nc.vector.tensor_tensor(out=o2[:], in0=o2[:], in1=bias2[:], op=AL.add)
                nc.sync.dma_start(out=y[b * 128:(b + 1) * 128, :], in_=o2[:])

            colA = colB = colG = 0
            for b in range(NB):
                edge_block(b, colA, colB, colG, table2[0:HALF, :], table2[HALF:NPAD, :],
                           ROW2, 65, 1, 64, adtab2[:, b:b + 1], fin2)
                colA += TA[b]; colB += TB[b]; colG += T1[b]

    nc.compile()
    return nc


_CACHE = {}


def _get_program(key):
    if key not in _CACHE:
        _CACHE[key] = build_program(*key)
    return _CACHE[key]


def run(inputs, trace=False, trace_kwargs=None):
    x = np.asarray(inputs["x"], np.float32)
    TA, TB, idxA, idxB, dstl, dstf = preprocess_edges(inputs["edge_index"])
    xT, W1ext, W2ext = prep_weights(
        x, inputs["W1"], inputs["att_src1"], inputs["att_dst1"],
        inputs["W2"], inputs["att_src2"], inputs["att_dst2"])
    b1v = np.asarray(inputs["b1"], np.float32).reshape(1, D_IN)
    b2v = np.asarray(inputs["b2"], np.float32).reshape(1, 64)
    iota = np.tile(np.arange(128, dtype=np.float32), (128, 1))
    iotac = np.arange(128, dtype=np.float32).reshape(128, 1)
    ident = np.eye(128, dtype=np.float32)

    nc = _get_program((TA, TB))
    in_maps = []
    for c in range(N_CORES):
        in_maps.append({
            "xT": xT, "xT_own": np.ascontiguousarray(xT[:, c * P_NODES:(c + 1) * P_NODES]),
            "W1ext": W1ext, "W2ext": W2ext, "b1v": b1v, "b2v": b2v,
            "iota128": iota, "iotacol": iotac, "ident128": ident,
            "idxA": idxA[c], "idxB": idxB[c], "dstl": dstl[c], "dstf": dstf[c],
        })
    res = run_bass_kernel_spmd(nc, in_maps, core_ids=list(range(N_CORES)),
                               trace=trace, **(trace_kwargs or {}))
    out = np.concatenate([res.results[c]["y"] for c in range(N_CORES)], axis=0)
    return np.ascontiguousarray(out[:N_NODES]), res


def kernel(**inputs):
    out, _ = run(inputs, trace=False)
    return out


# revision 11
# speedup vs baseline: 1.9536x; 1.2252x over previous
"""2-layer GAT (PyG GATConv eval semantics) on 8 Trainium2 NeuronCores.

Sharding: nodes by contiguous id range (6272/core, 49 blocks of 128); edges
(with self loops) partitioned by destination core/block so segment softmax and
scatter-add stay local. Per layer a replicated node table (bf16 rows) is
gathered by source id via the GPSIMD dma_gather ucode; ad[dst] is expanded
per edge on the tensor engine (K=1 outer-product of local dst ids + is_equal
against a per-partition iota builds a transposed one-hot ohT[m,e]; a small
matmul ohT^T @ adblk yields ad per edge) instead of a second dma_gather.
Appended self loops are not gathered at all: each destination block gets one
"self tile" whose rows are the block's own table rows (kept in SBUF), whose
aggregation one-hot is the identity and whose ad is the block's adblk
directly. Attention weights t = exp(leaky_relu(as[src]+ad[dst])) scale the
messages in SBUF and the per-destination-block aggregation (numerator +
denominator) is a one-hot matmul accumulated in PSUM. Layer-2's table is
built per block right after the layer-1 finish and shared via one bf16
AllGather. dma_gather indices are int16, so tables are split in two halves
(A: rows < 25088, B: rest); tile counts are per-block maxima over the 8
cores so one SPMD program serves all cores with minimal padding.
"""

import numpy as np
import ml_dtypes

import concourse.bacc as bacc
import concourse.bass as bass
import concourse.mybir as mybir
import concourse.tile as tile
from concourse import library_config
from concourse.bass_utils import run_bass_kernel_spmd

N_NODES = 50000
N_CORES = 8
P_NODES = 6272                  # nodes per core (49 blocks of 128)
NPAD = P_NODES * N_CORES        # 50176
HALF = NPAD // 2                # 25088 (A/B table split, int16-safe)
NB = P_NODES // 128             # 49 destination blocks per core
TBLK = NPAD // 128              # 392 table-build blocks
BDIM = 8                        # table-build blocks per DMA batch
D_IN = 256
H1 = 4
ROW1 = 384                      # bf16 row: [h(256) | as1(4) | ad1(4) | pad], 768B
C1 = 264                        # computed cols of a layer-1 table row
ROW2 = 128                      # bf16 row: [h2(64) | as2(1) | ad2(1) | pad], 256B
C2 = 66                        # computed cols of a layer-2 table row
NEG = 0.2
EPS = 1e-16
PADV = 960.0                    # pad dst-id sentinel (bf16-exact, != 0..127)
GCHUNK = 8                      # gather tiles per dma_gather call (<=1024 idx)

F32 = mybir.dt.float32
BF16 = mybir.dt.bfloat16
I16 = mybir.dt.int16
BF = ml_dtypes.bfloat16


# ---------------------------------------------------------------- host prep

def _wrap16(vals, n_slots):
    """dma_gather index layout: index j at [j%16, j//16], replicated to all
    eight 16-partition groups."""
    a = np.zeros((16, n_slots // 16), np.int16)
    j = np.arange(len(vals))
    a[j % 16, j // 16] = vals
    return np.tile(a, (8, 1))


def preprocess_edges(edge_index):
    """Partition input edges by destination core/block (self loops handled
    separately on-device). Tile counts are per-block maxima over cores."""
    ei = np.asarray(edge_index).astype(np.int64)
    src, dst = ei[0], ei[1]

    percore = []                # [c][b] = (srcA, dlocA, srcB, dlocB)
    for c in range(N_CORES):
        lo = c * P_NODES
        m = (dst >= lo) & (dst < lo + P_NODES)
        s, d = src[m], dst[m] - lo
        blocks = []
        for b in range(NB):
            mb = (d // 128) == b
            sb_, db_ = s[mb], d[mb] - b * 128
            am = sb_ < HALF
            blocks.append((sb_[am], db_[am], sb_[~am] - HALF, db_[~am]))
        percore.append(blocks)

    TA = [max(max(1, -(-len(percore[c][b][0]) // 128)) for c in range(N_CORES))
          for b in range(NB)]
    TB = [max(max(1, -(-len(percore[c][b][2]) // 128)) for c in range(N_CORES))
          for b in range(NB)]
    T1 = [a + b for a, b in zip(TA, TB)]
    sumA, sumB, sumT = sum(TA), sum(TB), sum(T1)

    idxA = np.zeros((N_CORES, 128, sumA * 8), np.int16)
    idxB = np.zeros((N_CORES, 128, sumB * 8), np.int16)
    dstl = np.full((N_CORES, 128, sumT), PADV, np.float32)
    dstf = np.full((N_CORES, 1, sumT * 128), PADV, np.float32)

    for c in range(N_CORES):
        colA = colB = colG = 0
        for b in range(NB):
            sa, da, sb_, db_ = percore[c][b]
            va = np.zeros(TA[b] * 128, np.int64)
            va[:len(sa)] = sa
            vb = np.zeros(TB[b] * 128, np.int64)
            vb[:len(sb_)] = sb_
            idxA[c, :, colA * 8:(colA + TA[b]) * 8] = _wrap16(va, TA[b] * 128)
            idxB[c, :, colB * 8:(colB + TB[b]) * 8] = _wrap16(vb, TB[b] * 128)
            lo_sl = np.full(T1[b] * 128, PADV, np.float32)
            lo_sl[:len(da)] = da
            lo_sl[TA[b] * 128:TA[b] * 128 + len(db_)] = db_
            dstl[c, :, colG:colG + T1[b]] = lo_sl.reshape(T1[b], 128).T
            dstf[c, 0, colG * 128:(colG + T1[b]) * 128] = lo_sl
            colA += TA[b]
            colB += TB[b]
            colG += T1[b]
    return tuple(TA), tuple(TB), idxA, idxB, dstl, dstf.astype(BF)


def prep_weights(x, W1, att_src1, att_dst1, W2, att_src2, att_dst2):
    x = np.asarray(x, np.float32)
    W1 = np.asarray(W1, np.float32)
    W2 = np.asarray(W2, np.float32)
    As1 = np.einsum("khc,hc->kh", W1.reshape(D_IN, H1, 64), np.asarray(att_src1, np.float32))
    Ad1 = np.einsum("khc,hc->kh", W1.reshape(D_IN, H1, 64), np.asarray(att_dst1, np.float32))
    As2 = W2 @ np.asarray(att_src2, np.float32).reshape(64)
    Ad2 = W2 @ np.asarray(att_dst2, np.float32).reshape(64)
    W1ext = np.zeros((D_IN, ROW1), np.float32)
    W1ext[:, :256] = W1
    W1ext[:, 256:260] = As1
    W1ext[:, 260:264] = Ad1
    W2ext = np.zeros((D_IN, ROW2), np.float32)
    W2ext[:, :64] = W2
    W2ext[:, 64] = As2
    W2ext[:, 65] = Ad2
    xT = np.zeros((D_IN, NPAD), np.float32)
    xT[:, :N_NODES] = x.T
    return xT.astype(BF), W1ext.astype(BF), W2ext.astype(BF)


# ------------------------------------------------------------- bass program

def build_program(TA, TB):
    T1 = [a + b for a, b in zip(TA, TB)]
    sumA, sumB, sumT = sum(TA), sum(TB), sum(T1)
    T1MAX = max(T1)
    nc = bacc.Bacc("TRN2", target_bir_lowering=False, debug=False,
                   num_devices=N_CORES)

    xT = nc.dram_tensor("xT", [D_IN, NPAD], BF16, kind="ExternalInput").ap()
    xTo = nc.dram_tensor("xT_own", [D_IN, P_NODES], BF16, kind="ExternalInput").ap()
    w1d = nc.dram_tensor("W1ext", [D_IN, ROW1], BF16, kind="ExternalInput").ap()
    w2d = nc.dram_tensor("W2ext", [D_IN, ROW2], BF16, kind="ExternalInput").ap()
    b1d = nc.dram_tensor("b1v", [1, D_IN], F32, kind="ExternalInput").ap()
    b2d = nc.dram_tensor("b2v", [1, 64], F32, kind="ExternalInput").ap()
    iotad = nc.dram_tensor("iota128", [128, 128], F32, kind="ExternalInput").ap()
    iotacd = nc.dram_tensor("iotacol", [128, 1], F32, kind="ExternalInput").ap()
    identd = nc.dram_tensor("ident128", [128, 128], F32, kind="ExternalInput").ap()
    identbd = nc.dram_tensor("identb", [128, 128], BF16, kind="ExternalInput").ap()
    idxAd = nc.dram_tensor("idxA", [128, sumA * 8], I16, kind="ExternalInput").ap()
    idxBd = nc.dram_tensor("idxB", [128, sumB * 8], I16, kind="ExternalInput").ap()
    dstld = nc.dram_tensor("dstl", [128, sumT], F32, kind="ExternalInput").ap()
    dstfd = nc.dram_tensor("dstf", [1, sumT * 128], BF16, kind="ExternalInput").ap()
    y = nc.dram_tensor("y", [P_NODES, 64], F32, kind="ExternalOutput").ap()

    AL = mybir.AluOpType
    ACT = mybir.ActivationFunctionType

    with tile.TileContext(nc) as tc:
        with (
            tc.tile_pool(name="const", bufs=1) as cp,
            tc.tile_pool(name="sb", bufs=2) as sb,
            tc.tile_pool(name="psum", bufs=2, space="PSUM") as pp,
            tc.tile_pool(name="dram", bufs=1, space="DRAM") as dram,
        ):
            nc.gpsimd.load_library(library_config.mlp)

            tableA = dram.tile([HALF, ROW1], BF16)
            tableB = dram.tile([NPAD - HALF, ROW1], BF16)
            h2own = dram.tile([P_NODES, ROW2], BF16)
            table2 = dram.tile([NPAD, ROW2], BF16, addr_space="Shared")

            # ---------------- constants
            w1e = cp.tile([128, 2, ROW1], BF16)
            nc.sync.dma_start(out=w1e[:, 0], in_=w1d[0:128, :])
            nc.sync.dma_start(out=w1e[:, 1], in_=w1d[128:256, :])
            w2e = cp.tile([128, 2, ROW2], BF16)
            nc.sync.dma_start(out=w2e[:, 0], in_=w2d[0:128, :])
            nc.sync.dma_start(out=w2e[:, 1], in_=w2d[128:256, :])
            iota_f = cp.tile([128, 128], F32)
            nc.sync.dma_start(out=iota_f[:], in_=iotad[:])
            iota_c = cp.tile([128, 1], F32)
            nc.sync.dma_start(out=iota_c[:], in_=iotacd[:])
            ident = cp.tile([128, 128], F32)
            nc.sync.dma_start(out=ident[:], in_=identd[:])
            identb = cp.tile([128, 128], BF16)
            nc.sync.dma_start(out=identb[:], in_=identbd[:])

            b1row = cp.tile([1, D_IN], F32)
            nc.sync.dma_start(out=b1row[:], in_=b1d[:])
            b2row = cp.tile([1, 64], F32)
            nc.sync.dma_start(out=b2row[:], in_=b2d[:])
            ones1 = cp.tile([1, 128], F32)
            nc.vector.memset(ones1[:], 1.0)
            ones1b = cp.tile([1, 128], BF16)
            nc.vector.memset(ones1b[:], 1.0)
            bias_ps = pp.tile([128, D_IN], F32, tag="ps_tab")
            nc.tensor.matmul(bias_ps[:], lhsT=ones1[:], rhs=b1row[:], start=True, stop=True)
            bias1 = cp.tile([128, D_IN], F32)
            nc.vector.tensor_copy(out=bias1[:], in_=bias_ps[:])
            bias_ps2 = pp.tile([128, 64], F32, tag="ps_tab")
            nc.tensor.matmul(bias_ps2[:], lhsT=ones1[:], rhs=b2row[:], start=True, stop=True)
            bias2 = cp.tile([128, 64], F32)
            nc.vector.tensor_copy(out=bias2[:], in_=bias_ps2[:])

            idxA = cp.tile([128, sumA * 8], I16)
            nc.sync.dma_start(out=idxA[:], in_=idxAd[:])
            idxB = cp.tile([128, sumB * 8], I16)
            nc.sync.dma_start(out=idxB[:], in_=idxBd[:])
            dstl = cp.tile([128, sumT], F32)
            nc.sync.dma_start(out=dstl[:], in_=dstld[:])

            adtab1 = cp.tile([128, NB * H1], BF16)
            adtab2 = cp.tile([128, NB], BF16)
            own1 = cp.tile([128, NB, 260], BF16)   # own-node layer-1 rows (self tiles)
            own2 = cp.tile([128, NB, 65], BF16)    # own-node layer-2 rows

            # ---------------- phase 0: own-node rows + adtab1 (from xTo)
            xo = cp.tile([128, 2, P_NODES], BF16)
            nc.sync.dma_start(out=xo[:, 0], in_=xTo[0:128, :])
            nc.sync.dma_start(out=xo[:, 1], in_=xTo[128:256, :])
            for b in range(NB):
                pso = pp.tile([128, C1], F32, tag="ps_tab")
                nc.tensor.matmul(pso[:], lhsT=xo[:, 0, b * 128:(b + 1) * 128],
                                 rhs=w1e[:, 0, 0:C1], start=True, stop=False)
                nc.tensor.matmul(pso[:], lhsT=xo[:, 1, b * 128:(b + 1) * 128],
                                 rhs=w1e[:, 1, 0:C1], start=False, stop=True)
                nc.scalar.copy(out=own1[:, b], in_=pso[:, 0:260])
                nc.vector.tensor_copy(out=adtab1[:, b * H1:(b + 1) * H1], in_=pso[:, 260:264])

            # ---------------- phase 1: layer-1 node table (replicated build)
            for ch in range(TBLK // BDIM):
                c0 = ch * BDIM * 128
                xt = sb.tile([128, 2, BDIM * 128], BF16, tag="xt")
                nc.sync.dma_start(out=xt[:, 0], in_=xT[0:128, c0:c0 + BDIM * 128])
                nc.sync.dma_start(out=xt[:, 1], in_=xT[128:256, c0:c0 + BDIM * 128])
                ev = sb.tile([128, BDIM, C1], BF16, tag="ev")
                for j in range(BDIM):
                    ps = pp.tile([128, C1], F32, tag="ps_tab")
                    nc.tensor.matmul(ps[:], lhsT=xt[:, 0, j * 128:(j + 1) * 128],
                                     rhs=w1e[:, 0, 0:C1], start=True, stop=False)
                    nc.tensor.matmul(ps[:], lhsT=xt[:, 1, j * 128:(j + 1) * 128],
                                     rhs=w1e[:, 1, 0:C1], start=False, stop=True)
                    nc.scalar.copy(out=ev[:, j], in_=ps[:])
                # batched table write (rows j*128+p <-> SBUF [p, j, :])
                for tab, r0, j0, j1 in (
                    (tableA, c0, 0, min(BDIM, max(0, (HALF - c0) // 128))),
                    (tableB, c0 - HALF, max(0, (HALF - c0) // 128), BDIM),
                ):
                    if j1 <= j0:
                        continue
                    rb = r0 + j0 * 128 if tab is tableB else r0
                    nj = j1 - j0
                    view = tab[rb:rb + nj * 128, 0:C1].rearrange(
                        "(j p) f -> p j f", p=128)
                    nc.sync.dma_start(out=view, in_=ev[:, j0:j1])

            def edge_block(b, colA, colB, colG, tabA, tabB, row, cols, nh,
                           t_off, adtab_s, own_s, out_cb):
                """Gather + attention + aggregation for one destination block.
                Gathered tiles 0..nt-1, then one self tile (own-node rows)."""
                nt = T1[b]
                gb = sb.tile([128, T1MAX + 1, row], BF16, tag=f"gbuf{row}", bufs=3)
                gbv = gb

                def chunked(out_v, tab_ap, idx_t, col0, ntiles):
                    t0 = 0
                    while t0 < ntiles:
                        ct = min(GCHUNK, ntiles - t0)
                        nc.gpsimd.dma_gather(
                            out_v[:, t0:t0 + ct, :], tab_ap,
                            idx_t[:, (col0 + t0) * 8:(col0 + t0 + ct) * 8],
                            ct * 128, ct * 128, row)
                        t0 += ct

                chunked(gbv[:, 0:TA[b]], tabA, idxA, colA, TA[b])
                chunked(gbv[:, TA[b]:nt], tabB, idxB, colB, TB[b])
                nc.scalar.copy(out=gbv[:, nt, 0:cols], in_=own_s[:, 0:cols])

                # dst ids of this block's edge slots, replicated to all
                # partitions via a K=1 outer product; is_equal vs iota gives
                # the m-major one-hot ohT.
                dstf_t = sb.tile([1, T1MAX * 128], BF16, tag="dstf", bufs=3)
                nc.sync.dma_start(out=dstf_t[:, 0:nt * 128],
                                  in_=dstfd[:, colG * 128:(colG + nt) * 128])
                ohT = sb.tile([128, T1MAX * 128], BF16, tag="ohT", bufs=3)
                c0 = 0
                while c0 < nt:
                    cc = min(4, nt - c0)
                    ps_rep = pp.tile([128, 512], F32, tag="ps_rep")
                    nc.tensor.matmul(ps_rep[:, 0:cc * 128], lhsT=ones1b[:],
                                     rhs=dstf_t[:, c0 * 128:(c0 + cc) * 128],
                                     start=True, stop=True)
                    nc.vector.tensor_scalar(
                        out=ohT[:, c0 * 128:(c0 + cc) * 128],
                        in0=ps_rep[:, 0:cc * 128], scalar1=iota_c[:],
                        scalar2=None, op0=AL.is_equal)
                    c0 += cc

                # ad[dst] per edge: ohT^T @ adblk, one matmul per tile
                # (slots at stride 4 so PSUM column offsets stay 16B-aligned)
                ps_ad = pp.tile([128, T1MAX * 4], F32, tag="ps_ad", bufs=1)
                for t in range(nt):
                    nc.tensor.matmul(ps_ad[:, t * 4:t * 4 + nh],
                                     lhsT=ohT[:, t * 128:(t + 1) * 128],
                                     rhs=adtab_s, start=True, stop=True)
                ps_adv = ps_ad[:, 0:nt * 4].rearrange("p (t q) -> p t q", q=4)

                # t = exp(leaky_relu(as[src] + ad[dst])), self tile last
                as_f = sb.tile([128, (T1MAX + 1) * 4], F32, tag="asf")
                as_fv = as_f[:, 0:(nt + 1) * nh].rearrange("p (t h) -> p t h", h=nh)
                nc.scalar.copy(out=as_fv, in_=gbv[:, 0:nt + 1, t_off:t_off + nh])
                e0 = sb.tile([128, (T1MAX + 1) * 4], F32, tag="e0")
                e0v = e0[:, 0:(nt + 1) * nh].rearrange("p (t h) -> p t h", h=nh)
                nc.vector.tensor_tensor(out=e0v[:, 0:nt], in0=as_fv[:, 0:nt],
                                        in1=ps_adv[:, :, 0:nh], op=AL.add)
                nc.vector.tensor_tensor(out=e0v[:, nt], in0=as_fv[:, nt],
                                        in1=adtab_s, op=AL.add)
                e1 = sb.tile([128, (T1MAX + 1) * 4], F32, tag="e1")
                nc.vector.scalar_tensor_tensor(
                    out=e1[:, 0:(nt + 1) * nh], in0=e0[:, 0:(nt + 1) * nh],
                    scalar=NEG, in1=e0[:, 0:(nt + 1) * nh], op0=AL.mult, op1=AL.max)
                tbf = sb.tile([128, (T1MAX + 1) * 4], BF16, tag="tbf")
                nc.scalar.activation(tbf[:, 0:(nt + 1) * nh], e1[:, 0:(nt + 1) * nh], ACT.Exp)
                tv = tbf[:, 0:(nt + 1) * nh].rearrange("p (t h) -> p t h", h=nh)
                # messages in place; t into the as slot (denominator column)
                nc.vector.tensor_tensor(
                    out=gbv[:, 0:nt + 1, 0:nh * 64].rearrange("p t (h c) -> p t h c", c=64),
                    in0=gbv[:, 0:nt + 1, 0:nh * 64].rearrange("p t (h c) -> p t h c", c=64),
                    in1=tv[:, :, :, None].to_broadcast([128, nt + 1, nh, 64]),
                    op=AL.mult)
                nc.scalar.copy(out=gbv[:, 0:nt + 1, t_off:t_off + nh], in_=tv)

                # edge-major one-hot for the aggregation matmul
                oh = sb.tile([128, T1MAX * 128], BF16, tag="oh", bufs=3)
                nc.vector.tensor_tensor(
                    out=oh[:, 0:nt * 128].rearrange("p (t m) -> p t m", m=128),
                    in0=dstl[:, colG:colG + nt][:, :, None].to_broadcast([128, nt, 128]),
                    in1=iota_f[:, None, :].to_broadcast([128, nt, 128]),
                    op=AL.is_equal)

                ps_agg = pp.tile([128, cols], F32, tag="ps_agg")
                for t in range(nt):
                    nc.tensor.matmul(ps_agg[:],
                                     lhsT=oh[:, t * 128:(t + 1) * 128],
                                     rhs=gb[:, t, 0:cols],
                                     start=(t == 0), stop=False)
                nc.tensor.matmul(ps_agg[:], lhsT=identb[:], rhs=gb[:, nt, 0:cols],
                                 start=False, stop=True)
                out_cb(ps_agg, b)

            # ---------------- phase 3: layer-1 edges; fused layer-2 table
            def fin1(ps_agg, b):
                den = sb.tile([128, H1], F32, tag="den")
                nc.vector.tensor_scalar_add(den[:], ps_agg[:, 256:260], EPS)
                rec = sb.tile([128, H1], F32, tag="rec")
                nc.vector.reciprocal(rec[:], den[:])
                o1 = sb.tile([128, D_IN], F32, tag="o1")
                o1v = o1.rearrange("p (h c) -> p h c", c=64)
                nc.vector.tensor_tensor(
                    out=o1v,
                    in0=ps_agg[:, 0:256].rearrange("p (h c) -> p h c", c=64),
                    in1=rec[:, :, None].to_broadcast([128, H1, 64]),
                    op=AL.mult)
                nc.vector.tensor_tensor(out=o1[:], in0=o1[:], in1=bias1[:], op=AL.add)
                eu = sb.tile([128, D_IN], F32, tag="eu")
                nc.vector.tensor_scalar_min(eu[:], o1[:], 0.0)
                nc.scalar.activation(eu[:], eu[:], ACT.Exp)
                nc.scalar.activation(o1[:], o1[:], ACT.Relu)
                nc.vector.scalar_tensor_tensor(out=o1[:], in0=eu[:], scalar=-1.0,
                                               in1=o1[:], op0=AL.add, op1=AL.add)
                # layer-2 table row for this block (h1 must be transposed)
                h1tb = sb.tile([128, 2, 128], BF16, tag="h1tb")
                for hf in range(2):
                    tps = pp.tile([128, 128], F32, tag="ps_tr", bufs=1)
                    nc.tensor.transpose(out=tps[:], in_=o1[:, hf * 128:(hf + 1) * 128],
                                        identity=ident[:])
                    nc.vector.tensor_copy(out=h1tb[:, hf], in_=tps[:])
                ps2 = pp.tile([128, C2], F32, tag="ps_tab")
                nc.tensor.matmul(ps2[:], lhsT=h1tb[:, 0], rhs=w2e[:, 0, 0:C2],
                                 start=True, stop=False)
                nc.tensor.matmul(ps2[:], lhsT=h1tb[:, 1], rhs=w2e[:, 1, 0:C2],
                                 start=False, stop=True)
                ev2 = sb.tile([128, C2], BF16, tag="ev2")
                nc.scalar.copy(out=ev2[:], in_=ps2[:])
                nc.sync.dma_start(out=h2own[b * 128:(b + 1) * 128, 0:C2], in_=ev2[:])
                nc.scalar.copy(out=own2[:, b], in_=ev2[:, 0:65])
                nc.vector.tensor_copy(out=adtab2[:, b:b + 1], in_=ev2[:, 65:66])

            colA = colB = colG = 0
            for b in range(NB):
                edge_block(b, colA, colB, colG, tableA[:, :], tableB[:, :],
                           ROW1, 260, H1, 256, adtab1[:, b * H1:(b + 1) * H1],
                           own1[:, b], fin1)
                colA += TA[b]; colB += TB[b]; colG += T1[b]

            # ---------------- phase 4: share layer-2 table (split for overlap)
            nc.gpsimd.collective_compute(
                "AllGather", AL.bypass,
                replica_groups=[list(range(N_CORES))],
                ins=[h2own.opt()], outs=[table2.opt()])

            # ---------------- phase 6: layer-2 edges + output
            def fin2(ps_agg, b):
                den = sb.tile([128, 1], F32, tag="den")
                nc.vector.tensor_scalar_add(den[:], ps_agg[:, 64:65], EPS)
                rec = sb.tile([128, 1], F32, tag="rec")
                nc.vector.reciprocal(rec[:], den[:])
                o2 = sb.tile([128, 64], F32, tag="o2s")
                nc.vector.tensor_scalar(out=o2[:], in0=ps_agg[:, 0:64],
                                        scalar1=rec[:], scalar2=None, op0=AL.mult)
                nc.vector.tensor_tensor(out=o2[:], in0=o2[:], in1=bias2[:], op=AL.add)
                nc.sync.dma_start(out=y[b * 128:(b + 1) * 128, :], in_=o2[:])

            colA = colB = colG = 0
            for b in range(NB):
                edge_block(b, colA, colB, colG, table2[0:HALF, :], table2[HALF:NPAD, :],
                           ROW2, 65, 1, 64, adtab2[:, b:b + 1], own2[:, b], fin2)
                colA += TA[b]; colB += TB[b]; colG += T1[b]

    nc.compile()
    return nc


_CACHE = {}


def _get_program(key):
    if key not in _CACHE:
        _CACHE[key] = build_program(*key)
    return _CACHE[key]


def run(inputs, trace=False, trace_kwargs=None):
    x = np.asarray(inputs["x"], np.float32)
    TA, TB, idxA, idxB, dstl, dstf = preprocess_edges(inputs["edge_index"])
    xT, W1ext, W2ext = prep_weights(
        x, inputs["W1"], inputs["att_src1"], inputs["att_dst1"],
        inputs["W2"], inputs["att_src2"], inputs["att_dst2"])
    b1v = np.asarray(inputs["b1"], np.float32).reshape(1, D_IN)
    b2v = np.asarray(inputs["b2"], np.float32).reshape(1, 64)
    iota = np.tile(np.arange(128, dtype=np.float32), (128, 1))
    iotac = np.arange(128, dtype=np.float32).reshape(128, 1)
    ident = np.eye(128, dtype=np.float32)

    nc = _get_program((TA, TB))
    in_maps = []
    for c in range(N_CORES):
        in_maps.append({
            "xT": xT, "xT_own": np.ascontiguousarray(xT[:, c * P_NODES:(c + 1) * P_NODES]),
            "W1ext": W1ext, "W2ext": W2ext, "b1v": b1v, "b2v": b2v,
            "iota128": iota, "iotacol": iotac, "ident128": ident,
            "identb": ident.astype(BF),
            "idxA": idxA[c], "idxB": idxB[c], "dstl": dstl[c], "dstf": dstf[c],
        })
    res = run_bass_kernel_spmd(nc, in_maps, core_ids=list(range(N_CORES)),
                               trace=trace, **(trace_kwargs or {}))
    out = np.concatenate([res.results[c]["y"] for c in range(N_CORES)], axis=0)
    return np.ascontiguousarray(out[:N_NODES]), res


def kernel(**inputs):
    out, _ = run(inputs, trace=False)
    return out


# revision 13
# speedup vs baseline: 2.0042x; 1.0259x over previous
"""2-layer GAT (PyG GATConv eval semantics) on 8 Trainium2 NeuronCores.

Sharding: nodes by contiguous id range (6272/core, 49 blocks of 128); edges
(with self loops) partitioned by destination core/block so segment softmax and
scatter-add stay local. Per layer a replicated node table (bf16 rows) is
gathered by source id via the GPSIMD dma_gather ucode; ad[dst] is expanded
per edge on the tensor engine (K=1 outer-product of local dst ids + is_equal
against a per-partition iota builds a transposed one-hot ohT[m,e]; a small
matmul ohT^T @ adblk yields ad per edge) instead of a second dma_gather.
Appended self loops are not gathered at all: each destination block gets one
"self tile" whose rows are the block's own table rows (kept in SBUF), whose
aggregation one-hot is the identity and whose ad is the block's adblk
directly. Attention weights t = exp(leaky_relu(as[src]+ad[dst])) scale the
messages in SBUF and the per-destination-block aggregation (numerator +
denominator) is a one-hot matmul accumulated in PSUM. The block loop is
software-pipelined (gathers + ohT of block b+1 issue before the attention/
aggregation of block b) so the strictly-ordered PE/DVE queues never head-of-
line block the next block's gathers. Layer-2's table rows are built per block
right after the layer-1 finish and shared via two bf16 AllGathers: rows of
blocks 0-24 gather mid-way through the layer-1 loop (fully hidden), the rest
at its end; layer-2 gather indices use the matching gathered-row layout.
dma_gather indices are int16, so each layer's table is split in two halves.
Tile counts are per-block maxima over the 8 cores so one SPMD program serves
all cores with minimal padding.
"""

import numpy as np
import ml_dtypes

import concourse.bacc as bacc
import concourse.bass as bass
import concourse.mybir as mybir
import concourse.tile as tile
from concourse import library_config
from concourse.bass_utils import run_bass_kernel_spmd

N_NODES = 50000
N_CORES = 8
P_NODES = 6272                  # nodes per core (49 blocks of 128)
NPAD = P_NODES * N_CORES        # 50176
HALF = NPAD // 2                # 25088 (layer-1 A/B table split, int16-safe)
NB = P_NODES // 128             # 49 destination blocks per core
TBLK = NPAD // 128              # 392 table-build blocks
BDIM = 8                        # table-build blocks per DMA batch
SPLIT2 = 3200                   # per-core row split for layer-2 AllGather (25 blocks)
NBSPLIT = SPLIT2 // 128         # 25
D_IN = 256
H1 = 4
ROW1 = 384                      # bf16 row: [h(256) | as1(4) | ad1(4) | pad], 768B
ROW2 = 128                      # bf16 row: [h2(64) | as2(1) | ad2(1) | pad], 256B
C2 = 66                         # computed cols of a layer-2 table row
NEG = 0.2
EPS = 1e-16
PADV = 960.0                    # pad dst-id sentinel (bf16-exact, != 0..127)
GCHUNK = 8                      # gather tiles per dma_gather call (<=1024 idx)

F32 = mybir.dt.float32
BF16 = mybir.dt.bfloat16
I16 = mybir.dt.int16
BF = ml_dtypes.bfloat16


# ---------------------------------------------------------------- host prep

def _wrap16(vals, n_slots):
    """dma_gather index layout: index j at [j%16, j//16], replicated to all
    eight 16-partition groups."""
    a = np.zeros((16, n_slots // 16), np.int16)
    j = np.arange(len(vals))
    a[j % 16, j // 16] = vals
    return np.tile(a, (8, 1))


def _layer_pack(percore):
    """Pack per-(core, block) A/B edge lists into uniform tiles.

    percore[c][b] = (rowA, dlocA, rowB, dlocB) with rows already mapped into
    the layer's A/B table row spaces. Returns per-block tile counts (maxima
    over cores) and the packed idx/dstl/dstf arrays."""
    TA = [max(max(1, -(-len(percore[c][b][0]) // 128)) for c in range(N_CORES))
          for b in range(NB)]
    TB = [max(max(1, -(-len(percore[c][b][2]) // 128)) for c in range(N_CORES))
          for b in range(NB)]
    T1 = [a + b for a, b in zip(TA, TB)]
    sumA, sumB, sumT = sum(TA), sum(TB), sum(T1)

    idxA = np.zeros((N_CORES, 128, sumA * 8), np.int16)
    idxB = np.zeros((N_CORES, 128, sumB * 8), np.int16)
    dstl = np.full((N_CORES, 128, sumT), PADV, np.float32)
    dstf = np.full((N_CORES, 1, sumT * 128), PADV, np.float32)

    for c in range(N_CORES):
        colA = colB = colG = 0
        for b in range(NB):
            ra, da, rb_, db_ = percore[c][b]
            va = np.zeros(TA[b] * 128, np.int64)
            va[:len(ra)] = ra
            vb = np.zeros(TB[b] * 128, np.int64)
            vb[:len(rb_)] = rb_
            idxA[c, :, colA * 8:(colA + TA[b]) * 8] = _wrap16(va, TA[b] * 128)
            idxB[c, :, colB * 8:(colB + TB[b]) * 8] = _wrap16(vb, TB[b] * 128)
            lo_sl = np.full(T1[b] * 128, PADV, np.float32)
            lo_sl[:len(da)] = da
            lo_sl[TA[b] * 128:TA[b] * 128 + len(db_)] = db_
            dstl[c, :, colG:colG + T1[b]] = lo_sl.reshape(T1[b], 128).T
            dstf[c, 0, colG * 128:(colG + T1[b]) * 128] = lo_sl
            colA += TA[b]
            colB += TB[b]
            colG += T1[b]
    return (tuple(TA), tuple(TB)), idxA, idxB, dstl, dstf.astype(BF)


def preprocess_edges(edge_index):
    """Partition input edges by destination core/block (self loops handled
    separately on-device) and build both layers' gather index layouts."""
    ei = np.asarray(edge_index).astype(np.int64)
    src, dst = ei[0], ei[1]

    per1, per2 = [], []
    for c in range(N_CORES):
        lo = c * P_NODES
        m = (dst >= lo) & (dst < lo + P_NODES)
        s, d = src[m], dst[m] - lo
        bl1, bl2 = [], []
        for b in range(NB):
            mb = (d // 128) == b
            sb_, db_ = s[mb], d[mb] - b * 128
            # layer 1: table rows = global node id, split at HALF
            am = sb_ < HALF
            bl1.append((sb_[am], db_[am], sb_[~am] - HALF, db_[~am]))
            # layer 2: AllGather layout — rows c*3200+i (i<3200) | c*3072+(i-3200)
            sc, si = np.divmod(sb_, P_NODES)
            am2 = si < SPLIT2
            rowA = sc * SPLIT2 + si
            rowB = sc * (P_NODES - SPLIT2) + (si - SPLIT2)
            bl2.append((rowA[am2], db_[am2], rowB[~am2], db_[~am2]))
        per1.append(bl1)
        per2.append(bl2)
    return _layer_pack(per1), _layer_pack(per2)


def prep_weights(x, W1, att_src1, att_dst1, W2, att_src2, att_dst2):
    x = np.asarray(x, np.float32)
    W1 = np.asarray(W1, np.float32)
    W2 = np.asarray(W2, np.float32)
    As1 = np.einsum("khc,hc->kh", W1.reshape(D_IN, H1, 64), np.asarray(att_src1, np.float32))
    Ad1 = np.einsum("khc,hc->kh", W1.reshape(D_IN, H1, 64), np.asarray(att_dst1, np.float32))
    As2 = W2 @ np.asarray(att_src2, np.float32).reshape(64)
    Ad2 = W2 @ np.asarray(att_dst2, np.float32).reshape(64)
    W1ext = np.zeros((D_IN, ROW1), np.float32)
    W1ext[:, :256] = W1
    W1ext[:, 256:260] = As1
    W1ext[:, 260:264] = Ad1
    W2ext = np.zeros((D_IN, ROW2), np.float32)
    W2ext[:, :64] = W2
    W2ext[:, 64] = As2
    W2ext[:, 65] = Ad2
    xT = np.zeros((D_IN, NPAD), np.float32)
    xT[:, :N_NODES] = x.T
    return xT.astype(BF), W1ext.astype(BF), W2ext.astype(BF)


# ------------------------------------------------------------- bass program

def build_program(key1, key2):
    TA1, TB1 = key1
    TA2, TB2 = key2
    T11 = [a + b for a, b in zip(TA1, TB1)]
    T12 = [a + b for a, b in zip(TA2, TB2)]
    sumA1, sumB1, sumT1 = sum(TA1), sum(TB1), sum(T11)
    sumA2, sumB2, sumT2 = sum(TA2), sum(TB2), sum(T12)
    T1MAX = max(max(T11), max(T12))
    GBMAX = {ROW1: max(T11) + 1, ROW2: max(T12) + 1}
    nc = bacc.Bacc("TRN2", target_bir_lowering=False, debug=False,
                   num_devices=N_CORES)

    xT = nc.dram_tensor("xT", [D_IN, NPAD], BF16, kind="ExternalInput").ap()
    xTo = nc.dram_tensor("xT_own", [D_IN, P_NODES], BF16, kind="ExternalInput").ap()
    w1d = nc.dram_tensor("W1ext", [D_IN, ROW1], BF16, kind="ExternalInput").ap()
    w2d = nc.dram_tensor("W2ext", [D_IN, ROW2], BF16, kind="ExternalInput").ap()
    b1d = nc.dram_tensor("b1v", [1, D_IN], F32, kind="ExternalInput").ap()
    b2d = nc.dram_tensor("b2v", [1, 64], F32, kind="ExternalInput").ap()
    iotad = nc.dram_tensor("iota128", [128, 128], F32, kind="ExternalInput").ap()
    iotacd = nc.dram_tensor("iotacol", [128, 1], F32, kind="ExternalInput").ap()
    identd = nc.dram_tensor("ident128", [128, 128], F32, kind="ExternalInput").ap()
    identbd = nc.dram_tensor("identb", [128, 128], BF16, kind="ExternalInput").ap()
    idxA1d = nc.dram_tensor("idxA1", [128, sumA1 * 8], I16, kind="ExternalInput").ap()
    idxB1d = nc.dram_tensor("idxB1", [128, sumB1 * 8], I16, kind="ExternalInput").ap()
    dstl1d = nc.dram_tensor("dstl1", [128, sumT1], F32, kind="ExternalInput").ap()
    dstf1d = nc.dram_tensor("dstf1", [1, sumT1 * 128], BF16, kind="ExternalInput").ap()
    idxA2d = nc.dram_tensor("idxA2", [128, sumA2 * 8], I16, kind="ExternalInput").ap()
    idxB2d = nc.dram_tensor("idxB2", [128, sumB2 * 8], I16, kind="ExternalInput").ap()
    dstl2d = nc.dram_tensor("dstl2", [128, sumT2], F32, kind="ExternalInput").ap()
    dstf2d = nc.dram_tensor("dstf2", [1, sumT2 * 128], BF16, kind="ExternalInput").ap()
    y = nc.dram_tensor("y", [P_NODES, 64], F32, kind="ExternalOutput").ap()

    AL = mybir.AluOpType
    ACT = mybir.ActivationFunctionType

    with tile.TileContext(nc) as tc:
        with (
            tc.tile_pool(name="const", bufs=1) as cp,
            tc.tile_pool(name="sb", bufs=2) as sb,
            tc.tile_pool(name="psum", bufs=2, space="PSUM") as pp,
            tc.tile_pool(name="dram", bufs=1, space="DRAM") as dram,
        ):
            nc.gpsimd.load_library(library_config.mlp)

            tableA = dram.tile([HALF, ROW1], BF16)
            tableB = dram.tile([NPAD - HALF, ROW1], BF16)
            h2own = dram.tile([P_NODES, ROW2], BF16)
            table2a = dram.tile([N_CORES * SPLIT2, ROW2], BF16, addr_space="Shared")
            table2b = dram.tile([NPAD - N_CORES * SPLIT2, ROW2], BF16, addr_space="Shared")

            # ---------------- constants
            w1e = cp.tile([128, 2, ROW1], BF16)
            nc.sync.dma_start(out=w1e[:, 0], in_=w1d[0:128, :])
            nc.sync.dma_start(out=w1e[:, 1], in_=w1d[128:256, :])
            w2e = cp.tile([128, 2, ROW2], BF16)
            nc.sync.dma_start(out=w2e[:, 0], in_=w2d[0:128, :])
            nc.sync.dma_start(out=w2e[:, 1], in_=w2d[128:256, :])
            iota_f = cp.tile([128, 128], F32)
            nc.sync.dma_start(out=iota_f[:], in_=iotad[:])
            iota_c = cp.tile([128, 1], F32)
            nc.sync.dma_start(out=iota_c[:], in_=iotacd[:])
            ident = cp.tile([128, 128], F32)
            nc.sync.dma_start(out=ident[:], in_=identd[:])
            identb = cp.tile([128, 128], BF16)
            nc.sync.dma_start(out=identb[:], in_=identbd[:])

            b1row = cp.tile([1, D_IN], F32)
            nc.sync.dma_start(out=b1row[:], in_=b1d[:])
            b2row = cp.tile([1, 64], F32)
            nc.sync.dma_start(out=b2row[:], in_=b2d[:])
            ones1 = cp.tile([1, 128], F32)
            nc.vector.memset(ones1[:], 1.0)
            ones1b = cp.tile([1, 128], BF16)
            nc.vector.memset(ones1b[:], 1.0)
            bias_ps = pp.tile([128, D_IN], F32, tag="ps_tab")
            nc.tensor.matmul(bias_ps[:], lhsT=ones1[:], rhs=b1row[:], start=True, stop=True)
            bias1 = cp.tile([128, D_IN], F32)
            nc.vector.tensor_copy(out=bias1[:], in_=bias_ps[:])
            bias_ps2 = pp.tile([128, 64], F32, tag="ps_tab")
            nc.tensor.matmul(bias_ps2[:], lhsT=ones1[:], rhs=b2row[:], start=True, stop=True)
            bias2 = cp.tile([128, 64], F32)
            nc.vector.tensor_copy(out=bias2[:], in_=bias_ps2[:])

            idxA1 = cp.tile([128, sumA1 * 8], I16)
            nc.sync.dma_start(out=idxA1[:], in_=idxA1d[:])
            idxB1 = cp.tile([128, sumB1 * 8], I16)
            nc.sync.dma_start(out=idxB1[:], in_=idxB1d[:])
            dstl1 = cp.tile([128, sumT1], F32)
            nc.sync.dma_start(out=dstl1[:], in_=dstl1d[:])
            idxA2 = cp.tile([128, sumA2 * 8], I16)
            nc.sync.dma_start(out=idxA2[:], in_=idxA2d[:])
            idxB2 = cp.tile([128, sumB2 * 8], I16)
            nc.sync.dma_start(out=idxB2[:], in_=idxB2d[:])
            dstl2 = cp.tile([128, sumT2], F32)
            nc.sync.dma_start(out=dstl2[:], in_=dstl2d[:])

            adtab1 = cp.tile([128, NB * H1], BF16)
            adtab2 = cp.tile([128, NB], BF16)
            own1 = cp.tile([128, NB, 260], BF16)   # own-node layer-1 rows (self tiles)
            own2 = cp.tile([128, NB, 65], BF16)    # own-node layer-2 rows

            # ---------------- phase 0: own-node rows + adtab1 (from xTo)
            NBH = 25
            for hbase in (0, NBH):
                nb_h = min(NBH, NB - hbase)
                xo = sb.tile([128, 2, NBH * 128], BF16, tag="xo", bufs=1)
                nc.sync.dma_start(out=xo[:, 0, 0:nb_h * 128],
                                  in_=xTo[0:128, hbase * 128:(hbase + nb_h) * 128])
                nc.sync.dma_start(out=xo[:, 1, 0:nb_h * 128],
                                  in_=xTo[128:256, hbase * 128:(hbase + nb_h) * 128])
                for j in range(nb_h):
                    b = hbase + j
                    pso = pp.tile([128, ROW1], F32, tag="ps_tab")
                    nc.tensor.matmul(pso[:], lhsT=xo[:, 0, j * 128:(j + 1) * 128],
                                     rhs=w1e[:, 0], start=True, stop=False)
                    nc.tensor.matmul(pso[:], lhsT=xo[:, 1, j * 128:(j + 1) * 128],
                                     rhs=w1e[:, 1], start=False, stop=True)
                    nc.scalar.copy(out=own1[:, b], in_=pso[:, 0:260])
                    nc.vector.tensor_copy(out=adtab1[:, b * H1:(b + 1) * H1], in_=pso[:, 260:264])

            # ---------------- phase 1: layer-1 node table (replicated build)
            for ch in range(TBLK // BDIM):
                c0 = ch * BDIM * 128
                xt = sb.tile([128, 2, BDIM * 128], BF16, tag="xt")
                nc.sync.dma_start(out=xt[:, 0], in_=xT[0:128, c0:c0 + BDIM * 128])
                nc.sync.dma_start(out=xt[:, 1], in_=xT[128:256, c0:c0 + BDIM * 128])
                ev = sb.tile([128, BDIM, ROW1], BF16, tag="ev")
                for j in range(BDIM):
                    ps = pp.tile([128, ROW1], F32, tag="ps_tab")
                    nc.tensor.matmul(ps[:], lhsT=xt[:, 0, j * 128:(j + 1) * 128],
                                     rhs=w1e[:, 0], start=True, stop=False)
                    nc.tensor.matmul(ps[:], lhsT=xt[:, 1, j * 128:(j + 1) * 128],
                                     rhs=w1e[:, 1], start=False, stop=True)
                    nc.scalar.copy(out=ev[:, j], in_=ps[:])
                # batched, fully-contiguous table write (row j*128+p <-> [p, j, :])
                jsplit = min(BDIM, max(0, (HALF - c0) // 128))
                for tab, rb, j0, j1 in (
                    (tableA, c0, 0, jsplit),
                    (tableB, c0 + jsplit * 128 - HALF, jsplit, BDIM),
                ):
                    if j1 <= j0:
                        continue
                    nj = j1 - j0
                    view = tab[rb:rb + nj * 128, :].rearrange("(j p) f -> p j f", p=128)
                    nc.sync.dma_start(out=view, in_=ev[:, j0:j1])

            # -------- software-pipelined edge phase (per destination block)
            def stage1(b, lay):
                """Issue gathers + build ohT for block b; returns live tiles."""
                (TAl, TBl, T1l, colA, colB, colG,
                 idxA_t, idxB_t, dstl_t, dstfd_ap, tabA, tabB,
                 row, cols, nh, t_off, adt, ownt, fin) = lay
                nt = T1l[b]
                gb = sb.tile([128, GBMAX[row], row], BF16, tag=f"gbuf{row}", bufs=3)

                def chunked(out_v, tab_ap, idx_t, col0, ntiles):
                    t0 = 0
                    while t0 < ntiles:
                        ct = min(GCHUNK, ntiles - t0)
                        nc.gpsimd.dma_gather(
                            out_v[:, t0:t0 + ct, :], tab_ap,
                            idx_t[:, (col0 + t0) * 8:(col0 + t0 + ct) * 8],
                            ct * 128, ct * 128, row)
                        t0 += ct

                chunked(gb[:, 0:TAl[b]], tabA, idxA_t, colA[b], TAl[b])
                chunked(gb[:, TAl[b]:nt], tabB, idxB_t, colB[b], TBl[b])
                nc.scalar.copy(out=gb[:, nt, 0:cols], in_=ownt[:, b, 0:cols])

                # dst ids of this block's edge slots, replicated to all
                # partitions via a K=1 outer product; is_equal vs iota gives
                # the m-major one-hot ohT.
                dstf_t = sb.tile([1, T1MAX * 128], BF16, tag="dstf", bufs=3)
                nc.sync.dma_start(out=dstf_t[:, 0:nt * 128],
                                  in_=dstfd_ap[:, colG[b] * 128:(colG[b] + nt) * 128])
                ohT = sb.tile([128, T1MAX * 128], BF16, tag="ohT", bufs=3)
                c0 = 0
                while c0 < nt:
                    cc = min(4, nt - c0)
                    ps_rep = pp.tile([128, 512], F32, tag="ps_rep")
                    nc.tensor.matmul(ps_rep[:, 0:cc * 128], lhsT=ones1b[:],
                                     rhs=dstf_t[:, c0 * 128:(c0 + cc) * 128],
                                     start=True, stop=True)
                    nc.vector.tensor_scalar(
                        out=ohT[:, c0 * 128:(c0 + cc) * 128],
                        in0=ps_rep[:, 0:cc * 128], scalar1=iota_c[:],
                        scalar2=None, op0=AL.is_equal)
                    c0 += cc
                return gb, ohT

            def stage2(b, lay, gb, ohT):
                """Attention + aggregation + finish for block b."""
                (TAl, TBl, T1l, colA, colB, colG,
                 idxA_t, idxB_t, dstl_t, dstfd_ap, tabA, tabB,
                 row, cols, nh, t_off, adt, ownt, fin) = lay
                nt = T1l[b]
                adtab_s = adt(b)

                # ad[dst] per edge: ohT^T @ adblk, one matmul per tile
                # (slots at stride 4 so PSUM column offsets stay 16B-aligned)
                ps_ad = pp.tile([128, T1MAX * 4], F32, tag="ps_ad", bufs=1)
                for t in range(nt):
                    nc.tensor.matmul(ps_ad[:, t * 4:t * 4 + nh],
                                     lhsT=ohT[:, t * 128:(t + 1) * 128],
                                     rhs=adtab_s, start=True, stop=True)
                ps_adv = ps_ad[:, 0:nt * 4].rearrange("p (t q) -> p t q", q=4)

                # t = exp(leaky_relu(as[src] + ad[dst])), self tile last
                as_f = sb.tile([128, (T1MAX + 1) * 4], F32, tag="asf")
                as_fv = as_f[:, 0:(nt + 1) * nh].rearrange("p (t h) -> p t h", h=nh)
                nc.scalar.copy(out=as_fv, in_=gb[:, 0:nt + 1, t_off:t_off + nh])
                e0 = sb.tile([128, (T1MAX + 1) * 4], F32, tag="e0")
                e0v = e0[:, 0:(nt + 1) * nh].rearrange("p (t h) -> p t h", h=nh)
                nc.vector.tensor_tensor(out=e0v[:, 0:nt], in0=as_fv[:, 0:nt],
                                        in1=ps_adv[:, :, 0:nh], op=AL.add)
                nc.vector.tensor_tensor(out=e0v[:, nt], in0=as_fv[:, nt],
                                        in1=adtab_s, op=AL.add)
                e1 = sb.tile([128, (T1MAX + 1) * 4], F32, tag="e1")
                nc.vector.scalar_tensor_tensor(
                    out=e1[:, 0:(nt + 1) * nh], in0=e0[:, 0:(nt + 1) * nh],
                    scalar=NEG, in1=e0[:, 0:(nt + 1) * nh], op0=AL.mult, op1=AL.max)
                tbf = sb.tile([128, (T1MAX + 1) * 4], BF16, tag="tbf")
                nc.scalar.activation(tbf[:, 0:(nt + 1) * nh], e1[:, 0:(nt + 1) * nh], ACT.Exp)
                tv = tbf[:, 0:(nt + 1) * nh].rearrange("p (t h) -> p t h", h=nh)
                # messages in place; t into the as slot (denominator column)
                nc.vector.tensor_tensor(
                    out=gb[:, 0:nt + 1, 0:nh * 64].rearrange("p t (h c) -> p t h c", c=64),
                    in0=gb[:, 0:nt + 1, 0:nh * 64].rearrange("p t (h c) -> p t h c", c=64),
                    in1=tv[:, :, :, None].to_broadcast([128, nt + 1, nh, 64]),
                    op=AL.mult)
                nc.scalar.copy(out=gb[:, 0:nt + 1, t_off:t_off + nh], in_=tv)

                # edge-major one-hot for the aggregation matmul
                oh = sb.tile([128, T1MAX * 128], BF16, tag="oh", bufs=2)
                nc.vector.tensor_tensor(
                    out=oh[:, 0:nt * 128].rearrange("p (t m) -> p t m", m=128),
                    in0=dstl_t[:, colG[b]:colG[b] + nt][:, :, None].to_broadcast([128, nt, 128]),
                    in1=iota_f[:, None, :].to_broadcast([128, nt, 128]),
                    op=AL.is_equal)

                ps_agg = pp.tile([128, cols], F32, tag="ps_agg")
                for t in range(nt):
                    nc.tensor.matmul(ps_agg[:],
                                     lhsT=oh[:, t * 128:(t + 1) * 128],
                                     rhs=gb[:, t, 0:cols],
                                     start=(t == 0), stop=False)
                nc.tensor.matmul(ps_agg[:], lhsT=identb[:], rhs=gb[:, nt, 0:cols],
                                 start=False, stop=True)
                fin(ps_agg, b)

            def run_blocks(lay, after_block=None):
                prev = None
                for b in range(NB):
                    cur = stage1(b, lay)
                    if prev is not None:
                        stage2(b - 1, lay, *prev)
                        if after_block is not None:
                            after_block(b - 1)
                    prev = cur
                stage2(NB - 1, lay, *prev)
                if after_block is not None:
                    after_block(NB - 1)

            # ---------------- phase 3: layer-1 edges; fused layer-2 table
            def fin1(ps_agg, b):
                den = sb.tile([128, H1], F32, tag="den")
                nc.vector.tensor_scalar_add(den[:], ps_agg[:, 256:260], EPS)
                rec = sb.tile([128, H1], F32, tag="rec")
                nc.vector.reciprocal(rec[:], den[:])
                o1 = sb.tile([128, D_IN], F32, tag="o1")
                o1v = o1.rearrange("p (h c) -> p h c", c=64)
                nc.vector.tensor_tensor(
                    out=o1v,
                    in0=ps_agg[:, 0:256].rearrange("p (h c) -> p h c", c=64),
                    in1=rec[:, :, None].to_broadcast([128, H1, 64]),
                    op=AL.mult)
                nc.vector.tensor_tensor(out=o1[:], in0=o1[:], in1=bias1[:], op=AL.add)
                eu = sb.tile([128, D_IN], F32, tag="eu")
                nc.vector.tensor_scalar_min(eu[:], o1[:], 0.0)
                nc.scalar.activation(eu[:], eu[:], ACT.Exp)
                nc.scalar.activation(o1[:], o1[:], ACT.Relu)
                nc.vector.scalar_tensor_tensor(out=o1[:], in0=eu[:], scalar=-1.0,
                                               in1=o1[:], op0=AL.add, op1=AL.add)
                # layer-2 table row for this block (h1 must be transposed)
                h1tb = sb.tile([128, 2, 128], BF16, tag="h1tb")
                for hf in range(2):
                    tps = pp.tile([128, 512], F32, tag="ps_rep")
                    nc.tensor.transpose(out=tps[:, 0:128], in_=o1[:, hf * 128:(hf + 1) * 128],
                                        identity=ident[:])
                    nc.vector.tensor_copy(out=h1tb[:, hf], in_=tps[:, 0:128])
                ps2 = pp.tile([128, C2], F32, tag="ps_tab")
                nc.tensor.matmul(ps2[:], lhsT=h1tb[:, 0], rhs=w2e[:, 0, 0:C2],
                                 start=True, stop=False)
                nc.tensor.matmul(ps2[:], lhsT=h1tb[:, 1], rhs=w2e[:, 1, 0:C2],
                                 start=False, stop=True)
                ev2 = sb.tile([128, C2], BF16, tag="ev2")
                nc.scalar.copy(out=ev2[:], in_=ps2[:])
                nc.sync.dma_start(out=h2own[b * 128:(b + 1) * 128, 0:C2], in_=ev2[:])
                nc.scalar.copy(out=own2[:, b], in_=ev2[:, 0:65])
                nc.vector.tensor_copy(out=adtab2[:, b:b + 1], in_=ev2[:, 65:66])

            def cums(tl):
                c, out = 0, []
                for v in tl:
                    out.append(c)
                    c += v
                return out

            lay1 = (TA1, TB1, T11, cums(TA1), cums(TB1), cums(T11),
                    idxA1, idxB1, dstl1, dstf1d, tableA[:, :], tableB[:, :],
                    ROW1, 260, H1, 256,
                    lambda b: adtab1[:, b * H1:(b + 1) * H1], own1, fin1)

            def ag_after(b):
                if b == NBSPLIT - 1:
                    nc.gpsimd.collective_compute(
                        "AllGather", AL.bypass,
                        replica_groups=[list(range(N_CORES))],
                        ins=[h2own[0:SPLIT2, :].opt()], outs=[table2a.opt()])
                elif b == NB - 1:
                    nc.gpsimd.collective_compute(
                        "AllGather", AL.bypass,
                        replica_groups=[list(range(N_CORES))],
                        ins=[h2own[SPLIT2:P_NODES, :].opt()], outs=[table2b.opt()])

            run_blocks(lay1, after_block=ag_after)

            # ---------------- phase 6: layer-2 edges + output
            def fin2(ps_agg, b):
                den = sb.tile([128, 1], F32, tag="den")
                nc.vector.tensor_scalar_add(den[:], ps_agg[:, 64:65], EPS)
                rec = sb.tile([128, 1], F32, tag="rec")
                nc.vector.reciprocal(rec[:], den[:])
                o2 = sb.tile([128, 64], F32, tag="o2s")
                nc.vector.tensor_scalar(out=o2[:], in0=ps_agg[:, 0:64],
                                        scalar1=rec[:], scalar2=None, op0=AL.mult)
                nc.vector.tensor_tensor(out=o2[:], in0=o2[:], in1=bias2[:], op=AL.add)
                nc.sync.dma_start(out=y[b * 128:(b + 1) * 128, :], in_=o2[:])

            lay2 = (TA2, TB2, T12, cums(TA2), cums(TB2), cums(T12),
                    idxA2, idxB2, dstl2, dstf2d, table2a[:, :], table2b[:, :],
                    ROW2, 65, 1, 64,
                    lambda b: adtab2[:, b:b + 1], own2, fin2)
            run_blocks(lay2)

    nc.compile()
    return nc


_CACHE = {}


def _get_program(key):
    if key not in _CACHE:
        _CACHE[key] = build_program(*key)
    return _CACHE[key]


def run(inputs, trace=False, trace_kwargs=None):
    x = np.asarray(inputs["x"], np.float32)
    (key1, idxA1, idxB1, dstl1, dstf1), (key2, idxA2, idxB2, dstl2, dstf2) = \
        preprocess_edges(inputs["edge_index"])
    xT, W1ext, W2ext = prep_weights(
        x, inputs["W1"], inputs["att_src1"], inputs["att_dst1"],
        inputs["W2"], inputs["att_src2"], inputs["att_dst2"])
    b1v = np.asarray(inputs["b1"], np.float32).reshape(1, D_IN)
    b2v = np.asarray(inputs["b2"], np.float32).reshape(1, 64)
    iota = np.tile(np.arange(128, dtype=np.float32), (128, 1))
    iotac = np.arange(128, dtype=np.float32).reshape(128, 1)
    ident = np.eye(128, dtype=np.float32)

    nc = _get_program((key1, key2))
    in_maps = []
    for c in range(N_CORES):
        in_maps.append({
            "xT": xT, "xT_own": np.ascontiguousarray(xT[:, c * P_NODES:(c + 1) * P_NODES]),
            "W1ext": W1ext, "W2ext": W2ext, "b1v": b1v, "b2v": b2v,
            "iota128": iota, "iotacol": iotac, "ident128": ident,
            "identb": ident.astype(BF),
            "idxA1": idxA1[c], "idxB1": idxB1[c], "dstl1": dstl1[c], "dstf1": dstf1[c],
            "idxA2": idxA2[c], "idxB2": idxB2[c], "dstl2": dstl2[c], "dstf2": dstf2[c],
        })
    res = run_bass_kernel_spmd(nc, in_maps, core_ids=list(range(N_CORES)),
                               trace=trace, **(trace_kwargs or {}))
    out = np.concatenate([res.results[c]["y"] for c in range(N_CORES)], axis=0)
    return np.ascontiguousarray(out[:N_NODES]), res


def kernel(**inputs):
    out, _ = run(inputs, trace=False)
    return out


# revision 16
# speedup vs baseline: 2.1459x; 1.0707x over previous
"""2-layer GAT (PyG GATConv eval semantics) on 8 Trainium2 NeuronCores.

Sharding: nodes by contiguous id range (6272/core, 49 blocks of 128); edges
(with self loops) partitioned by destination core/block so segment softmax and
scatter-add stay local. Per layer a replicated node table (bf16 rows) is
gathered by source id via the GPSIMD dma_gather ucode; ad[dst] is expanded
per edge on the tensor engine (K=1 outer-product of local dst ids + is_equal
against a per-partition iota builds a transposed one-hot ohT[m,e]; a small
matmul ohT^T @ adblk yields ad per edge) instead of a second dma_gather.
Appended self loops are not gathered at all: each destination block gets one
"self tile" whose rows are the block's own table rows (kept in SBUF), whose
aggregation one-hot is the identity and whose ad is the block's adblk
directly. Attention weights t = exp(leaky_relu(as[src]+ad[dst])) scale the
messages in SBUF and the per-destination-block aggregation (numerator +
denominator) is a one-hot matmul accumulated in PSUM. The block loop is
software-pipelined (gathers + ohT of block b+1 issue before the attention/
aggregation of block b) so the strictly-ordered PE/DVE queues never head-of-
line block the next block's gathers. Layer-2's table rows are built per block
right after the layer-1 finish and shared via two bf16 AllGathers: rows of
blocks 0-24 gather mid-way through the layer-1 loop (fully hidden), the rest
at its end; layer-2 gather indices use the matching gathered-row layout.
dma_gather indices are int16, so each layer's table is split in two halves.
Tile counts are per-block maxima over the 8 cores so one SPMD program serves
all cores with minimal padding.
"""

import numpy as np
import ml_dtypes

import concourse.bacc as bacc
import concourse.bass as bass
import concourse.mybir as mybir
import concourse.tile as tile
from concourse import library_config
from concourse.bass_utils import run_bass_kernel_spmd

N_NODES = 50000
N_CORES = 8
P_NODES = 6272                  # nodes per core (49 blocks of 128)
NPAD = P_NODES * N_CORES        # 50176
HALF = NPAD // 2                # 25088 (layer-1 A/B table split, int16-safe)
NB = P_NODES // 128             # 49 destination blocks per core
TBLK = NPAD // 128              # 392 table-build blocks
BDIM = 8                        # table-build blocks per DMA batch
SPLIT2 = 3200                   # per-core row split for layer-2 AllGather (25 blocks)
NBSPLIT = SPLIT2 // 128         # 25
D_IN = 256
H1 = 4
ROW1 = 384                      # bf16 row: [h(256) | as1(4) | ad1(4) | pad], 768B
ROW2 = 128                      # bf16 row: [h2(64) | as2(1) | ad2(1) | pad], 256B
C2 = 66                         # computed cols of a layer-2 table row
NEG = 0.2
EPS = 1e-16
PADV = 960.0                    # pad dst-id sentinel (bf16-exact, != 0..127)
GCHUNK = 8                      # gather tiles per dma_gather call (<=1024 idx)

F32 = mybir.dt.float32
BF16 = mybir.dt.bfloat16
I16 = mybir.dt.int16
BF = ml_dtypes.bfloat16


# ---------------------------------------------------------------- host prep

def _wrap16(vals, n_slots):
    """dma_gather index layout: index j at [j%16, j//16], replicated to all
    eight 16-partition groups."""
    a = np.zeros((16, n_slots // 16), np.int16)
    j = np.arange(len(vals))
    a[j % 16, j // 16] = vals
    return np.tile(a, (8, 1))


def _layer_pack(percore):
    """Pack per-(core, block) A/B edge lists into uniform tiles.

    percore[c][b] = (rowA, dlocA, rowB, dlocB) with rows already mapped into
    the layer's A/B table row spaces. Returns per-block tile counts (maxima
    over cores) and the packed idx/dstl/dstf arrays."""
    TA = [max(max(1, -(-len(percore[c][b][0]) // 128)) for c in range(N_CORES))
          for b in range(NB)]
    TB = [max(max(1, -(-len(percore[c][b][2]) // 128)) for c in range(N_CORES))
          for b in range(NB)]
    T1 = [a + b for a, b in zip(TA, TB)]
    sumA, sumB, sumT = sum(TA), sum(TB), sum(T1)

    idxA = np.zeros((N_CORES, 128, sumA * 8), np.int16)
    idxB = np.zeros((N_CORES, 128, sumB * 8), np.int16)
    dstl = np.full((N_CORES, 128, sumT), PADV, np.float32)
    dstf = np.full((N_CORES, 1, sumT * 128), PADV, np.float32)

    for c in range(N_CORES):
        colA = colB = colG = 0
        for b in range(NB):
            ra, da, rb_, db_ = percore[c][b]
            va = np.zeros(TA[b] * 128, np.int64)
            va[:len(ra)] = ra
            vb = np.zeros(TB[b] * 128, np.int64)
            vb[:len(rb_)] = rb_
            idxA[c, :, colA * 8:(colA + TA[b]) * 8] = _wrap16(va, TA[b] * 128)
            idxB[c, :, colB * 8:(colB + TB[b]) * 8] = _wrap16(vb, TB[b] * 128)
            lo_sl = np.full(T1[b] * 128, PADV, np.float32)
            lo_sl[:len(da)] = da
            lo_sl[TA[b] * 128:TA[b] * 128 + len(db_)] = db_
            dstl[c, :, colG:colG + T1[b]] = lo_sl.reshape(T1[b], 128).T
            dstf[c, 0, colG * 128:(colG + T1[b]) * 128] = lo_sl
            colA += TA[b]
            colB += TB[b]
            colG += T1[b]
    return (tuple(TA), tuple(TB)), idxA, idxB, dstl.astype(BF), dstf.astype(BF)


def preprocess_edges(edge_index):
    """Partition input edges by destination core/block (self loops handled
    separately on-device) and build both layers' gather index layouts."""
    ei = np.asarray(edge_index).astype(np.int64)
    src, dst = ei[0], ei[1]

    per1, per2 = [], []
    for c in range(N_CORES):
        lo = c * P_NODES
        m = (dst >= lo) & (dst < lo + P_NODES)
        s, d = src[m], dst[m] - lo
        bl1, bl2 = [], []
        for b in range(NB):
            mb = (d // 128) == b
            sb_, db_ = s[mb], d[mb] - b * 128
            # layer 1: table rows = global node id, split at HALF
            am = sb_ < HALF
            bl1.append((sb_[am], db_[am], sb_[~am] - HALF, db_[~am]))
            # layer 2: AllGather layout — rows c*3200+i (i<3200) | c*3072+(i-3200)
            sc, si = np.divmod(sb_, P_NODES)
            am2 = si < SPLIT2
            rowA = sc * SPLIT2 + si
            rowB = sc * (P_NODES - SPLIT2) + (si - SPLIT2)
            bl2.append((rowA[am2], db_[am2], rowB[~am2], db_[~am2]))
        per1.append(bl1)
        per2.append(bl2)
    return _layer_pack(per1), _layer_pack(per2)


def prep_weights(x, W1, att_src1, att_dst1, W2, att_src2, att_dst2):
    x = np.asarray(x, np.float32)
    W1 = np.asarray(W1, np.float32)
    W2 = np.asarray(W2, np.float32)
    As1 = np.einsum("khc,hc->kh", W1.reshape(D_IN, H1, 64), np.asarray(att_src1, np.float32))
    Ad1 = np.einsum("khc,hc->kh", W1.reshape(D_IN, H1, 64), np.asarray(att_dst1, np.float32))
    As2 = W2 @ np.asarray(att_src2, np.float32).reshape(64)
    Ad2 = W2 @ np.asarray(att_dst2, np.float32).reshape(64)
    W1ext = np.zeros((D_IN, ROW1), np.float32)
    W1ext[:, :256] = W1
    W1ext[:, 256:260] = As1
    W1ext[:, 260:264] = Ad1
    W2ext = np.zeros((D_IN, ROW2), np.float32)
    W2ext[:, :64] = W2
    W2ext[:, 64] = As2
    W2ext[:, 65] = Ad2
    xT = np.zeros((D_IN, NPAD), np.float32)
    xT[:, :N_NODES] = x.T
    return xT.astype(BF), W1ext.astype(BF), W2ext.astype(BF)


# ------------------------------------------------------------- bass program

def build_program(key1, key2):
    TA1, TB1 = key1
    TA2, TB2 = key2
    T11 = [a + b for a, b in zip(TA1, TB1)]
    T12 = [a + b for a, b in zip(TA2, TB2)]
    sumA1, sumB1, sumT1 = sum(TA1), sum(TB1), sum(T11)
    sumA2, sumB2, sumT2 = sum(TA2), sum(TB2), sum(T12)
    T1MAX = max(max(T11), max(T12))
    GBMAX = {ROW1: max(T11) + 1, ROW2: max(T12) + 1}
    nc = bacc.Bacc("TRN2", target_bir_lowering=False, debug=False,
                   num_devices=N_CORES)

    xT = nc.dram_tensor("xT", [D_IN, NPAD], BF16, kind="ExternalInput").ap()
    xTo = nc.dram_tensor("xT_own", [D_IN, P_NODES], BF16, kind="ExternalInput").ap()
    w1d = nc.dram_tensor("W1ext", [D_IN, ROW1], BF16, kind="ExternalInput").ap()
    w2d = nc.dram_tensor("W2ext", [D_IN, ROW2], BF16, kind="ExternalInput").ap()
    b1d = nc.dram_tensor("b1v", [1, D_IN], F32, kind="ExternalInput").ap()
    b2d = nc.dram_tensor("b2v", [1, 64], F32, kind="ExternalInput").ap()
    iotad = nc.dram_tensor("iota128", [128, 128], BF16, kind="ExternalInput").ap()
    iotacd = nc.dram_tensor("iotacol", [128, 1], F32, kind="ExternalInput").ap()
    identd = nc.dram_tensor("ident128", [128, 128], F32, kind="ExternalInput").ap()
    identbd = nc.dram_tensor("identb", [128, 128], BF16, kind="ExternalInput").ap()
    idxA1d = nc.dram_tensor("idxA1", [128, sumA1 * 8], I16, kind="ExternalInput").ap()
    idxB1d = nc.dram_tensor("idxB1", [128, sumB1 * 8], I16, kind="ExternalInput").ap()
    dstl1d = nc.dram_tensor("dstl1", [128, sumT1], BF16, kind="ExternalInput").ap()
    dstf1d = nc.dram_tensor("dstf1", [1, sumT1 * 128], BF16, kind="ExternalInput").ap()
    idxA2d = nc.dram_tensor("idxA2", [128, sumA2 * 8], I16, kind="ExternalInput").ap()
    idxB2d = nc.dram_tensor("idxB2", [128, sumB2 * 8], I16, kind="ExternalInput").ap()
    dstl2d = nc.dram_tensor("dstl2", [128, sumT2], BF16, kind="ExternalInput").ap()
    dstf2d = nc.dram_tensor("dstf2", [1, sumT2 * 128], BF16, kind="ExternalInput").ap()
    y = nc.dram_tensor("y", [P_NODES, 64], F32, kind="ExternalOutput").ap()

    AL = mybir.AluOpType
    ACT = mybir.ActivationFunctionType

    with tile.TileContext(nc) as tc:
        with (
            tc.tile_pool(name="const", bufs=1) as cp,
            tc.tile_pool(name="sb", bufs=2) as sb,
            tc.tile_pool(name="psum", bufs=2, space="PSUM") as pp,
            tc.tile_pool(name="dram", bufs=1, space="DRAM") as dram,
        ):
            nc.gpsimd.load_library(library_config.mlp)

            tableA = dram.tile([HALF, ROW1], BF16)
            tableB = dram.tile([NPAD - HALF, ROW1], BF16)
            h2own = dram.tile([P_NODES, ROW2], BF16)
            table2a = dram.tile([N_CORES * SPLIT2, ROW2], BF16, addr_space="Shared")
            table2b = dram.tile([NPAD - N_CORES * SPLIT2, ROW2], BF16, addr_space="Shared")

            # ---------------- constants
            w1e = cp.tile([128, 2, ROW1], BF16)
            nc.sync.dma_start(out=w1e[:, 0], in_=w1d[0:128, :])
            nc.sync.dma_start(out=w1e[:, 1], in_=w1d[128:256, :])
            w2e = cp.tile([128, 2, ROW2], BF16)
            nc.sync.dma_start(out=w2e[:, 0], in_=w2d[0:128, :])
            nc.sync.dma_start(out=w2e[:, 1], in_=w2d[128:256, :])
            iota_f = cp.tile([128, 128], BF16)
            nc.sync.dma_start(out=iota_f[:], in_=iotad[:])
            iota_c = cp.tile([128, 1], F32)
            nc.sync.dma_start(out=iota_c[:], in_=iotacd[:])
            ident = cp.tile([128, 128], F32)
            nc.sync.dma_start(out=ident[:], in_=identd[:])
            identb = cp.tile([128, 128], BF16)
            nc.sync.dma_start(out=identb[:], in_=identbd[:])

            b1row = cp.tile([1, D_IN], F32)
            nc.sync.dma_start(out=b1row[:], in_=b1d[:])
            b2row = cp.tile([1, 64], F32)
            nc.sync.dma_start(out=b2row[:], in_=b2d[:])
            ones1 = cp.tile([1, 128], F32)
            nc.vector.memset(ones1[:], 1.0)
            ones1b = cp.tile([1, 128], BF16)
            nc.vector.memset(ones1b[:], 1.0)
            bias_ps = pp.tile([128, D_IN], F32, tag="ps_tab")
            nc.tensor.matmul(bias_ps[:], lhsT=ones1[:], rhs=b1row[:], start=True, stop=True)
            bias1 = cp.tile([128, D_IN], F32)
            nc.vector.tensor_copy(out=bias1[:], in_=bias_ps[:])
            bias_ps2 = pp.tile([128, 64], F32, tag="ps_tab")
            nc.tensor.matmul(bias_ps2[:], lhsT=ones1[:], rhs=b2row[:], start=True, stop=True)
            bias2 = cp.tile([128, 64], F32)
            nc.vector.tensor_copy(out=bias2[:], in_=bias_ps2[:])

            idxA1 = cp.tile([128, sumA1 * 8], I16)
            nc.sync.dma_start(out=idxA1[:], in_=idxA1d[:])
            idxB1 = cp.tile([128, sumB1 * 8], I16)
            nc.sync.dma_start(out=idxB1[:], in_=idxB1d[:])
            dstl1 = cp.tile([128, sumT1], BF16)
            nc.sync.dma_start(out=dstl1[:], in_=dstl1d[:])
            idxA2 = cp.tile([128, sumA2 * 8], I16)
            nc.sync.dma_start(out=idxA2[:], in_=idxA2d[:])
            idxB2 = cp.tile([128, sumB2 * 8], I16)
            nc.sync.dma_start(out=idxB2[:], in_=idxB2d[:])
            dstl2 = cp.tile([128, sumT2], BF16)
            nc.sync.dma_start(out=dstl2[:], in_=dstl2d[:])

            adtab1 = cp.tile([128, NB * H1], BF16)
            adtab2 = cp.tile([128, NB], BF16)
            own1 = cp.tile([128, NB, 260], BF16)   # own-node layer-1 rows (self tiles)
            own2 = cp.tile([128, NB, 65], BF16)    # own-node layer-2 rows

            # ---------------- phase 0: own-node rows + adtab1 (from xTo)
            NBH = 25
            for hbase in (0, NBH):
                nb_h = min(NBH, NB - hbase)
                xo = sb.tile([128, 2, NBH * 128], BF16, tag="xo", bufs=1)
                nc.sync.dma_start(out=xo[:, 0, 0:nb_h * 128],
                                  in_=xTo[0:128, hbase * 128:(hbase + nb_h) * 128])
                nc.sync.dma_start(out=xo[:, 1, 0:nb_h * 128],
                                  in_=xTo[128:256, hbase * 128:(hbase + nb_h) * 128])
                for j in range(nb_h):
                    b = hbase + j
                    pso = pp.tile([128, ROW1], F32, tag="ps_tab")
                    nc.tensor.matmul(pso[:], lhsT=xo[:, 0, j * 128:(j + 1) * 128],
                                     rhs=w1e[:, 0], start=True, stop=False)
                    nc.tensor.matmul(pso[:], lhsT=xo[:, 1, j * 128:(j + 1) * 128],
                                     rhs=w1e[:, 1], start=False, stop=True)
                    nc.scalar.copy(out=own1[:, b], in_=pso[:, 0:260])
                    nc.vector.tensor_copy(out=adtab1[:, b * H1:(b + 1) * H1], in_=pso[:, 260:264])

            # ---------------- phase 1: layer-1 node table (replicated build)
            for ch in range(TBLK // BDIM):
                c0 = ch * BDIM * 128
                xt = sb.tile([128, 2, BDIM * 128], BF16, tag="xt")
                nc.sync.dma_start(out=xt[:, 0], in_=xT[0:128, c0:c0 + BDIM * 128])
                nc.sync.dma_start(out=xt[:, 1], in_=xT[128:256, c0:c0 + BDIM * 128])
                ev = sb.tile([128, BDIM, ROW1], BF16, tag="ev")
                for j in range(BDIM):
                    ps = pp.tile([128, ROW1], F32, tag="ps_tab")
                    nc.tensor.matmul(ps[:], lhsT=xt[:, 0, j * 128:(j + 1) * 128],
                                     rhs=w1e[:, 0], start=True, stop=False)
                    nc.tensor.matmul(ps[:], lhsT=xt[:, 1, j * 128:(j + 1) * 128],
                                     rhs=w1e[:, 1], start=False, stop=True)
                    nc.scalar.copy(out=ev[:, j], in_=ps[:])
                # batched, fully-contiguous table write (row j*128+p <-> [p, j, :])
                jsplit = min(BDIM, max(0, (HALF - c0) // 128))
                for tab, rb, j0, j1 in (
                    (tableA, c0, 0, jsplit),
                    (tableB, c0 + jsplit * 128 - HALF, jsplit, BDIM),
                ):
                    if j1 <= j0:
                        continue
                    nj = j1 - j0
                    view = tab[rb:rb + nj * 128, :].rearrange("(j p) f -> p j f", p=128)
                    nc.sync.dma_start(out=view, in_=ev[:, j0:j1])

            # -------- software-pipelined edge phase (per destination block)
            def stage1(b, lay):
                """Issue gathers + build ohT for block b; returns live tiles."""
                (TAl, TBl, T1l, colA, colB, colG,
                 idxA_t, idxB_t, dstl_t, dstfd_ap, tabA, tabB,
                 row, cols, nh, t_off, adt, ownt, fin) = lay
                nt = T1l[b]
                gb = sb.tile([128, GBMAX[row], row], BF16, tag=f"gbuf{row}",
                             bufs=4 if row == ROW2 else 3)

                def chunked(out_v, tab_ap, idx_t, col0, ntiles):
                    t0 = 0
                    while t0 < ntiles:
                        ct = min(GCHUNK, ntiles - t0)
                        nc.gpsimd.dma_gather(
                            out_v[:, t0:t0 + ct, :], tab_ap,
                            idx_t[:, (col0 + t0) * 8:(col0 + t0 + ct) * 8],
                            ct * 128, ct * 128, row)
                        t0 += ct

                chunked(gb[:, 0:TAl[b]], tabA, idxA_t, colA[b], TAl[b])
                chunked(gb[:, TAl[b]:nt], tabB, idxB_t, colB[b], TBl[b])
                nc.scalar.copy(out=gb[:, nt, 0:cols], in_=ownt[:, b, 0:cols])

                # dst ids of this block's edge slots, replicated to all
                # partitions via a K=1 outer product; is_equal vs iota gives
                # the m-major one-hot ohT.
                dstf_t = sb.tile([1, T1MAX * 128], BF16, tag="dstf", bufs=3)
                nc.sync.dma_start(out=dstf_t[:, 0:nt * 128],
                                  in_=dstfd_ap[:, colG[b] * 128:(colG[b] + nt) * 128])
                ohT = sb.tile([128, T1MAX * 128], BF16, tag="ohT", bufs=2)
                c0 = 0
                while c0 < nt:
                    cc = min(4, nt - c0)
                    ps_rep = pp.tile([128, 512], F32, tag="ps_rep")
                    nc.tensor.matmul(ps_rep[:, 0:cc * 128], lhsT=ones1b[:],
                                     rhs=dstf_t[:, c0 * 128:(c0 + cc) * 128],
                                     start=True, stop=True)
                    nc.vector.tensor_scalar(
                        out=ohT[:, c0 * 128:(c0 + cc) * 128],
                        in0=ps_rep[:, 0:cc * 128], scalar1=iota_c[:],
                        scalar2=None, op0=AL.is_equal)
                    c0 += cc
                return gb, ohT

            def stage2(b, lay, gb, ohT):
                """Attention + aggregation + finish for block b."""
                (TAl, TBl, T1l, colA, colB, colG,
                 idxA_t, idxB_t, dstl_t, dstfd_ap, tabA, tabB,
                 row, cols, nh, t_off, adt, ownt, fin) = lay
                nt = T1l[b]
                adtab_s = adt(b)

                # ad[dst] per edge: ohT^T @ adblk, one matmul per tile
                # (slots at stride 4 so PSUM column offsets stay 16B-aligned)
                ps_ad = pp.tile([128, T1MAX * 4], F32, tag="ps_ad", bufs=1)
                for t in range(nt):
                    nc.tensor.matmul(ps_ad[:, t * 4:t * 4 + nh],
                                     lhsT=ohT[:, t * 128:(t + 1) * 128],
                                     rhs=adtab_s, start=True, stop=True)
                ps_adv = ps_ad[:, 0:nt * 4].rearrange("p (t q) -> p t q", q=4)

                # t = exp(leaky_relu(as[src] + ad[dst])), self tile last
                as_f = sb.tile([128, (T1MAX + 1) * 4], F32, tag="asf")
                as_fv = as_f[:, 0:(nt + 1) * nh].rearrange("p (t h) -> p t h", h=nh)
                nc.scalar.copy(out=as_fv, in_=gb[:, 0:nt + 1, t_off:t_off + nh])
                e0 = sb.tile([128, (T1MAX + 1) * 4], F32, tag="e0")
                e0v = e0[:, 0:(nt + 1) * nh].rearrange("p (t h) -> p t h", h=nh)
                nc.vector.tensor_tensor(out=e0v[:, 0:nt], in0=as_fv[:, 0:nt],
                                        in1=ps_adv[:, :, 0:nh], op=AL.add)
                nc.vector.tensor_tensor(out=e0v[:, nt], in0=as_fv[:, nt],
                                        in1=adtab_s, op=AL.add)
                e1 = sb.tile([128, (T1MAX + 1) * 4], F32, tag="e1")
                nc.vector.scalar_tensor_tensor(
                    out=e1[:, 0:(nt + 1) * nh], in0=e0[:, 0:(nt + 1) * nh],
                    scalar=NEG, in1=e0[:, 0:(nt + 1) * nh], op0=AL.mult, op1=AL.max)
                tbf = sb.tile([128, (T1MAX + 1) * 4], BF16, tag="tbf")
                nc.scalar.activation(tbf[:, 0:(nt + 1) * nh], e1[:, 0:(nt + 1) * nh], ACT.Exp)
                tv = tbf[:, 0:(nt + 1) * nh].rearrange("p (t h) -> p t h", h=nh)
                # messages in place; t into the as slot (denominator column)
                for h in range(nh):
                    msgv = gb[:, 0:nt + 1, h * 64:(h + 1) * 64]
                    nc.vector.tensor_tensor(
                        out=msgv, in0=msgv,
                        in1=tv[:, :, h][:, :, None].to_broadcast([128, nt + 1, 64]),
                        op=AL.mult)
                nc.scalar.copy(out=gb[:, 0:nt + 1, t_off:t_off + nh], in_=tv)

                # edge-major one-hot for the aggregation matmul
                oh = sb.tile([128, T1MAX * 128], BF16, tag="oh", bufs=2)
                nc.vector.tensor_tensor(
                    out=oh[:, 0:nt * 128].rearrange("p (t m) -> p t m", m=128),
                    in0=dstl_t[:, colG[b]:colG[b] + nt][:, :, None].to_broadcast([128, nt, 128]),
                    in1=iota_f[:, None, :].to_broadcast([128, nt, 128]),
                    op=AL.is_equal)

                ps_agg = pp.tile([128, cols], F32, tag="ps_agg")
                for t in range(nt):
                    nc.tensor.matmul(ps_agg[:],
                                     lhsT=oh[:, t * 128:(t + 1) * 128],
                                     rhs=gb[:, t, 0:cols],
                                     start=(t == 0), stop=False)
                nc.tensor.matmul(ps_agg[:], lhsT=identb[:], rhs=gb[:, nt, 0:cols],
                                 start=False, stop=True)
                fin(ps_agg, b)

            def run_blocks(lay, after_block=None):
                prev = None
                for b in range(NB):
                    cur = stage1(b, lay)
                    if prev is not None:
                        stage2(b - 1, lay, *prev)
                        if after_block is not None:
                            after_block(b - 1)
                    prev = cur
                stage2(NB - 1, lay, *prev)
                if after_block is not None:
                    after_block(NB - 1)

            # ---------------- phase 3: layer-1 edges; fused layer-2 table
            def fin1(ps_agg, b):
                den = sb.tile([128, H1], F32, tag="den")
                nc.vector.tensor_scalar_add(den[:], ps_agg[:, 256:260], EPS)
                rec = sb.tile([128, H1], F32, tag="rec")
                nc.vector.reciprocal(rec[:], den[:])
                o1 = sb.tile([128, D_IN], F32, tag="o1")
                o1v = o1.rearrange("p (h c) -> p h c", c=64)
                nc.vector.tensor_tensor(
                    out=o1v,
                    in0=ps_agg[:, 0:256].rearrange("p (h c) -> p h c", c=64),
                    in1=rec[:, :, None].to_broadcast([128, H1, 64]),
                    op=AL.mult)
                nc.vector.tensor_tensor(out=o1[:], in0=o1[:], in1=bias1[:], op=AL.add)
                eu = sb.tile([128, D_IN], F32, tag="eu")
                nc.vector.tensor_scalar_min(eu[:], o1[:], 0.0)
                nc.scalar.activation(eu[:], eu[:], ACT.Exp)
                nc.scalar.activation(o1[:], o1[:], ACT.Relu)
                nc.vector.scalar_tensor_tensor(out=o1[:], in0=eu[:], scalar=-1.0,
                                               in1=o1[:], op0=AL.add, op1=AL.add)
                # layer-2 table row for this block (h1 must be transposed)
                h1tb = sb.tile([128, 2, 128], BF16, tag="h1tb")
                for hf in range(2):
                    tps = pp.tile([128, 512], F32, tag="ps_rep")
                    nc.tensor.transpose(out=tps[:, 0:128], in_=o1[:, hf * 128:(hf + 1) * 128],
                                        identity=ident[:])
                    nc.vector.tensor_copy(out=h1tb[:, hf], in_=tps[:, 0:128])
                ps2 = pp.tile([128, C2], F32, tag="ps_tab")
                nc.tensor.matmul(ps2[:], lhsT=h1tb[:, 0], rhs=w2e[:, 0, 0:C2],
                                 start=True, stop=False)
                nc.tensor.matmul(ps2[:], lhsT=h1tb[:, 1], rhs=w2e[:, 1, 0:C2],
                                 start=False, stop=True)
                ev2 = sb.tile([128, C2], BF16, tag="ev2")
                nc.scalar.copy(out=ev2[:], in_=ps2[:])
                nc.sync.dma_start(out=h2own[b * 128:(b + 1) * 128, 0:C2], in_=ev2[:])
                nc.scalar.copy(out=own2[:, b], in_=ev2[:, 0:65])
                nc.vector.tensor_copy(out=adtab2[:, b:b + 1], in_=ev2[:, 65:66])

            def cums(tl):
                c, out = 0, []
                for v in tl:
                    out.append(c)
                    c += v
                return out

            lay1 = (TA1, TB1, T11, cums(TA1), cums(TB1), cums(T11),
                    idxA1, idxB1, dstl1, dstf1d, tableA[:, :], tableB[:, :],
                    ROW1, 260, H1, 256,
                    lambda b: adtab1[:, b * H1:(b + 1) * H1], own1, fin1)

            def ag_after(b):
                if b == NBSPLIT - 1:
                    nc.gpsimd.collective_compute(
                        "AllGather", AL.bypass,
                        replica_groups=[list(range(N_CORES))],
                        ins=[h2own[0:SPLIT2, :].opt()], outs=[table2a.opt()])
                elif b == NB - 1:
                    nc.gpsimd.collective_compute(
                        "AllGather", AL.bypass,
                        replica_groups=[list(range(N_CORES))],
                        ins=[h2own[SPLIT2:P_NODES, :].opt()], outs=[table2b.opt()])

            run_blocks(lay1, after_block=ag_after)

            # ---------------- phase 6: layer-2 edges + output
            def fin2(ps_agg, b):
                den = sb.tile([128, 1], F32, tag="den")
                nc.vector.tensor_scalar_add(den[:], ps_agg[:, 64:65], EPS)
                rec = sb.tile([128, 1], F32, tag="rec")
                nc.vector.reciprocal(rec[:], den[:])
                o2 = sb.tile([128, 64], F32, tag="o2s")
                nc.vector.tensor_scalar(out=o2[:], in0=ps_agg[:, 0:64],
                                        scalar1=rec[:], scalar2=None, op0=AL.mult)
                nc.vector.tensor_tensor(out=o2[:], in0=o2[:], in1=bias2[:], op=AL.add)
                nc.sync.dma_start(out=y[b * 128:(b + 1) * 128, :], in_=o2[:])

            lay2 = (TA2, TB2, T12, cums(TA2), cums(TB2), cums(T12),
                    idxA2, idxB2, dstl2, dstf2d, table2a[:, :], table2b[:, :],
                    ROW2, 65, 1, 64,
                    lambda b: adtab2[:, b:b + 1], own2, fin2)
            run_blocks(lay2)

    nc.compile()
    return nc


_CACHE = {}


def _get_program(key):
    if key not in _CACHE:
        _CACHE[key] = build_program(*key)
    return _CACHE[key]


def run(inputs, trace=False, trace_kwargs=None):
    x = np.asarray(inputs["x"], np.float32)
    (key1, idxA1, idxB1, dstl1, dstf1), (key2, idxA2, idxB2, dstl2, dstf2) = \
        preprocess_edges(inputs["edge_index"])
    xT, W1ext, W2ext = prep_weights(
        x, inputs["W1"], inputs["att_src1"], inputs["att_dst1"],
        inputs["W2"], inputs["att_src2"], inputs["att_dst2"])
    b1v = np.asarray(inputs["b1"], np.float32).reshape(1, D_IN)
    b2v = np.asarray(inputs["b2"], np.float32).reshape(1, 64)
    iota = np.tile(np.arange(128, dtype=np.float32), (128, 1)).astype(BF)
    iotac = np.arange(128, dtype=np.float32).reshape(128, 1)
    ident = np.eye(128, dtype=np.float32)

    nc = _get_program((key1, key2))
    in_maps = []
    for c in range(N_CORES):
        in_maps.append({
            "xT": xT, "xT_own": np.ascontiguousarray(xT[:, c * P_NODES:(c + 1) * P_NODES]),
            "W1ext": W1ext, "W2ext": W2ext, "b1v": b1v, "b2v": b2v,
            "iota128": iota, "iotacol": iotac, "ident128": ident,
            "identb": ident.astype(BF),
            "idxA1": idxA1[c], "idxB1": idxB1[c], "dstl1": dstl1[c], "dstf1": dstf1[c],
            "idxA2": idxA2[c], "idxB2": idxB2[c], "dstl2": dstl2[c], "dstf2": dstf2[c],
        })
    res = run_bass_kernel_spmd(nc, in_maps, core_ids=list(range(N_CORES)),
                               trace=trace, **(trace_kwargs or {}))
    out = np.concatenate([res.results[c]["y"] for c in range(N_CORES)], axis=0)
    return np.ascontiguousarray(out[:N_NODES]), res


def kernel(**inputs):
    out, _ = run(inputs, trace=False)
    return out


# revision 17
# speedup vs baseline: 2.2085x; 1.0292x over previous
"""2-layer GAT (PyG GATConv eval semantics) on 8 Trainium2 NeuronCores.

Sharding: nodes by contiguous id range (6272/core, 49 blocks of 128); edges
(with self loops) partitioned by destination core/block so segment softmax and
scatter-add stay local. Per layer a replicated node table (bf16 rows) is
gathered by source id via the GPSIMD dma_gather ucode; ad[dst] is expanded
per edge on the tensor engine (K=1 outer-product of local dst ids + is_equal
against a per-partition iota builds a transposed one-hot ohT[m,e]; a small
matmul ohT^T @ adblk yields ad per edge) instead of a second dma_gather.
Appended self loops are not gathered at all: each destination block gets one
"self tile" whose rows are the block's own table rows (kept in SBUF), whose
aggregation one-hot is the identity and whose ad is the block's adblk
directly. Attention weights t = exp(leaky_relu(as[src]+ad[dst])) scale the
messages in SBUF and the per-destination-block aggregation (numerator +
denominator) is a one-hot matmul accumulated in PSUM. The block loop is
software-pipelined (gathers + ohT of block b+1 issue before the attention/
aggregation of block b) so the strictly-ordered PE/DVE queues never head-of-
line block the next block's gathers. Layer-2's table rows are built per block
right after the layer-1 finish and shared via two bf16 AllGathers: rows of
blocks 0-24 gather mid-way through the layer-1 loop (fully hidden), the rest
at its end; layer-2 gather indices use the matching gathered-row layout.
dma_gather indices are int16, so each layer's table is split in two halves.
Tile counts are per-block maxima over the 8 cores so one SPMD program serves
all cores with minimal padding.
"""

import numpy as np
import ml_dtypes

import concourse.bacc as bacc
import concourse.bass as bass
import concourse.mybir as mybir
import concourse.tile as tile
from concourse import library_config
from concourse.bass_utils import run_bass_kernel_spmd

N_NODES = 50000
N_CORES = 8
P_NODES = 6272                  # nodes per core (49 blocks of 128)
NPAD = P_NODES * N_CORES        # 50176
HALF = NPAD // 2                # 25088 (layer-1 A/B table split, int16-safe)
NB = P_NODES // 128             # 49 destination blocks per core
TBLK = NPAD // 128              # 392 table-build blocks
BDIM = 8                        # table-build blocks per DMA batch
SPLIT2 = 3200                   # per-core row split for layer-2 AllGather (25 blocks)
NBSPLIT = SPLIT2 // 128         # 25
D_IN = 256
H1 = 4
ROW1 = 384                      # bf16 row: [h(256) | as1(4) | ad1(4) | pad], 768B
ROW2 = 128                      # bf16 row: [h2(64) | as2(1) | ad2(1) | pad], 256B
C2 = 66                         # computed cols of a layer-2 table row
NEG = 0.2
EPS = 1e-16
PADV = 960.0                    # pad dst-id sentinel (bf16-exact, != 0..127)
GCHUNK = 8                      # gather tiles per dma_gather call (<=1024 idx)

F32 = mybir.dt.float32
BF16 = mybir.dt.bfloat16
I16 = mybir.dt.int16
BF = ml_dtypes.bfloat16


# ---------------------------------------------------------------- host prep

def _wrap16(vals, n_slots):
    """dma_gather index layout: index j at [j%16, j//16], replicated to all
    eight 16-partition groups."""
    a = np.zeros((16, n_slots // 16), np.int16)
    j = np.arange(len(vals))
    a[j % 16, j // 16] = vals
    return np.tile(a, (8, 1))


def _layer_pack(percore):
    """Pack per-(core, block) A/B edge lists into uniform tiles.

    percore[c][b] = (rowA, dlocA, rowB, dlocB) with rows already mapped into
    the layer's A/B table row spaces. Returns per-block tile counts (maxima
    over cores) and the packed idx/dstl/dstf arrays."""
    TA = [max(max(1, -(-len(percore[c][b][0]) // 128)) for c in range(N_CORES))
          for b in range(NB)]
    TB = [max(max(1, -(-len(percore[c][b][2]) // 128)) for c in range(N_CORES))
          for b in range(NB)]
    T1 = [a + b for a, b in zip(TA, TB)]
    sumA, sumB, sumT = sum(TA), sum(TB), sum(T1)

    idxA = np.zeros((N_CORES, 128, sumA * 8), np.int16)
    idxB = np.zeros((N_CORES, 128, sumB * 8), np.int16)
    dstl = np.full((N_CORES, 128, sumT), PADV, np.float32)
    dstf = np.full((N_CORES, 1, sumT * 128), PADV, np.float32)

    for c in range(N_CORES):
        colA = colB = colG = 0
        for b in range(NB):
            ra, da, rb_, db_ = percore[c][b]
            va = np.zeros(TA[b] * 128, np.int64)
            va[:len(ra)] = ra
            vb = np.zeros(TB[b] * 128, np.int64)
            vb[:len(rb_)] = rb_
            idxA[c, :, colA * 8:(colA + TA[b]) * 8] = _wrap16(va, TA[b] * 128)
            idxB[c, :, colB * 8:(colB + TB[b]) * 8] = _wrap16(vb, TB[b] * 128)
            lo_sl = np.full(T1[b] * 128, PADV, np.float32)
            lo_sl[:len(da)] = da
            lo_sl[TA[b] * 128:TA[b] * 128 + len(db_)] = db_
            dstl[c, :, colG:colG + T1[b]] = lo_sl.reshape(T1[b], 128).T
            dstf[c, 0, colG * 128:(colG + T1[b]) * 128] = lo_sl
            colA += TA[b]
            colB += TB[b]
            colG += T1[b]
    return (tuple(TA), tuple(TB)), idxA, idxB, dstl.astype(BF), dstf.astype(BF)


def preprocess_edges(edge_index):
    """Partition input edges by destination core/block (self loops handled
    separately on-device) and build both layers' gather index layouts."""
    ei = np.asarray(edge_index).astype(np.int64)
    src, dst = ei[0], ei[1]

    per1, per2 = [], []
    for c in range(N_CORES):
        lo = c * P_NODES
        m = (dst >= lo) & (dst < lo + P_NODES)
        s, d = src[m], dst[m] - lo
        bl1, bl2 = [], []
        for b in range(NB):
            mb = (d // 128) == b
            sb_, db_ = s[mb], d[mb] - b * 128
            # layer 1: table rows = global node id, split at HALF
            am = sb_ < HALF
            bl1.append((sb_[am], db_[am], sb_[~am] - HALF, db_[~am]))
            # layer 2: AllGather layout — rows c*3200+i (i<3200) | c*3072+(i-3200)
            sc, si = np.divmod(sb_, P_NODES)
            am2 = si < SPLIT2
            rowA = sc * SPLIT2 + si
            rowB = sc * (P_NODES - SPLIT2) + (si - SPLIT2)
            bl2.append((rowB[~am2], db_[~am2], rowA[am2], db_[am2]))
        per1.append(bl1)
        per2.append(bl2)
    return _layer_pack(per1), _layer_pack(per2)


def prep_weights(x, W1, att_src1, att_dst1, W2, att_src2, att_dst2):
    x = np.asarray(x, np.float32)
    W1 = np.asarray(W1, np.float32)
    W2 = np.asarray(W2, np.float32)
    As1 = np.einsum("khc,hc->kh", W1.reshape(D_IN, H1, 64), np.asarray(att_src1, np.float32))
    Ad1 = np.einsum("khc,hc->kh", W1.reshape(D_IN, H1, 64), np.asarray(att_dst1, np.float32))
    As2 = W2 @ np.asarray(att_src2, np.float32).reshape(64)
    Ad2 = W2 @ np.asarray(att_dst2, np.float32).reshape(64)
    W1ext = np.zeros((D_IN, ROW1), np.float32)
    W1ext[:, :256] = W1
    W1ext[:, 256:260] = As1
    W1ext[:, 260:264] = Ad1
    W2ext = np.zeros((D_IN, ROW2), np.float32)
    W2ext[:, :64] = W2
    W2ext[:, 64] = As2
    W2ext[:, 65] = Ad2
    xT = np.zeros((D_IN, NPAD), np.float32)
    xT[:, :N_NODES] = x.T
    return xT.astype(BF), W1ext.astype(BF), W2ext.astype(BF)


# ------------------------------------------------------------- bass program

def build_program(key1, key2):
    TA1, TB1 = key1
    TA2, TB2 = key2
    T11 = [a + b for a, b in zip(TA1, TB1)]
    T12 = [a + b for a, b in zip(TA2, TB2)]
    sumA1, sumB1, sumT1 = sum(TA1), sum(TB1), sum(T11)
    sumA2, sumB2, sumT2 = sum(TA2), sum(TB2), sum(T12)
    T1MAX = max(max(T11), max(T12))
    GBMAX = {ROW1: max(T11) + 1, ROW2: max(T12) + 1}
    nc = bacc.Bacc("TRN2", target_bir_lowering=False, debug=False,
                   num_devices=N_CORES)

    xT = nc.dram_tensor("xT", [D_IN, NPAD], BF16, kind="ExternalInput").ap()
    xTo = nc.dram_tensor("xT_own", [D_IN, P_NODES], BF16, kind="ExternalInput").ap()
    w1d = nc.dram_tensor("W1ext", [D_IN, ROW1], BF16, kind="ExternalInput").ap()
    w2d = nc.dram_tensor("W2ext", [D_IN, ROW2], BF16, kind="ExternalInput").ap()
    b1d = nc.dram_tensor("b1v", [1, D_IN], F32, kind="ExternalInput").ap()
    b2d = nc.dram_tensor("b2v", [1, 64], F32, kind="ExternalInput").ap()
    iotad = nc.dram_tensor("iota128", [128, 128], BF16, kind="ExternalInput").ap()
    iotacd = nc.dram_tensor("iotacol", [128, 1], F32, kind="ExternalInput").ap()
    identd = nc.dram_tensor("ident128", [128, 128], F32, kind="ExternalInput").ap()
    identbd = nc.dram_tensor("identb", [128, 128], BF16, kind="ExternalInput").ap()
    idxA1d = nc.dram_tensor("idxA1", [128, sumA1 * 8], I16, kind="ExternalInput").ap()
    idxB1d = nc.dram_tensor("idxB1", [128, sumB1 * 8], I16, kind="ExternalInput").ap()
    dstl1d = nc.dram_tensor("dstl1", [128, sumT1], BF16, kind="ExternalInput").ap()
    dstf1d = nc.dram_tensor("dstf1", [1, sumT1 * 128], BF16, kind="ExternalInput").ap()
    idxA2d = nc.dram_tensor("idxA2", [128, sumA2 * 8], I16, kind="ExternalInput").ap()
    idxB2d = nc.dram_tensor("idxB2", [128, sumB2 * 8], I16, kind="ExternalInput").ap()
    dstl2d = nc.dram_tensor("dstl2", [128, sumT2], BF16, kind="ExternalInput").ap()
    dstf2d = nc.dram_tensor("dstf2", [1, sumT2 * 128], BF16, kind="ExternalInput").ap()
    y = nc.dram_tensor("y", [P_NODES, 64], F32, kind="ExternalOutput").ap()

    AL = mybir.AluOpType
    ACT = mybir.ActivationFunctionType

    with tile.TileContext(nc) as tc:
        with (
            tc.tile_pool(name="const", bufs=1) as cp,
            tc.tile_pool(name="sb", bufs=2) as sb,
            tc.tile_pool(name="psum", bufs=2, space="PSUM") as pp,
            tc.tile_pool(name="dram", bufs=1, space="DRAM") as dram,
        ):
            nc.gpsimd.load_library(library_config.mlp)

            tableA = dram.tile([HALF, ROW1], BF16)
            tableB = dram.tile([NPAD - HALF, ROW1], BF16)
            h2own = dram.tile([P_NODES, ROW2], BF16)
            table2a = dram.tile([N_CORES * SPLIT2, ROW2], BF16, addr_space="Shared")
            table2b = dram.tile([NPAD - N_CORES * SPLIT2, ROW2], BF16, addr_space="Shared")

            # ---------------- constants
            w1e = cp.tile([128, 2, ROW1], BF16)
            nc.sync.dma_start(out=w1e[:, 0], in_=w1d[0:128, :])
            nc.sync.dma_start(out=w1e[:, 1], in_=w1d[128:256, :])
            w2e = cp.tile([128, 2, ROW2], BF16)
            nc.sync.dma_start(out=w2e[:, 0], in_=w2d[0:128, :])
            nc.sync.dma_start(out=w2e[:, 1], in_=w2d[128:256, :])
            iota_f = cp.tile([128, 128], BF16)
            nc.sync.dma_start(out=iota_f[:], in_=iotad[:])
            iota_c = cp.tile([128, 1], F32)
            nc.sync.dma_start(out=iota_c[:], in_=iotacd[:])
            ident = cp.tile([128, 128], F32)
            nc.sync.dma_start(out=ident[:], in_=identd[:])
            identb = cp.tile([128, 128], BF16)
            nc.sync.dma_start(out=identb[:], in_=identbd[:])

            b1row = cp.tile([1, D_IN], F32)
            nc.sync.dma_start(out=b1row[:], in_=b1d[:])
            b2row = cp.tile([1, 64], F32)
            nc.sync.dma_start(out=b2row[:], in_=b2d[:])
            ones1 = cp.tile([1, 128], F32)
            nc.vector.memset(ones1[:], 1.0)
            ones1b = cp.tile([1, 128], BF16)
            nc.vector.memset(ones1b[:], 1.0)
            bias_ps = pp.tile([128, D_IN], F32, tag="ps_tab")
            nc.tensor.matmul(bias_ps[:], lhsT=ones1[:], rhs=b1row[:], start=True, stop=True)
            bias1 = cp.tile([128, D_IN], F32)
            nc.vector.tensor_copy(out=bias1[:], in_=bias_ps[:])
            bias_ps2 = pp.tile([128, 64], F32, tag="ps_tab")
            nc.tensor.matmul(bias_ps2[:], lhsT=ones1[:], rhs=b2row[:], start=True, stop=True)
            bias2 = cp.tile([128, 64], F32)
            nc.vector.tensor_copy(out=bias2[:], in_=bias_ps2[:])

            idxA1 = cp.tile([128, sumA1 * 8], I16)
            nc.sync.dma_start(out=idxA1[:], in_=idxA1d[:])
            idxB1 = cp.tile([128, sumB1 * 8], I16)
            nc.sync.dma_start(out=idxB1[:], in_=idxB1d[:])
            dstl1 = cp.tile([128, sumT1], BF16)
            nc.sync.dma_start(out=dstl1[:], in_=dstl1d[:])
            idxA2 = cp.tile([128, sumA2 * 8], I16)
            nc.sync.dma_start(out=idxA2[:], in_=idxA2d[:])
            idxB2 = cp.tile([128, sumB2 * 8], I16)
            nc.sync.dma_start(out=idxB2[:], in_=idxB2d[:])
            dstl2 = cp.tile([128, sumT2], BF16)
            nc.sync.dma_start(out=dstl2[:], in_=dstl2d[:])

            adtab1 = cp.tile([128, NB * H1], BF16)
            adtab2 = cp.tile([128, NB], BF16)
            own1d = dram.tile([P_NODES, 260], BF16)  # own-node layer-1 rows
            own2 = cp.tile([128, NB, 65], BF16)      # own-node layer-2 rows

            # ---------------- phase 0: own-node rows + adtab1 (from xTo)
            NBH = 25
            for hbase in (0, NBH):
                nb_h = min(NBH, NB - hbase)
                xo = sb.tile([128, 2, NBH * 128], BF16, tag="xo", bufs=1)
                nc.sync.dma_start(out=xo[:, 0, 0:nb_h * 128],
                                  in_=xTo[0:128, hbase * 128:(hbase + nb_h) * 128])
                nc.sync.dma_start(out=xo[:, 1, 0:nb_h * 128],
                                  in_=xTo[128:256, hbase * 128:(hbase + nb_h) * 128])
                for j in range(nb_h):
                    b = hbase + j
                    pso = pp.tile([128, ROW1], F32, tag="ps_tab")
                    nc.tensor.matmul(pso[:], lhsT=xo[:, 0, j * 128:(j + 1) * 128],
                                     rhs=w1e[:, 0], start=True, stop=False)
                    nc.tensor.matmul(pso[:], lhsT=xo[:, 1, j * 128:(j + 1) * 128],
                                     rhs=w1e[:, 1], start=False, stop=True)
                    evo = sb.tile([128, 260], BF16, tag="evo")
                    nc.scalar.copy(out=evo[:], in_=pso[:, 0:260])
                    nc.scalar.dma_start(out=own1d[b * 128:(b + 1) * 128, :], in_=evo[:])
                    nc.vector.tensor_copy(out=adtab1[:, b * H1:(b + 1) * H1], in_=pso[:, 260:264])

            # ---------------- phase 1: layer-1 node table (replicated build)
            for ch in range(TBLK // BDIM):
                c0 = ch * BDIM * 128
                xt = sb.tile([128, 2, BDIM * 128], BF16, tag="xt")
                nc.sync.dma_start(out=xt[:, 0], in_=xT[0:128, c0:c0 + BDIM * 128])
                nc.sync.dma_start(out=xt[:, 1], in_=xT[128:256, c0:c0 + BDIM * 128])
                ev = sb.tile([128, BDIM, ROW1], BF16, tag="ev")
                for j in range(BDIM):
                    ps = pp.tile([128, ROW1], F32, tag="ps_tab")
                    nc.tensor.matmul(ps[:], lhsT=xt[:, 0, j * 128:(j + 1) * 128],
                                     rhs=w1e[:, 0], start=True, stop=False)
                    nc.tensor.matmul(ps[:], lhsT=xt[:, 1, j * 128:(j + 1) * 128],
                                     rhs=w1e[:, 1], start=False, stop=True)
                    nc.scalar.copy(out=ev[:, j], in_=ps[:])
                # batched, fully-contiguous table write (row j*128+p <-> [p, j, :])
                jsplit = min(BDIM, max(0, (HALF - c0) // 128))
                for tab, rb, j0, j1 in (
                    (tableA, c0, 0, jsplit),
                    (tableB, c0 + jsplit * 128 - HALF, jsplit, BDIM),
                ):
                    if j1 <= j0:
                        continue
                    nj = j1 - j0
                    view = tab[rb:rb + nj * 128, :].rearrange("(j p) f -> p j f", p=128)
                    nc.scalar.dma_start(out=view, in_=ev[:, j0:j1])

            # -------- software-pipelined edge phase (per destination block)
            def stage1(b, lay):
                """Issue gathers + build ohT for block b; returns live tiles."""
                (TAl, TBl, T1l, colA, colB, colG,
                 idxA_t, idxB_t, dstl_t, dstfd_ap, tabA, tabB,
                 row, cols, nh, t_off, adt, ownt, fin) = lay
                nt = T1l[b]
                gb = sb.tile([128, GBMAX[row], row], BF16, tag=f"gbuf{row}",
                             bufs=4)

                def chunked(out_v, tab_ap, idx_t, col0, ntiles):
                    t0 = 0
                    while t0 < ntiles:
                        ct = min(GCHUNK, ntiles - t0)
                        nc.gpsimd.dma_gather(
                            out_v[:, t0:t0 + ct, :], tab_ap,
                            idx_t[:, (col0 + t0) * 8:(col0 + t0 + ct) * 8],
                            ct * 128, ct * 128, row)
                        t0 += ct

                chunked(gb[:, 0:TAl[b]], tabA, idxA_t, colA[b], TAl[b])
                chunked(gb[:, TAl[b]:nt], tabB, idxB_t, colB[b], TBl[b])
                if row == ROW1:
                    nc.scalar.dma_start(out=gb[:, nt, 0:cols],
                                        in_=own1d[b * 128:(b + 1) * 128, :])
                else:
                    nc.scalar.copy(out=gb[:, nt, 0:cols], in_=ownt[:, b, 0:cols])

                # dst ids of this block's edge slots, replicated to all
                # partitions via a K=1 outer product; is_equal vs iota gives
                # the m-major one-hot ohT.
                dstf_t = sb.tile([1, T1MAX * 128], BF16, tag="dstf", bufs=3)
                nc.sync.dma_start(out=dstf_t[:, 0:nt * 128],
                                  in_=dstfd_ap[:, colG[b] * 128:(colG[b] + nt) * 128])
                ohT = sb.tile([128, T1MAX * 128], BF16, tag="ohT", bufs=2)
                c0 = 0
                while c0 < nt:
                    cc = min(4, nt - c0)
                    ps_rep = pp.tile([128, 512], F32, tag="ps_rep")
                    nc.tensor.matmul(ps_rep[:, 0:cc * 128], lhsT=ones1b[:],
                                     rhs=dstf_t[:, c0 * 128:(c0 + cc) * 128],
                                     start=True, stop=True)
                    nc.vector.tensor_scalar(
                        out=ohT[:, c0 * 128:(c0 + cc) * 128],
                        in0=ps_rep[:, 0:cc * 128], scalar1=iota_c[:],
                        scalar2=None, op0=AL.is_equal)
                    c0 += cc
                return gb, ohT

            def stage2(b, lay, gb, ohT):
                """Attention + aggregation + finish for block b."""
                (TAl, TBl, T1l, colA, colB, colG,
                 idxA_t, idxB_t, dstl_t, dstfd_ap, tabA, tabB,
                 row, cols, nh, t_off, adt, ownt, fin) = lay
                nt = T1l[b]
                adtab_s = adt(b)

                # ad[dst] per edge: ohT^T @ adblk, one matmul per tile
                # (slots at stride 4 so PSUM column offsets stay 16B-aligned)
                ps_ad = pp.tile([128, T1MAX * 4], F32, tag="ps_ad", bufs=1)
                for t in range(nt):
                    nc.tensor.matmul(ps_ad[:, t * 4:t * 4 + nh],
                                     lhsT=ohT[:, t * 128:(t + 1) * 128],
                                     rhs=adtab_s, start=True, stop=True)
                ps_adv = ps_ad[:, 0:nt * 4].rearrange("p (t q) -> p t q", q=4)

                # t = exp(leaky_relu(as[src] + ad[dst])), self tile last
                as_f = sb.tile([128, (T1MAX + 1) * 4], F32, tag="asf")
                as_fv = as_f[:, 0:(nt + 1) * nh].rearrange("p (t h) -> p t h", h=nh)
                nc.scalar.copy(out=as_fv, in_=gb[:, 0:nt + 1, t_off:t_off + nh])
                e0 = sb.tile([128, (T1MAX + 1) * 4], F32, tag="e0")
                e0v = e0[:, 0:(nt + 1) * nh].rearrange("p (t h) -> p t h", h=nh)
                nc.vector.tensor_tensor(out=e0v[:, 0:nt], in0=as_fv[:, 0:nt],
                                        in1=ps_adv[:, :, 0:nh], op=AL.add)
                nc.vector.tensor_tensor(out=e0v[:, nt], in0=as_fv[:, nt],
                                        in1=adtab_s, op=AL.add)
                e1 = sb.tile([128, (T1MAX + 1) * 4], F32, tag="e1")
                nc.vector.scalar_tensor_tensor(
                    out=e1[:, 0:(nt + 1) * nh], in0=e0[:, 0:(nt + 1) * nh],
                    scalar=NEG, in1=e0[:, 0:(nt + 1) * nh], op0=AL.mult, op1=AL.max)
                tbf = sb.tile([128, (T1MAX + 1) * 4], BF16, tag="tbf")
                nc.scalar.activation(tbf[:, 0:(nt + 1) * nh], e1[:, 0:(nt + 1) * nh], ACT.Exp)
                tv = tbf[:, 0:(nt + 1) * nh].rearrange("p (t h) -> p t h", h=nh)
                # messages in place; t into the as slot (denominator column)
                for h in range(nh):
                    msgv = gb[:, 0:nt + 1, h * 64:(h + 1) * 64]
                    nc.vector.tensor_tensor(
                        out=msgv, in0=msgv,
                        in1=tv[:, :, h][:, :, None].to_broadcast([128, nt + 1, 64]),
                        op=AL.mult)
                nc.scalar.copy(out=gb[:, 0:nt + 1, t_off:t_off + nh], in_=tv)

                # edge-major one-hot for the aggregation matmul
                oh = sb.tile([128, T1MAX * 128], BF16, tag="oh", bufs=2)
                nc.vector.tensor_tensor(
                    out=oh[:, 0:nt * 128].rearrange("p (t m) -> p t m", m=128),
                    in0=dstl_t[:, colG[b]:colG[b] + nt][:, :, None].to_broadcast([128, nt, 128]),
                    in1=iota_f[:, None, :].to_broadcast([128, nt, 128]),
                    op=AL.is_equal)

                ps_agg = pp.tile([128, cols], F32, tag="ps_agg")
                for t in range(nt):
                    nc.tensor.matmul(ps_agg[:],
                                     lhsT=oh[:, t * 128:(t + 1) * 128],
                                     rhs=gb[:, t, 0:cols],
                                     start=(t == 0), stop=False)
                nc.tensor.matmul(ps_agg[:], lhsT=identb[:], rhs=gb[:, nt, 0:cols],
                                 start=False, stop=True)
                fin(ps_agg, b)

            def run_blocks(lay, order=None, after_block=None):
                order = list(range(NB)) if order is None else order
                prevb = prev = None
                for b in order:
                    cur = stage1(b, lay)
                    if prev is not None:
                        stage2(prevb, lay, *prev)
                        if after_block is not None:
                            after_block(prevb)
                    prevb, prev = b, cur
                stage2(prevb, lay, *prev)
                if after_block is not None:
                    after_block(prevb)

            # ---------------- phase 3: layer-1 edges; fused layer-2 table
            def fin1(ps_agg, b):
                den = sb.tile([128, H1], F32, tag="den")
                nc.vector.tensor_scalar_add(den[:], ps_agg[:, 256:260], EPS)
                rec = sb.tile([128, H1], F32, tag="rec")
                nc.vector.reciprocal(rec[:], den[:])
                o1 = sb.tile([128, D_IN], F32, tag="o1")
                o1v = o1.rearrange("p (h c) -> p h c", c=64)
                nc.vector.tensor_tensor(
                    out=o1v,
                    in0=ps_agg[:, 0:256].rearrange("p (h c) -> p h c", c=64),
                    in1=rec[:, :, None].to_broadcast([128, H1, 64]),
                    op=AL.mult)
                nc.vector.tensor_tensor(out=o1[:], in0=o1[:], in1=bias1[:], op=AL.add)
                eu = sb.tile([128, D_IN], F32, tag="eu")
                nc.vector.tensor_scalar_min(eu[:], o1[:], 0.0)
                nc.scalar.activation(eu[:], eu[:], ACT.Exp)
                nc.scalar.activation(o1[:], o1[:], ACT.Relu)
                nc.vector.scalar_tensor_tensor(out=o1[:], in0=eu[:], scalar=-1.0,
                                               in1=o1[:], op0=AL.add, op1=AL.add)
                # layer-2 table row for this block (h1 must be transposed)
                h1tb = sb.tile([128, 2, 128], BF16, tag="h1tb")
                for hf in range(2):
                    tps = pp.tile([128, 512], F32, tag="ps_rep")
                    nc.tensor.transpose(out=tps[:, 0:128], in_=o1[:, hf * 128:(hf + 1) * 128],
                                        identity=ident[:])
                    nc.vector.tensor_copy(out=h1tb[:, hf], in_=tps[:, 0:128])
                ps2 = pp.tile([128, C2], F32, tag="ps_tab")
                nc.tensor.matmul(ps2[:], lhsT=h1tb[:, 0], rhs=w2e[:, 0, 0:C2],
                                 start=True, stop=False)
                nc.tensor.matmul(ps2[:], lhsT=h1tb[:, 1], rhs=w2e[:, 1, 0:C2],
                                 start=False, stop=True)
                ev2 = sb.tile([128, C2], BF16, tag="ev2")
                nc.scalar.copy(out=ev2[:], in_=ps2[:])
                nc.sync.dma_start(out=h2own[b * 128:(b + 1) * 128, 0:C2], in_=ev2[:])
                nc.scalar.copy(out=own2[:, b], in_=ev2[:, 0:65])
                nc.vector.tensor_copy(out=adtab2[:, b:b + 1], in_=ev2[:, 65:66])

            def cums(tl):
                c, out = 0, []
                for v in tl:
                    out.append(c)
                    c += v
                return out

            lay1 = (TA1, TB1, T11, cums(TA1), cums(TB1), cums(T11),
                    idxA1, idxB1, dstl1, dstf1d, tableA[:, :], tableB[:, :],
                    ROW1, 260, H1, 256,
                    lambda b: adtab1[:, b * H1:(b + 1) * H1], None, fin1)

            def ag_after(b):
                if b == NB - 1:
                    # blocks 25-48 done -> share the second table chunk
                    nc.gpsimd.collective_compute(
                        "AllGather", AL.bypass,
                        replica_groups=[list(range(N_CORES))],
                        ins=[h2own[SPLIT2:P_NODES, :].opt()], outs=[table2b.opt()])
                elif b == NBSPLIT - 1:
                    nc.gpsimd.collective_compute(
                        "AllGather", AL.bypass,
                        replica_groups=[list(range(N_CORES))],
                        ins=[h2own[0:SPLIT2, :].opt()], outs=[table2a.opt()])

            run_blocks(lay1, order=list(range(NBSPLIT, NB)) + list(range(NBSPLIT)),
                       after_block=ag_after)

            # ---------------- phase 6: layer-2 edges + output
            def fin2(ps_agg, b):
                den = sb.tile([128, 1], F32, tag="den")
                nc.vector.tensor_scalar_add(den[:], ps_agg[:, 64:65], EPS)
                rec = sb.tile([128, 1], F32, tag="rec")
                nc.vector.reciprocal(rec[:], den[:])
                o2 = sb.tile([128, 64], F32, tag="o2s")
                nc.vector.tensor_scalar(out=o2[:], in0=ps_agg[:, 0:64],
                                        scalar1=rec[:], scalar2=None, op0=AL.mult)
                nc.vector.tensor_tensor(out=o2[:], in0=o2[:], in1=bias2[:], op=AL.add)
                nc.sync.dma_start(out=y[b * 128:(b + 1) * 128, :], in_=o2[:])

            lay2 = (TA2, TB2, T12, cums(TA2), cums(TB2), cums(T12),
                    idxA2, idxB2, dstl2, dstf2d, table2b[:, :], table2a[:, :],
                    ROW2, 65, 1, 64,
                    lambda b: adtab2[:, b:b + 1], own2, fin2)
            run_blocks(lay2)

    nc.compile()
    return nc


_CACHE = {}


def _get_program(key):
    if key not in _CACHE:
        _CACHE[key] = build_program(*key)
    return _CACHE[key]


def run(inputs, trace=False, trace_kwargs=None):
    x = np.asarray(inputs["x"], np.float32)
    (key1, idxA1, idxB1, dstl1, dstf1), (key2, idxA2, idxB2, dstl2, dstf2) = \
        preprocess_edges(inputs["edge_index"])
    xT, W1ext, W2ext = prep_weights(
        x, inputs["W1"], inputs["att_src1"], inputs["att_dst1"],
        inputs["W2"], inputs["att_src2"], inputs["att_dst2"])
    b1v = np.asarray(inputs["b1"], np.float32).reshape(1, D_IN)
    b2v = np.asarray(inputs["b2"], np.float32).reshape(1, 64)
    iota = np.tile(np.arange(128, dtype=np.float32), (128, 1)).astype(BF)
    iotac = np.arange(128, dtype=np.float32).reshape(128, 1)
    ident = np.eye(128, dtype=np.float32)

    nc = _get_program((key1, key2))
    in_maps = []
    for c in range(N_CORES):
        in_maps.append({
            "xT": xT, "xT_own": np.ascontiguousarray(xT[:, c * P_NODES:(c + 1) * P_NODES]),
            "W1ext": W1ext, "W2ext": W2ext, "b1v": b1v, "b2v": b2v,
            "iota128": iota, "iotacol": iotac, "ident128": ident,
            "identb": ident.astype(BF),
            "idxA1": idxA1[c], "idxB1": idxB1[c], "dstl1": dstl1[c], "dstf1": dstf1[c],
            "idxA2": idxA2[c], "idxB2": idxB2[c], "dstl2": dstl2[c], "dstf2": dstf2[c],
        })
    res = run_bass_kernel_spmd(nc, in_maps, core_ids=list(range(N_CORES)),
                               trace=trace, **(trace_kwargs or {}))
    out = np.concatenate([res.results[c]["y"] for c in range(N_CORES)], axis=0)
    return np.ascontiguousarray(out[:N_NODES]), res


def kernel(**inputs):
    out, _ = run(inputs, trace=False)
    return out


# revision 19
# speedup vs baseline: 2.2786x; 1.0318x over previous
"""2-layer GAT (PyG GATConv eval semantics) on 8 Trainium2 NeuronCores.

Sharding: nodes by contiguous id range (6272/core, 49 blocks of 128); edges
(with self loops) partitioned by destination core/block so segment softmax and
scatter-add stay local. Per layer a replicated node table (bf16 rows) is
gathered by source id via the GPSIMD dma_gather ucode; ad[dst] is expanded
per edge on the tensor engine (K=1 outer-product of local dst ids + is_equal
against a per-partition iota builds a transposed one-hot ohT[m,e]; a small
matmul ohT^T @ adblk yields ad per edge) instead of a second dma_gather.
Appended self loops are not gathered at all: each destination block gets one
"self tile" whose rows are the block's own table rows (kept in SBUF), whose
aggregation one-hot is the identity and whose ad is the block's adblk
directly. Attention weights t = exp(leaky_relu(as[src]+ad[dst])) scale the
messages in SBUF and the per-destination-block aggregation (numerator +
denominator) is a one-hot matmul accumulated in PSUM. The block loop is
software-pipelined (gathers + ohT of block b+1 issue before the attention/
aggregation of block b) so the strictly-ordered PE/DVE queues never head-of-
line block the next block's gathers. Layer-2's table rows are built per block
right after the layer-1 finish and shared via two bf16 AllGathers: rows of
blocks 0-24 gather mid-way through the layer-1 loop (fully hidden), the rest
at its end; layer-2 gather indices use the matching gathered-row layout.
dma_gather indices are int16, so each layer's table is split in two halves.
Tile counts are per-block maxima over the 8 cores so one SPMD program serves
all cores with minimal padding.
"""

import numpy as np
import ml_dtypes

import concourse.bacc as bacc
import concourse.bass as bass
import concourse.mybir as mybir
import concourse.tile as tile
from concourse import library_config
from concourse.bass_utils import run_bass_kernel_spmd

N_NODES = 50000
N_CORES = 8
P_NODES = 6272                  # nodes per core (49 blocks of 128)
NPAD = P_NODES * N_CORES        # 50176
HALF = NPAD // 2                # 25088 (layer-1 A/B table split, int16-safe)
NB = P_NODES // 128             # 49 destination blocks per core
TBLK = NPAD // 128              # 392 table-build blocks
BDIM = 8                        # table-build blocks per DMA batch
SPLIT2 = 2304                   # per-core row split for layer-2 AllGather (18 blocks;
                                # both halves must stay within int16: 8*s<=32767, 8*(6272-s)<=32767)
NBSPLIT = SPLIT2 // 128         # 18
D_IN = 256
H1 = 4
ROW1 = 384                      # bf16 row: [h(256) | as1(4) | ad1(4) | pad], 768B
ROW2 = 128                      # bf16 row: [h2(64) | as2(1) | ad2(1) | pad], 256B
C2 = 66                         # computed cols of a layer-2 table row
NEG = 0.2
EPS = 1e-16
PADV = 960.0                    # pad dst-id sentinel (bf16-exact, != 0..127)
GCHUNK = 8                      # gather tiles per dma_gather call (<=1024 idx)

F32 = mybir.dt.float32
BF16 = mybir.dt.bfloat16
I16 = mybir.dt.int16
BF = ml_dtypes.bfloat16


# ---------------------------------------------------------------- host prep

def _wrap16(vals, n_slots):
    """dma_gather index layout: index j at [j%16, j//16], replicated to all
    eight 16-partition groups."""
    a = np.zeros((16, n_slots // 16), np.int16)
    j = np.arange(len(vals))
    a[j % 16, j // 16] = vals
    return np.tile(a, (8, 1))


def _layer_pack(percore):
    """Pack per-(core, block) A/B edge lists into uniform tiles.

    percore[c][b] = (rowA, dlocA, rowB, dlocB) with rows already mapped into
    the layer's A/B table row spaces. Returns per-block tile counts (maxima
    over cores) and the packed idx/dstl/dstf arrays."""
    TA = [max(max(1, -(-len(percore[c][b][0]) // 128)) for c in range(N_CORES))
          for b in range(NB)]
    TB = [max(max(1, -(-len(percore[c][b][2]) // 128)) for c in range(N_CORES))
          for b in range(NB)]
    T1 = [a + b for a, b in zip(TA, TB)]
    sumA, sumB, sumT = sum(TA), sum(TB), sum(T1)

    idxA = np.zeros((N_CORES, 128, sumA * 8), np.int16)
    idxB = np.zeros((N_CORES, 128, sumB * 8), np.int16)
    dstl = np.full((N_CORES, 128, sumT), PADV, np.float32)
    dstf = np.full((N_CORES, 1, sumT * 128), PADV, np.float32)

    for c in range(N_CORES):
        colA = colB = colG = 0
        for b in range(NB):
            ra, da, rb_, db_ = percore[c][b]
            va = np.zeros(TA[b] * 128, np.int64)
            va[:len(ra)] = ra
            vb = np.zeros(TB[b] * 128, np.int64)
            vb[:len(rb_)] = rb_
            idxA[c, :, colA * 8:(colA + TA[b]) * 8] = _wrap16(va, TA[b] * 128)
            idxB[c, :, colB * 8:(colB + TB[b]) * 8] = _wrap16(vb, TB[b] * 128)
            lo_sl = np.full(T1[b] * 128, PADV, np.float32)
            lo_sl[:len(da)] = da
            lo_sl[TA[b] * 128:TA[b] * 128 + len(db_)] = db_
            dstl[c, :, colG:colG + T1[b]] = lo_sl.reshape(T1[b], 128).T
            dstf[c, 0, colG * 128:(colG + T1[b]) * 128] = lo_sl
            colA += TA[b]
            colB += TB[b]
            colG += T1[b]
    return (tuple(TA), tuple(TB)), idxA, idxB, dstl.astype(BF), dstf.astype(BF)


def preprocess_edges(edge_index):
    """Partition input edges by destination core/block (self loops handled
    separately on-device) and build both layers' gather index layouts."""
    ei = np.asarray(edge_index).astype(np.int64)
    src, dst = ei[0], ei[1]

    per1, per2 = [], []
    for c in range(N_CORES):
        lo = c * P_NODES
        m = (dst >= lo) & (dst < lo + P_NODES)
        s, d = src[m], dst[m] - lo
        bl1, bl2 = [], []
        for b in range(NB):
            mb = (d // 128) == b
            sb_, db_ = s[mb], d[mb] - b * 128
            # layer 1: table rows = global node id, split at HALF
            am = sb_ < HALF
            bl1.append((sb_[am], db_[am], sb_[~am] - HALF, db_[~am]))
            # layer 2: AllGather layout — rows c*3200+i (i<3200) | c*3072+(i-3200)
            sc, si = np.divmod(sb_, P_NODES)
            am2 = si < SPLIT2
            rowA = sc * SPLIT2 + si
            rowB = sc * (P_NODES - SPLIT2) + (si - SPLIT2)
            bl2.append((rowB[~am2], db_[~am2], rowA[am2], db_[am2]))
        per1.append(bl1)
        per2.append(bl2)
    return _layer_pack(per1), _layer_pack(per2)


def prep_weights(x, W1, att_src1, att_dst1, W2, att_src2, att_dst2):
    x = np.asarray(x, np.float32)
    W1 = np.asarray(W1, np.float32)
    W2 = np.asarray(W2, np.float32)
    As1 = np.einsum("khc,hc->kh", W1.reshape(D_IN, H1, 64), np.asarray(att_src1, np.float32))
    Ad1 = np.einsum("khc,hc->kh", W1.reshape(D_IN, H1, 64), np.asarray(att_dst1, np.float32))
    As2 = W2 @ np.asarray(att_src2, np.float32).reshape(64)
    Ad2 = W2 @ np.asarray(att_dst2, np.float32).reshape(64)
    W1ext = np.zeros((D_IN, ROW1), np.float32)
    W1ext[:, :256] = W1
    W1ext[:, 256:260] = As1
    W1ext[:, 260:264] = Ad1
    W2ext = np.zeros((D_IN, ROW2), np.float32)
    W2ext[:, :64] = W2
    W2ext[:, 64] = As2
    W2ext[:, 65] = Ad2
    xT = np.zeros((D_IN, NPAD), np.float32)
    xT[:, :N_NODES] = x.T
    return xT.astype(BF), W1ext.astype(BF), W2ext.astype(BF)


# ------------------------------------------------------------- bass program

def build_program(key1, key2):
    TA1, TB1 = key1
    TA2, TB2 = key2
    T11 = [a + b for a, b in zip(TA1, TB1)]
    T12 = [a + b for a, b in zip(TA2, TB2)]
    sumA1, sumB1, sumT1 = sum(TA1), sum(TB1), sum(T11)
    sumA2, sumB2, sumT2 = sum(TA2), sum(TB2), sum(T12)
    T1MAX = max(max(T11), max(T12))
    GBMAX = {ROW1: max(T11) + 1, ROW2: max(T12) + 1}
    nc = bacc.Bacc("TRN2", target_bir_lowering=False, debug=False,
                   num_devices=N_CORES)

    xT = nc.dram_tensor("xT", [D_IN, NPAD], BF16, kind="ExternalInput").ap()
    xTo = nc.dram_tensor("xT_own", [D_IN, P_NODES], BF16, kind="ExternalInput").ap()
    w1d = nc.dram_tensor("W1ext", [D_IN, ROW1], BF16, kind="ExternalInput").ap()
    w2d = nc.dram_tensor("W2ext", [D_IN, ROW2], BF16, kind="ExternalInput").ap()
    b1d = nc.dram_tensor("b1v", [1, D_IN], F32, kind="ExternalInput").ap()
    b2d = nc.dram_tensor("b2v", [1, 64], F32, kind="ExternalInput").ap()
    iotad = nc.dram_tensor("iota128", [128, 128], BF16, kind="ExternalInput").ap()
    iotacd = nc.dram_tensor("iotacol", [128, 1], F32, kind="ExternalInput").ap()
    identd = nc.dram_tensor("ident128", [128, 128], F32, kind="ExternalInput").ap()
    identbd = nc.dram_tensor("identb", [128, 128], BF16, kind="ExternalInput").ap()
    idxA1d = nc.dram_tensor("idxA1", [128, sumA1 * 8], I16, kind="ExternalInput").ap()
    idxB1d = nc.dram_tensor("idxB1", [128, sumB1 * 8], I16, kind="ExternalInput").ap()
    dstl1d = nc.dram_tensor("dstl1", [128, sumT1], BF16, kind="ExternalInput").ap()
    dstf1d = nc.dram_tensor("dstf1", [1, sumT1 * 128], BF16, kind="ExternalInput").ap()
    idxA2d = nc.dram_tensor("idxA2", [128, sumA2 * 8], I16, kind="ExternalInput").ap()
    idxB2d = nc.dram_tensor("idxB2", [128, sumB2 * 8], I16, kind="ExternalInput").ap()
    dstl2d = nc.dram_tensor("dstl2", [128, sumT2], BF16, kind="ExternalInput").ap()
    dstf2d = nc.dram_tensor("dstf2", [1, sumT2 * 128], BF16, kind="ExternalInput").ap()
    y = nc.dram_tensor("y", [P_NODES, 64], F32, kind="ExternalOutput").ap()

    AL = mybir.AluOpType
    ACT = mybir.ActivationFunctionType

    with tile.TileContext(nc) as tc:
        with (
            tc.tile_pool(name="const", bufs=1) as cp,
            tc.tile_pool(name="sb", bufs=2) as sb,
            tc.tile_pool(name="psum", bufs=2, space="PSUM") as pp,
            tc.tile_pool(name="dram", bufs=1, space="DRAM") as dram,
        ):
            nc.gpsimd.load_library(library_config.mlp)

            tableA = dram.tile([HALF, ROW1], BF16)
            tableB = dram.tile([NPAD - HALF, ROW1], BF16)
            h2own = dram.tile([P_NODES, ROW2], BF16)
            table2a = dram.tile([N_CORES * SPLIT2, ROW2], BF16, addr_space="Shared")
            table2b = dram.tile([NPAD - N_CORES * SPLIT2, ROW2], BF16, addr_space="Shared")

            # ---------------- constants
            w1e = cp.tile([128, 2, ROW1], BF16)
            nc.sync.dma_start(out=w1e[:, 0], in_=w1d[0:128, :])
            nc.sync.dma_start(out=w1e[:, 1], in_=w1d[128:256, :])
            w2e = cp.tile([128, 2, ROW2], BF16)
            nc.sync.dma_start(out=w2e[:, 0], in_=w2d[0:128, :])
            nc.sync.dma_start(out=w2e[:, 1], in_=w2d[128:256, :])
            iota_f = cp.tile([128, 128], BF16)
            nc.sync.dma_start(out=iota_f[:], in_=iotad[:])
            iota_c = cp.tile([128, 1], F32)
            nc.sync.dma_start(out=iota_c[:], in_=iotacd[:])
            ident = cp.tile([128, 128], F32)
            nc.sync.dma_start(out=ident[:], in_=identd[:])
            identb = cp.tile([128, 128], BF16)
            nc.sync.dma_start(out=identb[:], in_=identbd[:])

            b1row = cp.tile([1, D_IN], F32)
            nc.sync.dma_start(out=b1row[:], in_=b1d[:])
            b2row = cp.tile([1, 64], F32)
            nc.sync.dma_start(out=b2row[:], in_=b2d[:])
            ones1 = cp.tile([1, 128], F32)
            nc.vector.memset(ones1[:], 1.0)
            ones1b = cp.tile([1, 128], BF16)
            nc.vector.memset(ones1b[:], 1.0)
            bias_ps = pp.tile([128, D_IN], F32, tag="ps_tab")
            nc.tensor.matmul(bias_ps[:], lhsT=ones1[:], rhs=b1row[:], start=True, stop=True)
            bias1 = cp.tile([128, D_IN], F32)
            nc.vector.tensor_copy(out=bias1[:], in_=bias_ps[:])
            bias_ps2 = pp.tile([128, 64], F32, tag="ps_tab")
            nc.tensor.matmul(bias_ps2[:], lhsT=ones1[:], rhs=b2row[:], start=True, stop=True)
            bias2 = cp.tile([128, 64], F32)
            nc.vector.tensor_copy(out=bias2[:], in_=bias_ps2[:])

            idxA1 = cp.tile([128, sumA1 * 8], I16)
            nc.sync.dma_start(out=idxA1[:], in_=idxA1d[:])
            idxB1 = cp.tile([128, sumB1 * 8], I16)
            nc.sync.dma_start(out=idxB1[:], in_=idxB1d[:])
            dstl1 = cp.tile([128, sumT1], BF16)
            nc.sync.dma_start(out=dstl1[:], in_=dstl1d[:])
            idxA2 = cp.tile([128, sumA2 * 8], I16)
            nc.sync.dma_start(out=idxA2[:], in_=idxA2d[:])
            idxB2 = cp.tile([128, sumB2 * 8], I16)
            nc.sync.dma_start(out=idxB2[:], in_=idxB2d[:])
            dstl2 = cp.tile([128, sumT2], BF16)
            nc.sync.dma_start(out=dstl2[:], in_=dstl2d[:])

            adtab1 = cp.tile([128, NB * H1], BF16)
            adtab2 = cp.tile([128, NB], BF16)
            own1d = dram.tile([P_NODES, 260], BF16)  # own-node layer-1 rows
            own2 = cp.tile([128, NB, 65], BF16)      # own-node layer-2 rows

            # ---------------- phase 0: own-node rows + adtab1 (from xTo)
            NBH = 25
            for hbase in (0, NBH):
                nb_h = min(NBH, NB - hbase)
                xo = sb.tile([128, 2, NBH * 128], BF16, tag="xo", bufs=1)
                nc.sync.dma_start(out=xo[:, 0, 0:nb_h * 128],
                                  in_=xTo[0:128, hbase * 128:(hbase + nb_h) * 128])
                nc.sync.dma_start(out=xo[:, 1, 0:nb_h * 128],
                                  in_=xTo[128:256, hbase * 128:(hbase + nb_h) * 128])
                evo = sb.tile([128, NBH, 260], BF16, tag="evo", bufs=1)
                for j in range(nb_h):
                    b = hbase + j
                    pso = pp.tile([128, ROW1], F32, tag="ps_tab")
                    nc.tensor.matmul(pso[:], lhsT=xo[:, 0, j * 128:(j + 1) * 128],
                                     rhs=w1e[:, 0], start=True, stop=False)
                    nc.tensor.matmul(pso[:], lhsT=xo[:, 1, j * 128:(j + 1) * 128],
                                     rhs=w1e[:, 1], start=False, stop=True)
                    nc.scalar.copy(out=evo[:, j], in_=pso[:, 0:260])
                    nc.vector.tensor_copy(out=adtab1[:, b * H1:(b + 1) * H1], in_=pso[:, 260:264])
                oview = own1d[hbase * 128:(hbase + nb_h) * 128, :].rearrange(
                    "(j p) f -> p j f", p=128)
                nc.scalar.dma_start(out=oview, in_=evo[:, 0:nb_h])

            # ---------------- phase 1: layer-1 node table (replicated build)
            for ch in range(TBLK // BDIM):
                c0 = ch * BDIM * 128
                xt = sb.tile([128, 2, BDIM * 128], BF16, tag="xt")
                nc.sync.dma_start(out=xt[:, 0], in_=xT[0:128, c0:c0 + BDIM * 128])
                nc.sync.dma_start(out=xt[:, 1], in_=xT[128:256, c0:c0 + BDIM * 128])
                ev = sb.tile([128, BDIM, ROW1], BF16, tag="ev")
                for j in range(BDIM):
                    ps = pp.tile([128, ROW1], F32, tag="ps_tab")
                    nc.tensor.matmul(ps[:], lhsT=xt[:, 0, j * 128:(j + 1) * 128],
                                     rhs=w1e[:, 0], start=True, stop=False)
                    nc.tensor.matmul(ps[:], lhsT=xt[:, 1, j * 128:(j + 1) * 128],
                                     rhs=w1e[:, 1], start=False, stop=True)
                    nc.vector.tensor_copy(out=ev[:, j], in_=ps[:])
                # batched, fully-contiguous table write (row j*128+p <-> [p, j, :])
                jsplit = min(BDIM, max(0, (HALF - c0) // 128))
                for tab, rb, j0, j1 in (
                    (tableA, c0, 0, jsplit),
                    (tableB, c0 + jsplit * 128 - HALF, jsplit, BDIM),
                ):
                    if j1 <= j0:
                        continue
                    nj = j1 - j0
                    view = tab[rb:rb + nj * 128, :].rearrange("(j p) f -> p j f", p=128)
                    nc.scalar.dma_start(out=view, in_=ev[:, j0:j1])

            # -------- software-pipelined edge phase (per destination block)
            def stage1(b, lay):
                """Issue gathers + build ohT for block b; returns live tiles."""
                (TAl, TBl, T1l, colA, colB, colG,
                 idxA_t, idxB_t, dstl_t, dstfd_ap, tabA, tabB,
                 row, cols, nh, t_off, adt, ownt, fin) = lay
                nt = T1l[b]
                gb = sb.tile([128, GBMAX[row], row], BF16, tag=f"gbuf{row}",
                             bufs=4)

                def chunked(out_v, tab_ap, idx_t, col0, ntiles):
                    t0 = 0
                    while t0 < ntiles:
                        ct = min(GCHUNK, ntiles - t0)
                        nc.gpsimd.dma_gather(
                            out_v[:, t0:t0 + ct, :], tab_ap,
                            idx_t[:, (col0 + t0) * 8:(col0 + t0 + ct) * 8],
                            ct * 128, ct * 128, row)
                        t0 += ct

                chunked(gb[:, 0:TAl[b]], tabA, idxA_t, colA[b], TAl[b])
                chunked(gb[:, TAl[b]:nt], tabB, idxB_t, colB[b], TBl[b])
                if row == ROW1:
                    nc.scalar.dma_start(out=gb[:, nt, 0:cols],
                                        in_=own1d[b * 128:(b + 1) * 128, :])
                else:
                    nc.scalar.copy(out=gb[:, nt, 0:cols], in_=ownt[:, b, 0:cols])

                # dst ids of this block's edge slots, replicated to all
                # partitions via a K=1 outer product; is_equal vs iota gives
                # the m-major one-hot ohT.
                dstf_t = sb.tile([1, T1MAX * 128], BF16, tag="dstf", bufs=3)
                nc.sync.dma_start(out=dstf_t[:, 0:nt * 128],
                                  in_=dstfd_ap[:, colG[b] * 128:(colG[b] + nt) * 128])
                ohT = sb.tile([128, T1MAX * 128], BF16, tag="ohT", bufs=2)
                c0 = 0
                while c0 < nt:
                    cc = min(4, nt - c0)
                    ps_rep = pp.tile([128, 512], F32, tag="ps_rep")
                    nc.tensor.matmul(ps_rep[:, 0:cc * 128], lhsT=ones1b[:],
                                     rhs=dstf_t[:, c0 * 128:(c0 + cc) * 128],
                                     start=True, stop=True)
                    nc.vector.tensor_scalar(
                        out=ohT[:, c0 * 128:(c0 + cc) * 128],
                        in0=ps_rep[:, 0:cc * 128], scalar1=iota_c[:],
                        scalar2=None, op0=AL.is_equal)
                    c0 += cc
                return gb, ohT

            def stage2(b, lay, gb, ohT):
                """Attention + aggregation + finish for block b."""
                (TAl, TBl, T1l, colA, colB, colG,
                 idxA_t, idxB_t, dstl_t, dstfd_ap, tabA, tabB,
                 row, cols, nh, t_off, adt, ownt, fin) = lay
                nt = T1l[b]
                adtab_s = adt(b)

                # ad[dst] per edge: ohT^T @ adblk, one matmul per tile
                # (slots at stride 4 so PSUM column offsets stay 16B-aligned)
                ps_ad = pp.tile([128, T1MAX * 4], F32, tag="ps_ad", bufs=1)
                for t in range(nt):
                    nc.tensor.matmul(ps_ad[:, t * 4:t * 4 + nh],
                                     lhsT=ohT[:, t * 128:(t + 1) * 128],
                                     rhs=adtab_s, start=True, stop=True)
                ps_adv = ps_ad[:, 0:nt * 4].rearrange("p (t q) -> p t q", q=4)

                # t = exp(leaky_relu(as[src] + ad[dst])), self tile last
                as_f = sb.tile([128, (T1MAX + 1) * 4], F32, tag="asf")
                as_fv = as_f[:, 0:(nt + 1) * nh].rearrange("p (t h) -> p t h", h=nh)
                nc.scalar.copy(out=as_fv, in_=gb[:, 0:nt + 1, t_off:t_off + nh])
                e0 = sb.tile([128, (T1MAX + 1) * 4], F32, tag="e0")
                e0v = e0[:, 0:(nt + 1) * nh].rearrange("p (t h) -> p t h", h=nh)
                nc.vector.tensor_tensor(out=e0v[:, 0:nt], in0=as_fv[:, 0:nt],
                                        in1=ps_adv[:, :, 0:nh], op=AL.add)
                nc.vector.tensor_tensor(out=e0v[:, nt], in0=as_fv[:, nt],
                                        in1=adtab_s, op=AL.add)
                e1 = sb.tile([128, (T1MAX + 1) * 4], F32, tag="e1")
                nc.vector.scalar_tensor_tensor(
                    out=e1[:, 0:(nt + 1) * nh], in0=e0[:, 0:(nt + 1) * nh],
                    scalar=NEG, in1=e0[:, 0:(nt + 1) * nh], op0=AL.mult, op1=AL.max)
                tbf = sb.tile([128, (T1MAX + 1) * 4], BF16, tag="tbf")
                nc.scalar.activation(tbf[:, 0:(nt + 1) * nh], e1[:, 0:(nt + 1) * nh], ACT.Exp)
                tv = tbf[:, 0:(nt + 1) * nh].rearrange("p (t h) -> p t h", h=nh)
                # messages in place; t into the as slot (denominator column)
                for h in range(nh):
                    msgv = gb[:, 0:nt + 1, h * 64:(h + 1) * 64]
                    nc.vector.tensor_tensor(
                        out=msgv, in0=msgv,
                        in1=tv[:, :, h][:, :, None].to_broadcast([128, nt + 1, 64]),
                        op=AL.mult)
                nc.scalar.copy(out=gb[:, 0:nt + 1, t_off:t_off + nh], in_=tv)

                # edge-major one-hot for the aggregation matmul
                oh = sb.tile([128, T1MAX * 128], BF16, tag="oh", bufs=2)
                nc.vector.tensor_tensor(
                    out=oh[:, 0:nt * 128].rearrange("p (t m) -> p t m", m=128),
                    in0=dstl_t[:, colG[b]:colG[b] + nt][:, :, None].to_broadcast([128, nt, 128]),
                    in1=iota_f[:, None, :].to_broadcast([128, nt, 128]),
                    op=AL.is_equal)

                ps_agg = pp.tile([128, cols], F32, tag="ps_agg")
                for t in range(nt):
                    nc.tensor.matmul(ps_agg[:],
                                     lhsT=oh[:, t * 128:(t + 1) * 128],
                                     rhs=gb[:, t, 0:cols],
                                     start=(t == 0), stop=False)
                nc.tensor.matmul(ps_agg[:], lhsT=identb[:], rhs=gb[:, nt, 0:cols],
                                 start=False, stop=True)
                fin(ps_agg, b)

            def run_blocks(lay, order=None, after_block=None):
                order = list(range(NB)) if order is None else order
                prevb = prev = None
                for b in order:
                    cur = stage1(b, lay)
                    if prev is not None:
                        stage2(prevb, lay, *prev)
                        if after_block is not None:
                            after_block(prevb)
                    prevb, prev = b, cur
                stage2(prevb, lay, *prev)
                if after_block is not None:
                    after_block(prevb)

            # ---------------- phase 3: layer-1 edges; fused layer-2 table
            def fin1(ps_agg, b):
                den = sb.tile([128, H1], F32, tag="den")
                nc.vector.tensor_scalar_add(den[:], ps_agg[:, 256:260], EPS)
                rec = sb.tile([128, H1], F32, tag="rec")
                nc.vector.reciprocal(rec[:], den[:])
                o1 = sb.tile([128, D_IN], F32, tag="o1")
                o1v = o1.rearrange("p (h c) -> p h c", c=64)
                nc.vector.tensor_tensor(
                    out=o1v,
                    in0=ps_agg[:, 0:256].rearrange("p (h c) -> p h c", c=64),
                    in1=rec[:, :, None].to_broadcast([128, H1, 64]),
                    op=AL.mult)
                nc.vector.tensor_tensor(out=o1[:], in0=o1[:], in1=bias1[:], op=AL.add)
                eu = sb.tile([128, D_IN], F32, tag="eu")
                nc.vector.tensor_scalar_min(eu[:], o1[:], 0.0)
                nc.scalar.activation(eu[:], eu[:], ACT.Exp)
                nc.scalar.activation(o1[:], o1[:], ACT.Relu)
                nc.vector.scalar_tensor_tensor(out=o1[:], in0=eu[:], scalar=-1.0,
                                               in1=o1[:], op0=AL.add, op1=AL.add)
                # layer-2 table row for this block (h1 must be transposed)
                h1tb = sb.tile([128, 2, 128], BF16, tag="h1tb")
                for hf in range(2):
                    tps = pp.tile([128, 512], F32, tag="ps_rep")
                    nc.tensor.transpose(out=tps[:, 0:128], in_=o1[:, hf * 128:(hf + 1) * 128],
                                        identity=ident[:])
                    nc.vector.tensor_copy(out=h1tb[:, hf], in_=tps[:, 0:128])
                ps2 = pp.tile([128, C2], F32, tag="ps_tab")
                nc.tensor.matmul(ps2[:], lhsT=h1tb[:, 0], rhs=w2e[:, 0, 0:C2],
                                 start=True, stop=False)
                nc.tensor.matmul(ps2[:], lhsT=h1tb[:, 1], rhs=w2e[:, 1, 0:C2],
                                 start=False, stop=True)
                ev2 = sb.tile([128, C2], BF16, tag="ev2")
                nc.scalar.copy(out=ev2[:], in_=ps2[:])
                nc.sync.dma_start(out=h2own[b * 128:(b + 1) * 128, 0:C2], in_=ev2[:])
                nc.scalar.copy(out=own2[:, b], in_=ev2[:, 0:65])
                nc.vector.tensor_copy(out=adtab2[:, b:b + 1], in_=ev2[:, 65:66])

            def cums(tl):
                c, out = 0, []
                for v in tl:
                    out.append(c)
                    c += v
                return out

            lay1 = (TA1, TB1, T11, cums(TA1), cums(TB1), cums(T11),
                    idxA1, idxB1, dstl1, dstf1d, tableA[:, :], tableB[:, :],
                    ROW1, 260, H1, 256,
                    lambda b: adtab1[:, b * H1:(b + 1) * H1], None, fin1)

            def ag_after(b):
                if b == NB - 1:
                    # blocks 25-48 done -> share the second table chunk
                    nc.gpsimd.collective_compute(
                        "AllGather", AL.bypass,
                        replica_groups=[list(range(N_CORES))],
                        ins=[h2own[SPLIT2:P_NODES, :].opt()], outs=[table2b.opt()])
                elif b == NBSPLIT - 1:
                    nc.gpsimd.collective_compute(
                        "AllGather", AL.bypass,
                        replica_groups=[list(range(N_CORES))],
                        ins=[h2own[0:SPLIT2, :].opt()], outs=[table2a.opt()])

            run_blocks(lay1, order=list(range(NBSPLIT, NB)) + list(range(NBSPLIT)),
                       after_block=ag_after)

            # ---------------- phase 6: layer-2 edges + output
            def fin2(ps_agg, b):
                den = sb.tile([128, 1], F32, tag="den")
                nc.vector.tensor_scalar_add(den[:], ps_agg[:, 64:65], EPS)
                rec = sb.tile([128, 1], F32, tag="rec")
                nc.vector.reciprocal(rec[:], den[:])
                o2 = sb.tile([128, 64], F32, tag="o2s")
                nc.vector.tensor_scalar(out=o2[:], in0=ps_agg[:, 0:64],
                                        scalar1=rec[:], scalar2=None, op0=AL.mult)
                nc.vector.tensor_tensor(out=o2[:], in0=o2[:], in1=bias2[:], op=AL.add)
                nc.sync.dma_start(out=y[b * 128:(b + 1) * 128, :], in_=o2[:])

            lay2 = (TA2, TB2, T12, cums(TA2), cums(TB2), cums(T12),
                    idxA2, idxB2, dstl2, dstf2d, table2b[:, :], table2a[:, :],
                    ROW2, 65, 1, 64,
                    lambda b: adtab2[:, b:b + 1], own2, fin2)
            run_blocks(lay2)

    nc.compile()
    return nc


_CACHE = {}


def _get_program(key):
    if key not in _CACHE:
        _CACHE[key] = build_program(*key)
    return _CACHE[key]


def run(inputs, trace=False, trace_kwargs=None):
    x = np.asarray(inputs["x"], np.float32)
    (key1, idxA1, idxB1, dstl1, dstf1), (key2, idxA2, idxB2, dstl2, dstf2) = \
        preprocess_edges(inputs["edge_index"])
    xT, W1ext, W2ext = prep_weights(
        x, inputs["W1"], inputs["att_src1"], inputs["att_dst1"],
        inputs["W2"], inputs["att_src2"], inputs["att_dst2"])
    b1v = np.asarray(inputs["b1"], np.float32).reshape(1, D_IN)
    b2v = np.asarray(inputs["b2"], np.float32).reshape(1, 64)
    iota = np.tile(np.arange(128, dtype=np.float32), (128, 1)).astype(BF)
    iotac = np.arange(128, dtype=np.float32).reshape(128, 1)
    ident = np.eye(128, dtype=np.float32)

    nc = _get_program((key1, key2))
    in_maps = []
    for c in range(N_CORES):
        in_maps.append({
            "xT": xT, "xT_own": np.ascontiguousarray(xT[:, c * P_NODES:(c + 1) * P_NODES]),
            "W1ext": W1ext, "W2ext": W2ext, "b1v": b1v, "b2v": b2v,
            "iota128": iota, "iotacol": iotac, "ident128": ident,
            "identb": ident.astype(BF),
            "idxA1": idxA1[c], "idxB1": idxB1[c], "dstl1": dstl1[c], "dstf1": dstf1[c],
            "idxA2": idxA2[c], "idxB2": idxB2[c], "dstl2": dstl2[c], "dstf2": dstf2[c],
        })
    res = run_bass_kernel_spmd(nc, in_maps, core_ids=list(range(N_CORES)),
                               trace=trace, **(trace_kwargs or {}))
    out = np.concatenate([res.results[c]["y"] for c in range(N_CORES)], axis=0)
    return np.ascontiguousarray(out[:N_NODES]), res


def kernel(**inputs):
    out, _ = run(inputs, trace=False)
    return out
